# revision 9
# baseline (speedup 1.0000x reference)
"""Trainium2 Bass kernel for the CPN/WCP loss (ce + Sinkhorn wcp).

Strategy:
  - M = 2048 Sinkhorn problems sharded 256/core over 8 cores.
  - Per core: compute its 64-row slab of the NxN (-eudis)/2 matrix via PE
    matmuls (rank-1 matmul folds in the -0.5*sq_j term; the per-row sq_i
    shift is dropped -- softmax/log-softmax are shift invariant).
  - CE pieces (row LSE at temp 5, target logit) computed in row layout.
  - Softmax p1 computed in row layout, transposed to [128 class, 256 prob]
    via PE transposes.
  - Sinkhorn runs in multiplicative form: a = p1 / (K@b), b = p2 / (K^T@a)
    with K = exp(-2*cost) fixed => two matmuls + DVE approx-reciprocals per
    iteration, no transcendentals in the loop.
  - wcp_m = ((K.C)^T a) . b ; per-partition partials DMA'd out, host sums.
"""

import sys

for _p in ("/opt/trn_rl_repo",):
    if _p not in sys.path:
        sys.path.insert(0, _p)

import numpy as np

AUG = 4
B = 128
D = 512
N = AUG * B          # 512 feature rows
NCORES = 8
RPC = N // NCORES    # 64 eudis rows per core
MPC = RPC * AUG      # 256 sinkhorn problems per core
M_TOT = N * AUG      # 2048
TEMP = 5.0
GAMMA = 0.2
SINK_ITR = 5
SCALE1 = 2.0 / float(np.sqrt(np.float32(D)))  # softmax scale on h
SCALE5 = 2.0 / TEMP                            # CE scale on h
LN128 = float(np.log(128.0))

_CACHE = {}


def _build_nc(stage=99):
    import concourse.bacc as bacc
    import concourse.tile as tile
    import concourse.mybir as mybir

    dt = mybir.dt.float32
    fp = mybir.ActivationFunctionType
    alu = mybir.AluOpType
    ax = mybir.AxisListType

    nc = bacc.Bacc(
        "TRN2",
        target_bir_lowering=False,
        debug=False,
        enable_asserts=False,
        num_devices=NCORES,
    )

    feat = nc.dram_tensor("features", [N, D], dt, kind="ExternalInput").ap()
    fsl = nc.dram_tensor("fslice", [RPC, D], dt, kind="ExternalInput").ap()
    mce = nc.dram_tensor("maskce", [RPC, B], dt, kind="ExternalInput").ap()
    idn = nc.dram_tensor("ident", [128, 128], dt, kind="ExternalInput").ap()
    onr = nc.dram_tensor("onesr", [1, 128], dt, kind="ExternalInput").ap()
    outd = nc.dram_tensor("out", [128, 2], dt, kind="ExternalOutput").ap()

    with tile.TileContext(nc) as tc:
        with (
            tc.tile_pool(name="sb", bufs=1) as sb,
            tc.tile_pool(name="scr", bufs=2) as scr,
            tc.tile_pool(name="ps_big", bufs=2, space="PSUM") as psb,
            tc.tile_pool(name="ps_t", bufs=3, space="PSUM") as pst,
            tc.tile_pool(name="ps_h", bufs=1, space="PSUM") as psh,
        ):
            dbg = None  # [*,1] tile flushed to out col0 for stage bisection

            # ---------------- loads ----------------
            F = []
            for t in range(4):
                Ft = sb.tile([128, D], dt, tag=f"F{t}", name=f"F{t}")
                nc.sync.dma_start(out=Ft[:], in_=feat[t * 128:(t + 1) * 128, :])
                F.append(Ft)
            fs = sb.tile([RPC, D], dt, tag="fs", name="fs")
            nc.sync.dma_start(out=fs[:], in_=fsl[:])
            mk = sb.tile([RPC, B], dt, tag="mk", name="mk")
            nc.sync.dma_start(out=mk[:], in_=mce[:])
            I = sb.tile([128, 128], dt, tag="I", name="I")
            nc.sync.dma_start(out=I[:], in_=idn[:])
            ones1 = sb.tile([1, 128], dt, tag="ones1", name="ones1")
            nc.sync.dma_start(out=ones1[:], in_=onr[:])

            ce_part = None
            wcp_part = None

            if stage >= 1:
                # ---------------- F^T tiles ----------------
                FT = []
                for q in range(4):
                    FTq = sb.tile([128, D], dt, tag=f"FT{q}", name=f"FT{q}")
                    FT.append(FTq)
                for t in range(4):
                    for q in range(4):
                        pt = pst.tile([128, 128], dt, tag="pt", name="pt")
                        nc.tensor.transpose(
                            pt[:], F[t][:, q * 128:(q + 1) * 128], I[:])
                        if (t + q) % 2 == 0:
                            nc.scalar.copy(
                                FT[q][:, t * 128:(t + 1) * 128], pt[:])
                        else:
                            nc.vector.tensor_copy(
                                FT[q][:, t * 128:(t + 1) * 128], pt[:])

                fsT = []
                for q in range(4):
                    pt = pst.tile([128, RPC], dt, tag="pt", name="pt")
                    nc.tensor.transpose(
                        pt[:], fs[:, q * 128:(q + 1) * 128], I[:RPC, :RPC])
                    fsTq = sb.tile([128, RPC], dt, tag=f"fsT{q}",
                                   name=f"fsT{q}")
                    nc.vector.tensor_copy(fsTq[:], pt[:])
                    fsT.append(fsTq)

                # sq_j row: -0.5 * sum_d F[j,:]^2
                sqc = sb.tile([128, 4], dt, tag="sqc", name="sqc")
                for t in range(4):
                    scrF = scr.tile([128, D], dt, tag="scrF", name="scrF")
                    nc.scalar.activation(scrF[:], F[t][:], fp.Square,
                                         accum_out=sqc[:, t:t + 1])
                pt4 = pst.tile([4, 128], dt, tag="pt", name="pt4")
                nc.tensor.transpose(pt4[:], sqc[:], I[:])
                s4 = sb.tile([4, 128], dt, tag="s4", name="s4")
                nc.vector.tensor_scalar_mul(s4[:], pt4[:], -0.5)
                sqrow = sb.tile([1, D], dt, tag="sqrow", name="sqrow")
                for t in range(4):
                    nc.sync.dma_start(out=sqrow[0:1, t * 128:(t + 1) * 128],
                                      in_=s4[t:t + 1, :])
                dbg = sqc

            if stage >= 2:
                # dist slab: h2 = dot - 0.5*sq_j  [64, 512]
                ph = psh.tile([RPC, D], dt, tag="ph", name="ph")
                for q in range(4):
                    nc.tensor.matmul(ph[:], fsT[q][:], FT[q][:],
                                     start=(q == 0), stop=False)
                nc.tensor.matmul(ph[:], ones1[0:1, 0:RPC], sqrow[0:1, :],
                                 start=False, stop=True)
                if stage == 2:
                    dbg = sb.tile([RPC, 1], dt, tag="dbg2", name="dbg2")
                    nc.vector.tensor_copy(dbg[:], ph[:, 0:1])

            if stage >= 3:
                # row stats / CE
                mh = sb.tile([RPC, 4], dt, tag="mh", name="mh")
                nc.vector.tensor_reduce(
                    mh[:], ph[:].rearrange("p (k x) -> p k x", k=4),
                    axis=ax.X, op=alu.max)
                bias1 = sb.tile([RPC, 4], dt, tag="bias1", name="bias1")
                nc.vector.tensor_scalar_mul(bias1[:], mh[:], -SCALE1)
                bias5 = sb.tile([RPC, 4], dt, tag="bias5", name="bias5")
                nc.vector.tensor_scalar_mul(bias5[:], mh[:], -SCALE5)

                E1 = sb.tile([RPC, D], dt, tag="E1", name="E1")
                S1 = sb.tile([RPC, 4], dt, tag="S1", name="S1")
                S5 = sb.tile([RPC, 4], dt, tag="S5", name="S5")
                hd = sb.tile([RPC, 4], dt, tag="hd", name="hd")
                for k in range(4):
                    ksl = slice(k * 128, (k + 1) * 128)
                    nc.scalar.activation(E1[:, ksl], ph[:, ksl], fp.Exp,
                                         bias=bias1[:, k:k + 1], scale=SCALE1,
                                         accum_out=S1[:, k:k + 1])
                    scrE = scr.tile([RPC, 128], dt, tag="scrE", name="scrE")
                    nc.scalar.activation(scrE[:], ph[:, ksl], fp.Exp,
                                         bias=bias5[:, k:k + 1], scale=SCALE5,
                                         accum_out=S5[:, k:k + 1])
                    scrD = scr.tile([RPC, 128], dt, tag="scrD", name="scrD")
                    nc.vector.tensor_mul(scrD[:], ph[:, ksl], mk[:])
                    nc.vector.tensor_reduce(hd[:, k:k + 1], scrD[:],
                                            axis=ax.X, op=alu.add)

                lnS5 = sb.tile([RPC, 4], dt, tag="lnS5", name="lnS5")
                nc.scalar.activation(lnS5[:], S5[:], fp.Ln)
                d1 = sb.tile([RPC, 4], dt, tag="d1", name="d1")
                nc.vector.tensor_sub(d1[:], mh[:], hd[:])
                ce4 = sb.tile([RPC, 4], dt, tag="ce4", name="ce4")
                nc.vector.scalar_tensor_tensor(
                    out=ce4[:], in0=d1[:], scalar=SCALE5, in1=lnS5[:],
                    op0=alu.mult, op1=alu.add)
                ce_part = sb.tile([RPC, 1], dt, tag="ce_part", name="ce_part")
                nc.vector.tensor_reduce(ce_part[:], ce4[:], axis=ax.X,
                                        op=alu.add)
                dbg = ce_part

            if stage >= 4:
                # p1 = softmax + 1e-12, transposed to [128, 256]
                rS1 = sb.tile([RPC, 4], dt, tag="rS1", name="rS1")
                nc.vector.reciprocal(rS1[:], S1[:])
                p1r = sb.tile([RPC, D], dt, tag="p1r", name="p1r")
                for k in range(4):
                    ksl = slice(k * 128, (k + 1) * 128)
                    nc.vector.tensor_scalar(
                        out=p1r[:, ksl], in0=E1[:, ksl],
                        scalar1=rS1[:, k:k + 1], scalar2=1e-12,
                        op0=alu.mult, op1=alu.add)
                p1T = sb.tile([128, MPC], dt, tag="p1T", name="p1T")
                for k in range(4):
                    pt = pst.tile([128, RPC], dt, tag="pt", name="ptp")
                    nc.tensor.transpose(pt[:], p1r[:, k * 128:(k + 1) * 128],
                                        I[:RPC, :RPC])
                    if k % 2 == 0:
                        nc.vector.tensor_copy(
                            p1T[:, k * RPC:(k + 1) * RPC], pt[:])
                    else:
                        nc.scalar.copy(p1T[:, k * RPC:(k + 1) * RPC], pt[:])
                if stage == 4:
                    dbg = sb.tile([128, 1], dt, tag="dbg4", name="dbg4")
                    nc.vector.tensor_copy(dbg[:], p1T[:, 0:1])

            if stage >= 5:
                # cost matrix / K / K2 / KC
                g = sb.tile([128, D], dt, tag="g", name="g")
                nc.vector.tensor_add(g[:], F[0][:], F[1][:])
                nc.vector.tensor_add(g[:], g[:], F[2][:])
                nc.vector.tensor_add(g[:], g[:], F[3][:])
                gsq = scr.tile([128, D], dt, tag="scrF", name="gsq")
                ssg = sb.tile([128, 1], dt, tag="ssg", name="ssg")
                nc.scalar.activation(gsq[:], g[:], fp.Square,
                                     accum_out=ssg[:])
                lssg = sb.tile([128, 1], dt, tag="lssg", name="lssg")
                nc.scalar.activation(lssg[:], ssg[:], fp.Ln)
                rn = sb.tile([128, 1], dt, tag="rn", name="rn")
                nc.scalar.activation(rn[:], lssg[:], fp.Exp, scale=-0.5)
                fn = sb.tile([128, D], dt, tag="fn", name="fn")
                nc.vector.tensor_scalar_mul(fn[:], g[:], rn[:, 0:1])

                fnT = []
                for q in range(4):
                    pt = pst.tile([128, 128], dt, tag="pt", name="ptf")
                    nc.tensor.transpose(pt[:], fn[:, q * 128:(q + 1) * 128],
                                        I[:])
                    fnTq = sb.tile([128, 128], dt, tag=f"fnT{q}",
                                   name=f"fnT{q}")
                    nc.vector.tensor_copy(fnTq[:], pt[:])
                    fnT.append(fnTq)

                pG = psb.tile([128, 128], dt, tag="big", name="pG")
                for q in range(4):
                    nc.tensor.matmul(pG[:], fnT[q][:], fnT[q][:],
                                     start=(q == 0), stop=(q == 3))
                gmax = sb.tile([128, 1], dt, tag="gmax", name="gmax")
                gmin = sb.tile([128, 1], dt, tag="gmin", name="gmin")
                nc.vector.tensor_reduce(gmax[:], pG[:], axis=ax.X, op=alu.max)
                nc.vector.tensor_reduce(gmin[:], pG[:], axis=ax.X, op=alu.min)
                den = sb.tile([128, 1], dt, tag="den", name="den")
                nc.vector.tensor_sub(den[:], gmax[:], gmin[:])
                rden = sb.tile([128, 1], dt, tag="rden", name="rden")
                nc.vector.reciprocal(rden[:], den[:])
                sA = sb.tile([128, 1], dt, tag="sA", name="sA")
                nc.vector.tensor_scalar_mul(sA[:], rden[:], -GAMMA)
                sB = sb.tile([128, 1], dt, tag="sB", name="sB")
                nc.vector.tensor_scalar(
                    out=sB[:], in0=gmax[:], scalar1=rden[:, 0:1],
                    scalar2=GAMMA, op0=alu.mult, op1=alu.mult)
                costm = sb.tile([128, 128], dt, tag="costm", name="costm")
                nc.vector.tensor_scalar(
                    out=costm[:], in0=pG[:], scalar1=sA[:, 0:1],
                    scalar2=sB[:, 0:1], op0=alu.mult, op1=alu.add)
                nc.vector.tensor_add(costm[:], costm[:], I[:])

                K = sb.tile([128, 128], dt, tag="K", name="K")
                nc.scalar.activation(K[:], costm[:], fp.Exp, scale=-2.0)
                ln128t = sb.tile([128, 1], dt, tag="ln128t", name="ln128t")
                nc.vector.memset(ln128t[:], LN128)
                K2 = sb.tile([128, 128], dt, tag="K2", name="K2")
                nc.scalar.activation(K2[:], costm[:], fp.Exp,
                                     bias=ln128t[:, 0:1], scale=-2.0)
                ptK = pst.tile([128, 128], dt, tag="pt", name="ptK")
                nc.tensor.transpose(ptK[:], K[:], I[:])
                KT = sb.tile([128, 128], dt, tag="KT", name="KT")
                nc.vector.tensor_copy(KT[:], ptK[:])
                KC = sb.tile([128, 128], dt, tag="KC", name="KC")
                nc.vector.tensor_mul(KC[:], K[:], costm[:])
                if stage == 5:
                    dbg = sb.tile([128, 1], dt, tag="dbg5", name="dbg5")
                    nc.vector.tensor_copy(dbg[:], K[:, 0:1])

            if stage >= 6:
                # Sinkhorn loop
                b = sb.tile([128, MPC], dt, tag="b0", name="b0")
                nc.vector.memset(b[:], 1.0)
                a = None
                for it in range(SINK_ITR):
                    py = psb.tile([128, MPC], dt, tag="big", name=f"py{it}")
                    nc.tensor.matmul(py[:], KT[:], b[:], start=True, stop=True)
                    r = scr.tile([128, MPC], dt, tag="r", name=f"r{it}")
                    nc.vector.reciprocal_approx_fast(out=r[:], in_=py[:])
                    a = scr.tile([128, MPC], dt, tag="a", name=f"a{it}")
                    nc.vector.tensor_mul(a[:], p1T[:], r[:])
                    pz = psb.tile([128, MPC], dt, tag="big", name=f"pz{it}")
                    nc.tensor.matmul(pz[:], K2[:], a[:], start=True, stop=True)
                    b = scr.tile([128, MPC], dt, tag="b", name=f"b{it}")
                    nc.vector.reciprocal_approx_fast(out=b[:], in_=pz[:])
                if stage == 6:
                    dbg = sb.tile([128, 1], dt, tag="dbg6", name="dbg6")
                    nc.vector.tensor_copy(dbg[:], b[:, 0:1])

            if stage >= 7:
                # wcp epilogue
                pw = psb.tile([128, MPC], dt, tag="big", name="pw")
                nc.tensor.matmul(pw[:], KC[:], a[:], start=True, stop=True)
                scrW = scr.tile([128, MPC], dt, tag="r", name="scrW")
                wcp_part = sb.tile([128, 1], dt, tag="wcp_part",
                                   name="wcp_part")
                nc.vector.tensor_mul(scrW[:], pw[:], b[:])
                nc.vector.tensor_reduce(wcp_part[:], scrW[:],
                                        axis=ax.X, op=alu.add)

            # ---------------- pack + store ----------------
            outS = sb.tile([128, 2], dt, tag="outS", name="outS")
            nc.vector.memset(outS[:], 0.0)
            if wcp_part is not None:
                nc.vector.tensor_copy(outS[:, 0:1], wcp_part[:])
            elif dbg is not None:
                p = min(dbg.shape[0], 128)
                nc.vector.tensor_copy(outS[0:p, 0:1], dbg[0:p, 0:1])
            if ce_part is not None:
                nc.vector.tensor_copy(outS[0:RPC, 1:2], ce_part[:])
            nc.sync.dma_start(out=outd[:], in_=outS[:])

    nc.compile()
    return nc


def _get_nc(stage=99):
    key = ("nc", stage)
    if key not in _CACHE:
        _CACHE[key] = _build_nc(stage)
    return _CACHE[key]


def _make_in_maps(features):
    ident = np.eye(128, dtype=np.float32)
    onesr = np.ones((1, 128), dtype=np.float32)
    in_maps = []
    for c in range(NCORES):
        maskce = np.zeros((RPC, B), dtype=np.float32)
        off = (c % 2) * 64
        maskce[np.arange(RPC), off + np.arange(RPC)] = 1.0
        in_maps.append({
            "features": features,
            "fslice": np.ascontiguousarray(features[c * RPC:(c + 1) * RPC, :]),
            "maskce": maskce,
            "ident": ident,
            "onesr": onesr,
        })
    return in_maps


def kernel(features, batch=None, **kwargs):
    from concourse.bass_utils import run_bass_kernel_spmd

    features = np.ascontiguousarray(np.asarray(features, dtype=np.float32))
    assert features.shape == (N, D)

    nc = _get_nc()
    res = run_bass_kernel_spmd(nc, _make_in_maps(features),
                               list(range(NCORES)))

    ce_sum = 0.0
    wcp_sum = 0.0
    for c in range(NCORES):
        o = res.results[c]["out"]
        wcp_sum += float(o[:, 0].sum(dtype=np.float64))
        ce_sum += float(o[:RPC, 1].sum(dtype=np.float64))
    loss = ce_sum / M_TOT + wcp_sum / M_TOT
    return np.float32(loss)


if __name__ == "__main__":
    x = np.random.randn(N, D).astype(np.float32)
    print(kernel(x, B))


# revision 12
# speedup vs baseline: 1.1843x; 1.1843x over previous
"""Trainium2 Bass kernel for the CPN/WCP loss (ce + Sinkhorn wcp).

Strategy:
  - M = 2048 Sinkhorn problems sharded 256/core over 8 cores.
  - Per core: compute its 64-row slab of the NxN (-eudis)/2 matrix via PE
    matmuls (rank-1 matmul folds in the -0.5*sq_j term; the per-row sq_i
    shift is dropped -- softmax/log-softmax are shift invariant).
  - CE pieces (row LSE at temp 5, target logit) computed in row layout.
  - Softmax p1 computed in row layout, transposed to [128 class, 256 prob]
    via PE transposes.
  - Sinkhorn runs in multiplicative form: a = p1 / (K@b), b = p2 / (K^T@a)
    with K = exp(-2*cost) fixed => two matmuls + DVE approx-reciprocals per
    iteration, no transcendentals in the loop.
  - wcp_m = ((K.C)^T a) . b ; per-partition partials DMA'd out, host sums.
"""

import sys

for _p in ("/opt/trn_rl_repo",):
    if _p not in sys.path:
        sys.path.insert(0, _p)

import numpy as np

AUG = 4
B = 128
D = 512
N = AUG * B          # 512 feature rows
NCORES = 8
RPC = N // NCORES    # 64 eudis rows per core
MPC = RPC * AUG      # 256 sinkhorn problems per core
M_TOT = N * AUG      # 2048
TEMP = 5.0
GAMMA = 0.2
SINK_ITR = 5
SCALE1 = 2.0 / float(np.sqrt(np.float32(D)))  # softmax scale on h
SCALE5 = 2.0 / TEMP                            # CE scale on h
LN128 = float(np.log(128.0))

_CACHE = {}


def _build_nc(stage=99):
    import concourse.bacc as bacc
    import concourse.tile as tile
    import concourse.mybir as mybir

    dt = mybir.dt.float32
    dtb = mybir.dt.bfloat16
    fp = mybir.ActivationFunctionType
    alu = mybir.AluOpType
    ax = mybir.AxisListType

    nc = bacc.Bacc(
        "TRN2",
        target_bir_lowering=False,
        debug=False,
        enable_asserts=False,
        num_devices=NCORES,
    )

    feat = nc.dram_tensor("features", [N, D], dt, kind="ExternalInput").ap()
    fsl = nc.dram_tensor("fslice", [RPC, D], dt, kind="ExternalInput").ap()
    mce = nc.dram_tensor("maskce", [RPC, B], dt, kind="ExternalInput").ap()
    idn = nc.dram_tensor("ident", [128, 128], dt, kind="ExternalInput").ap()
    onr = nc.dram_tensor("onesr", [1, 128], dt, kind="ExternalInput").ap()
    outd = nc.dram_tensor("out", [128, 2], dt, kind="ExternalOutput").ap()

    with tile.TileContext(nc) as tc:
        with (
            tc.tile_pool(name="sb", bufs=1) as sb,
            tc.tile_pool(name="scr", bufs=2) as scr,
            tc.tile_pool(name="ps_big", bufs=2, space="PSUM") as psb,
            tc.tile_pool(name="ps_t", bufs=3, space="PSUM") as pst,
            tc.tile_pool(name="ps_h", bufs=1, space="PSUM") as psh,
        ):
            dbg = None  # [*,1] tile flushed to out col0 for stage bisection

            # Preload the combined exp+ln ACT table set so the compiler's
            # per-func set picker doesn't ping-pong exp_and_others <->
            # natural_log (each reload costs ~2.7us).
            _tabs = list(__import__("concourse.hw_specs",
                                    fromlist=["hw_specs"]
                                    ).get_activation_tables(nc.m.arch))
            _set_id = _tabs.index("natural_log_exp_and_others")
            nc.scalar.add_instruction(mybir.InstLoadActFuncSet(
                name=nc.get_next_instruction_name(), ins=[], outs=[],
                act_func_set_id=_set_id))

            # ---------------- loads ----------------
            F = []
            for t in range(4):
                Ft = sb.tile([128, D], dt, tag=f"F{t}", name=f"F{t}")
                nc.sync.dma_start(out=Ft[:], in_=feat[t * 128:(t + 1) * 128, :])
                F.append(Ft)
            fs = sb.tile([RPC, D], dt, tag="fs", name="fs")
            nc.sync.dma_start(out=fs[:], in_=fsl[:])
            mk = sb.tile([RPC, B], dt, tag="mk", name="mk")
            nc.sync.dma_start(out=mk[:], in_=mce[:])
            I = sb.tile([128, 128], dt, tag="I", name="I")
            nc.sync.dma_start(out=I[:], in_=idn[:])
            ones1 = sb.tile([1, 128], dt, tag="ones1", name="ones1")
            nc.sync.dma_start(out=ones1[:], in_=onr[:])

            ce_part = None
            wcp_part = None

            if stage >= 1:
                # ---------------- F^T tiles ----------------
                FT = []
                for q in range(4):
                    FTq = sb.tile([128, D], dt, tag=f"FT{q}", name=f"FT{q}")
                    FT.append(FTq)
                for t in range(4):
                    for q in range(4):
                        pt = pst.tile([128, 128], dt, tag="pt", name="pt")
                        nc.tensor.transpose(
                            pt[:], F[t][:, q * 128:(q + 1) * 128], I[:])
                        if (t + q) % 2 == 0:
                            nc.scalar.copy(
                                FT[q][:, t * 128:(t + 1) * 128], pt[:])
                        else:
                            nc.vector.tensor_copy(
                                FT[q][:, t * 128:(t + 1) * 128], pt[:])

                fsT = []
                for q in range(4):
                    pt = pst.tile([128, RPC], dt, tag="pt", name="pt")
                    nc.tensor.transpose(
                        pt[:], fs[:, q * 128:(q + 1) * 128], I[:RPC, :RPC])
                    fsTq = sb.tile([128, RPC], dt, tag=f"fsT{q}",
                                   name=f"fsT{q}")
                    nc.vector.tensor_copy(fsTq[:], pt[:])
                    fsT.append(fsTq)

                # sq_j row: -0.5 * sum_d F[j,:]^2
                sqc = sb.tile([128, 4], dt, tag="sqc", name="sqc")
                for t in range(4):
                    scrF = scr.tile([128, D], dt, tag="scrF", name="scrF")
                    nc.scalar.activation(scrF[:], F[t][:], fp.Square,
                                         accum_out=sqc[:, t:t + 1])
                pt4 = pst.tile([4, 128], dt, tag="pt", name="pt4")
                nc.tensor.transpose(pt4[:], sqc[:], I[:])
                s4 = sb.tile([4, 128], dt, tag="s4", name="s4")
                nc.vector.tensor_scalar_mul(s4[:], pt4[:], -0.5)
                sqrow = sb.tile([1, D], dt, tag="sqrow", name="sqrow")
                for t in range(4):
                    nc.sync.dma_start(out=sqrow[0:1, t * 128:(t + 1) * 128],
                                      in_=s4[t:t + 1, :])
                dbg = sqc

            if stage >= 2:
                # dist slab: h2 = dot - 0.5*sq_j  [64, 512]
                ph = psh.tile([RPC, D], dt, tag="ph", name="ph")
                for q in range(4):
                    nc.tensor.matmul(ph[:], fsT[q][:], FT[q][:],
                                     start=(q == 0), stop=False)
                nc.tensor.matmul(ph[:], ones1[0:1, 0:RPC], sqrow[0:1, :],
                                 start=False, stop=True)
                if stage == 2:
                    dbg = sb.tile([RPC, 1], dt, tag="dbg2", name="dbg2")
                    nc.vector.tensor_copy(dbg[:], ph[:, 0:1])

            if stage >= 3:
                # row stats / CE
                mh = sb.tile([RPC, 4], dt, tag="mh", name="mh")
                nc.vector.tensor_reduce(
                    mh[:], ph[:].rearrange("p (k x) -> p k x", k=4),
                    axis=ax.X, op=alu.max)
                bias1 = sb.tile([RPC, 4], dt, tag="bias1", name="bias1")
                nc.vector.tensor_scalar_mul(bias1[:], mh[:], -SCALE1)
                bias5 = sb.tile([RPC, 4], dt, tag="bias5", name="bias5")
                nc.vector.tensor_scalar_mul(bias5[:], mh[:], -SCALE5)

                E1 = sb.tile([RPC, D], dt, tag="E1", name="E1")
                S1 = sb.tile([RPC, 4], dt, tag="S1", name="S1")
                S5 = sb.tile([RPC, 4], dt, tag="S5", name="S5")
                hd = sb.tile([RPC, 4], dt, tag="hd", name="hd")
                for k in range(4):
                    ksl = slice(k * 128, (k + 1) * 128)
                    nc.scalar.activation(E1[:, ksl], ph[:, ksl], fp.Exp,
                                         bias=bias1[:, k:k + 1], scale=SCALE1,
                                         accum_out=S1[:, k:k + 1])
                    scrE = scr.tile([RPC, 128], dt, tag="scrE", name="scrE")
                    nc.scalar.activation(scrE[:], ph[:, ksl], fp.Exp,
                                         bias=bias5[:, k:k + 1], scale=SCALE5,
                                         accum_out=S5[:, k:k + 1])
                    scrD = scr.tile([RPC, 128], dt, tag="scrD", name="scrD")
                    nc.vector.tensor_mul(scrD[:], ph[:, ksl], mk[:])
                    nc.vector.tensor_reduce(hd[:, k:k + 1], scrD[:],
                                            axis=ax.X, op=alu.add)

                lnS5 = sb.tile([RPC, 4], dt, tag="lnS5", name="lnS5")
                nc.scalar.activation(lnS5[:], S5[:], fp.Ln)
                d1 = sb.tile([RPC, 4], dt, tag="d1", name="d1")
                nc.vector.tensor_sub(d1[:], mh[:], hd[:])
                ce4 = sb.tile([RPC, 4], dt, tag="ce4", name="ce4")
                nc.vector.scalar_tensor_tensor(
                    out=ce4[:], in0=d1[:], scalar=SCALE5, in1=lnS5[:],
                    op0=alu.mult, op1=alu.add)
                ce_part = sb.tile([RPC, 1], dt, tag="ce_part", name="ce_part")
                nc.vector.tensor_reduce(ce_part[:], ce4[:], axis=ax.X,
                                        op=alu.add)
                dbg = ce_part

            if stage >= 4:
                # p1 = softmax + 1e-12, transposed to [128, 256]
                rS1 = sb.tile([RPC, 4], dt, tag="rS1", name="rS1")
                nc.vector.reciprocal(rS1[:], S1[:])
                p1r = sb.tile([RPC, D], dt, tag="p1r", name="p1r")
                for k in range(4):
                    ksl = slice(k * 128, (k + 1) * 128)
                    nc.vector.tensor_scalar(
                        out=p1r[:, ksl], in0=E1[:, ksl],
                        scalar1=rS1[:, k:k + 1], scalar2=1e-12,
                        op0=alu.mult, op1=alu.add)
                p1T = sb.tile([128, MPC], dtb, tag="p1T", name="p1T")
                for k in range(4):
                    pt = pst.tile([128, RPC], dt, tag="pt", name="ptp")
                    nc.tensor.transpose(pt[:], p1r[:, k * 128:(k + 1) * 128],
                                        I[:RPC, :RPC])
                    if k % 2 == 0:
                        nc.vector.tensor_copy(
                            p1T[:, k * RPC:(k + 1) * RPC], pt[:])
                    else:
                        nc.scalar.copy(p1T[:, k * RPC:(k + 1) * RPC], pt[:])
                if stage == 4:
                    dbg = sb.tile([128, 1], dt, tag="dbg4", name="dbg4")
                    nc.vector.tensor_copy(dbg[:], p1T[:, 0:1])

            if stage >= 5:
                # cost matrix / K / K2 / KC
                g = sb.tile([128, D], dt, tag="g", name="g")
                nc.vector.tensor_add(g[:], F[0][:], F[1][:])
                nc.vector.tensor_add(g[:], g[:], F[2][:])
                nc.vector.tensor_add(g[:], g[:], F[3][:])
                gsq = scr.tile([128, D], dt, tag="scrF", name="gsq")
                ssg = sb.tile([128, 1], dt, tag="ssg", name="ssg")
                nc.scalar.activation(gsq[:], g[:], fp.Square,
                                     accum_out=ssg[:])
                lssg = sb.tile([128, 1], dt, tag="lssg", name="lssg")
                nc.scalar.activation(lssg[:], ssg[:], fp.Ln)
                rn = sb.tile([128, 1], dt, tag="rn", name="rn")
                nc.scalar.activation(rn[:], lssg[:], fp.Exp, scale=-0.5)
                fn = sb.tile([128, D], dt, tag="fn", name="fn")
                nc.vector.tensor_scalar_mul(fn[:], g[:], rn[:, 0:1])

                fnT = []
                for q in range(4):
                    pt = pst.tile([128, 128], dt, tag="pt", name="ptf")
                    nc.tensor.transpose(pt[:], fn[:, q * 128:(q + 1) * 128],
                                        I[:])
                    fnTq = sb.tile([128, 128], dtb, tag=f"fnT{q}",
                                   name=f"fnT{q}")
                    nc.vector.tensor_copy(fnTq[:], pt[:])
                    fnT.append(fnTq)

                pG = psb.tile([128, 128], dt, tag="big", name="pG")
                for q in range(4):
                    nc.tensor.matmul(pG[:], fnT[q][:], fnT[q][:],
                                     start=(q == 0), stop=(q == 3))
                gmax = sb.tile([128, 1], dt, tag="gmax", name="gmax")
                gmin = sb.tile([128, 1], dt, tag="gmin", name="gmin")
                nc.vector.tensor_reduce(gmax[:], pG[:], axis=ax.X, op=alu.max)
                nc.vector.tensor_reduce(gmin[:], pG[:], axis=ax.X, op=alu.min)
                den = sb.tile([128, 1], dt, tag="den", name="den")
                nc.vector.tensor_sub(den[:], gmax[:], gmin[:])
                rden = sb.tile([128, 1], dt, tag="rden", name="rden")
                nc.vector.reciprocal(rden[:], den[:])
                sA = sb.tile([128, 1], dt, tag="sA", name="sA")
                nc.vector.tensor_scalar_mul(sA[:], rden[:], -GAMMA)
                sB = sb.tile([128, 1], dt, tag="sB", name="sB")
                nc.vector.tensor_scalar(
                    out=sB[:], in0=gmax[:], scalar1=rden[:, 0:1],
                    scalar2=GAMMA, op0=alu.mult, op1=alu.mult)
                costm = sb.tile([128, 128], dt, tag="costm", name="costm")
                nc.vector.tensor_scalar(
                    out=costm[:], in0=pG[:], scalar1=sA[:, 0:1],
                    scalar2=sB[:, 0:1], op0=alu.mult, op1=alu.add)
                nc.vector.tensor_add(costm[:], costm[:], I[:])

                K = sb.tile([128, 128], dt, tag="K", name="K")
                nc.scalar.activation(K[:], costm[:], fp.Exp, scale=-2.0)
                ln128t = sb.tile([128, 1], dt, tag="ln128t", name="ln128t")
                nc.vector.memset(ln128t[:], LN128)
                K2 = sb.tile([128, 128], dtb, tag="K2", name="K2")
                nc.scalar.activation(K2[:], costm[:], fp.Exp,
                                     bias=ln128t[:, 0:1], scale=-2.0)
                ptK = pst.tile([128, 128], dt, tag="pt", name="ptK")
                nc.tensor.transpose(ptK[:], costm[:], I[:])
                costmT = sb.tile([128, 128], dt, tag="costmT", name="costmT")
                nc.vector.tensor_copy(costmT[:], ptK[:])
                KT = sb.tile([128, 128], dtb, tag="KT", name="KT")
                nc.scalar.activation(KT[:], costmT[:], fp.Exp, scale=-2.0)
                KC = sb.tile([128, 128], dtb, tag="KC", name="KC")
                nc.vector.tensor_mul(KC[:], K[:], costm[:])
                if stage == 5:
                    dbg = sb.tile([128, 1], dt, tag="dbg5", name="dbg5")
                    nc.vector.tensor_copy(dbg[:], K[:, 0:1])

            if stage >= 6:
                # Sinkhorn loop
                b = sb.tile([128, MPC], dtb, tag="b0", name="b0")
                nc.vector.memset(b[:], 1.0)
                a = None
                for it in range(SINK_ITR):
                    py = psb.tile([128, MPC], dt, tag="big", name=f"py{it}")
                    nc.tensor.matmul(py[:], KT[:], b[:], start=True, stop=True)
                    r = scr.tile([128, MPC], dt, tag="r", name=f"r{it}")
                    nc.vector.reciprocal_approx_fast(out=r[:], in_=py[:])
                    a = scr.tile([128, MPC], dtb, tag="a", name=f"a{it}")
                    nc.vector.tensor_mul(a[:], p1T[:], r[:])
                    pz = psb.tile([128, MPC], dt, tag="big", name=f"pz{it}")
                    nc.tensor.matmul(pz[:], K2[:], a[:], start=True, stop=True)
                    rb = scr.tile([128, MPC], dt, tag="rb", name=f"rb{it}")
                    nc.vector.reciprocal_approx_fast(out=rb[:], in_=pz[:])
                    b = scr.tile([128, MPC], dtb, tag="b", name=f"b{it}")
                    nc.vector.tensor_copy(b[:], rb[:])
                if stage == 6:
                    dbg = sb.tile([128, 1], dt, tag="dbg6", name="dbg6")
                    nc.vector.tensor_copy(dbg[:], b[:, 0:1])

            if stage >= 7:
                # wcp epilogue
                pw = psb.tile([128, MPC], dt, tag="big", name="pw")
                nc.tensor.matmul(pw[:], KC[:], a[:], start=True, stop=True)
                scrW = scr.tile([128, MPC], dt, tag="r", name="scrW")
                wcp_part = sb.tile([128, 1], dt, tag="wcp_part",
                                   name="wcp_part")
                nc.vector.tensor_mul(scrW[:], pw[:], b[:])
                nc.vector.tensor_reduce(wcp_part[:], scrW[:],
                                        axis=ax.X, op=alu.add)

            # ---------------- pack + store ----------------
            outS = sb.tile([128, 2], dt, tag="outS", name="outS")
            nc.vector.memset(outS[:], 0.0)
            if wcp_part is not None:
                nc.vector.tensor_copy(outS[:, 0:1], wcp_part[:])
            elif dbg is not None:
                p = min(dbg.shape[0], 128)
                nc.vector.tensor_copy(outS[0:p, 0:1], dbg[0:p, 0:1])
            if ce_part is not None:
                nc.vector.tensor_copy(outS[0:RPC, 1:2], ce_part[:])
            nc.sync.dma_start(out=outd[:], in_=outS[:])

    nc.compile()
    return nc


def _get_nc(stage=99):
    key = ("nc", stage)
    if key not in _CACHE:
        _CACHE[key] = _build_nc(stage)
    return _CACHE[key]


def _make_in_maps(features):
    ident = np.eye(128, dtype=np.float32)
    onesr = np.ones((1, 128), dtype=np.float32)
    in_maps = []
    for c in range(NCORES):
        maskce = np.zeros((RPC, B), dtype=np.float32)
        off = (c % 2) * 64
        maskce[np.arange(RPC), off + np.arange(RPC)] = 1.0
        in_maps.append({
            "features": features,
            "fslice": np.ascontiguousarray(features[c * RPC:(c + 1) * RPC, :]),
            "maskce": maskce,
            "ident": ident,
            "onesr": onesr,
        })
    return in_maps


def kernel(features, batch=None, **kwargs):
    from concourse.bass_utils import run_bass_kernel_spmd

    features = np.ascontiguousarray(np.asarray(features, dtype=np.float32))
    assert features.shape == (N, D)

    nc = _get_nc()
    res = run_bass_kernel_spmd(nc, _make_in_maps(features),
                               list(range(NCORES)))

    ce_sum = 0.0
    wcp_sum = 0.0
    for c in range(NCORES):
        o = res.results[c]["out"]
        wcp_sum += float(o[:, 0].sum(dtype=np.float64))
        ce_sum += float(o[:RPC, 1].sum(dtype=np.float64))
    loss = ce_sum / M_TOT + wcp_sum / M_TOT
    return np.float32(loss)


if __name__ == "__main__":
    x = np.random.randn(N, D).astype(np.float32)
    print(kernel(x, B))


# revision 16
# speedup vs baseline: 1.2604x; 1.0642x over previous
"""Trainium2 Bass kernel for the CPN/WCP loss (ce + Sinkhorn wcp).

Strategy:
  - M = 2048 Sinkhorn problems sharded 256/core over 8 cores.
  - Per core: compute its 64-row slab of the NxN (-eudis)/2 matrix via PE
    matmuls (rank-1 matmul folds in the -0.5*sq_j term; the per-row sq_i
    shift is dropped -- softmax/log-softmax are shift invariant).
  - CE pieces (row LSE at temp 5, target logit) computed in row layout.
  - Softmax p1 computed in row layout, transposed to [128 class, 256 prob]
    via PE transposes.
  - Sinkhorn runs in multiplicative form: a = p1 / (K@b), b = p2 / (K^T@a)
    with K = exp(-2*cost) fixed => two matmuls + DVE approx-reciprocals per
    iteration, no transcendentals in the loop.
  - wcp_m = ((K.C)^T a) . b ; per-partition partials DMA'd out, host sums.
"""

import sys

for _p in ("/opt/trn_rl_repo",):
    if _p not in sys.path:
        sys.path.insert(0, _p)

import numpy as np

AUG = 4
B = 128
D = 512
N = AUG * B          # 512 feature rows
NCORES = 8
RPC = N // NCORES    # 64 eudis rows per core
MPC = RPC * AUG      # 256 sinkhorn problems per core
M_TOT = N * AUG      # 2048
TEMP = 5.0
GAMMA = 0.2
SINK_ITR = 5
SCALE1 = 2.0 / float(np.sqrt(np.float32(D)))  # softmax scale on h
SCALE5 = 2.0 / TEMP                            # CE scale on h
LN128 = float(np.log(128.0))

_CACHE = {}


def _build_nc(stage=99):
    import concourse.bacc as bacc
    import concourse.tile as tile
    import concourse.mybir as mybir

    dt = mybir.dt.float32
    dtr = mybir.dt.float32r
    dtb = mybir.dt.bfloat16
    fp = mybir.ActivationFunctionType
    alu = mybir.AluOpType
    ax = mybir.AxisListType

    nc = bacc.Bacc(
        "TRN2",
        target_bir_lowering=False,
        debug=False,
        enable_asserts=False,
        num_devices=NCORES,
    )

    feat = nc.dram_tensor("features", [N, D], dt, kind="ExternalInput").ap()
    fsl = nc.dram_tensor("fslice", [RPC, D], dt, kind="ExternalInput").ap()
    mce = nc.dram_tensor("maskce", [RPC, B], dt, kind="ExternalInput").ap()
    idn = nc.dram_tensor("ident", [128, 128], dt, kind="ExternalInput").ap()
    onr = nc.dram_tensor("onesr", [1, 128], dtr, kind="ExternalInput").ap()
    outd = nc.dram_tensor("out", [128, 2], dt, kind="ExternalOutput").ap()

    with tile.TileContext(nc) as tc:
        with (
            tc.tile_pool(name="sb", bufs=1) as sb,
            tc.tile_pool(name="scr", bufs=2) as scr,
            tc.tile_pool(name="ps_big", bufs=2, space="PSUM") as psb,
            tc.tile_pool(name="ps_t", bufs=3, space="PSUM") as pst,
            tc.tile_pool(name="ps_h", bufs=1, space="PSUM") as psh,
        ):
            dbg = None  # [*,1] tile flushed to out col0 for stage bisection

            # Preload the combined exp+ln ACT table set so the compiler's
            # per-func set picker doesn't ping-pong exp_and_others <->
            # natural_log (each reload costs ~2.7us).
            _tabs = list(__import__("concourse.hw_specs",
                                    fromlist=["hw_specs"]
                                    ).get_activation_tables(nc.m.arch))
            _set_id = _tabs.index("natural_log_exp_and_others")
            nc.scalar.add_instruction(mybir.InstLoadActFuncSet(
                name=nc.get_next_instruction_name(), ins=[], outs=[],
                act_func_set_id=_set_id))

            # ---------------- loads ----------------
            # ident first (gates all PE transposes), then fslice, then F.
            I = sb.tile([128, 128], dt, tag="I", name="I")
            nc.sync.dma_start(out=I[:], in_=idn[:])
            fs = sb.tile([RPC, D], dt, tag="fs", name="fs")
            nc.sync.dma_start(out=fs[:], in_=fsl[:])
            F = []
            for t in range(4):
                Ft = sb.tile([128, D], dt, tag=f"F{t}", name=f"F{t}")
                eng = nc.sync if t % 2 == 0 else nc.gpsimd
                eng.dma_start(out=Ft[:], in_=feat[t * 128:(t + 1) * 128, :])
                F.append(Ft)
            mk = sb.tile([RPC, B], dt, tag="mk", name="mk")
            nc.gpsimd.dma_start(out=mk[:], in_=mce[:])
            ones1 = sb.tile([1, 128], dtr, tag="ones1", name="ones1")
            nc.gpsimd.dma_start(out=ones1[:], in_=onr[:])

            ce_part = None
            wcp_part = None

            if stage >= 1:
                # ---------------- F^T tiles ----------------
                FT = []
                for q in range(4):
                    FTq = sb.tile([128, D], dtr, tag=f"FT{q}", name=f"FT{q}")
                    FT.append(FTq)
                for t in range(4):
                    for q in range(4):
                        pt = pst.tile([128, 128], dt, tag="pt", name="pt")
                        nc.tensor.transpose(
                            pt[:], F[t][:, q * 128:(q + 1) * 128], I[:])
                        if (t + q) % 2 == 0:
                            nc.scalar.copy(
                                FT[q][:, t * 128:(t + 1) * 128], pt[:])
                        else:
                            nc.vector.tensor_copy(
                                FT[q][:, t * 128:(t + 1) * 128], pt[:])

                fsT = []
                for q in range(4):
                    pt = pst.tile([128, RPC], dt, tag="pt", name="pt")
                    nc.tensor.transpose(
                        pt[:], fs[:, q * 128:(q + 1) * 128], I[:RPC, :RPC])
                    fsTq = sb.tile([128, RPC], dtr, tag=f"fsT{q}",
                                   name=f"fsT{q}")
                    nc.vector.tensor_copy(fsTq[:], pt[:])
                    fsT.append(fsTq)

                # sq_j row: -0.5 * sum_d F[j,:]^2
                sqc = sb.tile([128, 4], dt, tag="sqc", name="sqc")
                for t in range(4):
                    scrF = scr.tile([128, D], dt, tag="scrF", name="scrF")
                    nc.scalar.activation(scrF[:], F[t][:], fp.Square,
                                         accum_out=sqc[:, t:t + 1])
                pt4 = pst.tile([4, 128], dt, tag="pt", name="pt4")
                nc.tensor.transpose(pt4[:], sqc[:], I[:])
                s4 = sb.tile([4, 128], dtr, tag="s4", name="s4")
                nc.vector.tensor_scalar_mul(s4[:], pt4[:], -0.5)
                sqrow = sb.tile([1, D], dtr, tag="sqrow", name="sqrow")
                for t in range(4):
                    nc.gpsimd.dma_start(out=sqrow[0:1, t * 128:(t + 1) * 128],
                                        in_=s4[t:t + 1, :])

                # mean-feature branch (gpsimd: off the DVE critical path)
                g = sb.tile([128, D], dt, tag="g", name="g")
                g2 = sb.tile([128, D], dt, tag="g2", name="g2")
                nc.gpsimd.tensor_add(g2[:], F[0][:], F[1][:])
                nc.gpsimd.tensor_add(g[:], F[2][:], F[3][:])
                nc.gpsimd.tensor_add(g[:], g[:], g2[:])
                gsq = scr.tile([128, D], dt, tag="scrF", name="gsq")
                ssg = sb.tile([128, 1], dt, tag="ssg", name="ssg")
                nc.scalar.activation(gsq[:], g[:], fp.Square,
                                     accum_out=ssg[:])
                lssg = sb.tile([128, 1], dt, tag="lssg", name="lssg")
                nc.scalar.activation(lssg[:], ssg[:], fp.Ln)
                rn = sb.tile([128, 1], dt, tag="rn", name="rn")
                nc.scalar.activation(rn[:], lssg[:], fp.Exp, scale=-0.5)
                fn = sb.tile([128, D], dt, tag="fn", name="fn")
                nc.vector.tensor_scalar_mul(fn[:], g[:], rn[:, 0:1])
                dbg = sqc

            if stage >= 2:
                # dist slab: h2 = dot - 0.5*sq_j  [64, 512]
                ph = psh.tile([RPC, D], dt, tag="ph", name="ph")
                for q in range(4):
                    nc.tensor.matmul(ph[:], fsT[q][:], FT[q][:],
                                     start=(q == 0), stop=False)
                nc.tensor.matmul(ph[:], ones1[0:1, 0:RPC], sqrow[0:1, :],
                                 start=False, stop=True)
                if stage == 2:
                    dbg = sb.tile([RPC, 1], dt, tag="dbg2", name="dbg2")
                    nc.vector.tensor_copy(dbg[:], ph[:, 0:1])

            if stage >= 3:
                # row stats / CE
                mh = sb.tile([RPC, 4], dt, tag="mh", name="mh")
                nc.vector.tensor_reduce(
                    mh[:], ph[:].rearrange("p (k x) -> p k x", k=4),
                    axis=ax.X, op=alu.max)
                bias1 = sb.tile([RPC, 4], dt, tag="bias1", name="bias1")
                nc.vector.tensor_scalar_mul(bias1[:], mh[:], -SCALE1)
                bias5 = sb.tile([RPC, 4], dt, tag="bias5", name="bias5")
                nc.vector.tensor_scalar_mul(bias5[:], mh[:], -SCALE5)

                E1 = sb.tile([RPC, D], dt, tag="E1", name="E1")
                S1 = sb.tile([RPC, 4], dt, tag="S1", name="S1")
                S5 = sb.tile([RPC, 4], dt, tag="S5", name="S5")
                hd = sb.tile([RPC, 4], dt, tag="hd", name="hd")
                for k in range(4):
                    ksl = slice(k * 128, (k + 1) * 128)
                    nc.scalar.activation(E1[:, ksl], ph[:, ksl], fp.Exp,
                                         bias=bias1[:, k:k + 1], scale=SCALE1,
                                         accum_out=S1[:, k:k + 1])
                    scrE = scr.tile([RPC, 128], dt, tag="scrE", name="scrE")
                    nc.scalar.activation(scrE[:], ph[:, ksl], fp.Exp,
                                         bias=bias5[:, k:k + 1], scale=SCALE5,
                                         accum_out=S5[:, k:k + 1])
                    scrD = scr.tile([RPC, 128], dt, tag="scrD", name="scrD")
                    nc.vector.tensor_mul(scrD[:], ph[:, ksl], mk[:])
                    nc.vector.tensor_reduce(hd[:, k:k + 1], scrD[:],
                                            axis=ax.X, op=alu.add)

                lnS5 = sb.tile([RPC, 4], dt, tag="lnS5", name="lnS5")
                nc.scalar.activation(lnS5[:], S5[:], fp.Ln)
                d1 = sb.tile([RPC, 4], dt, tag="d1", name="d1")
                nc.vector.tensor_sub(d1[:], mh[:], hd[:])
                ce4 = sb.tile([RPC, 4], dt, tag="ce4", name="ce4")
                nc.vector.scalar_tensor_tensor(
                    out=ce4[:], in0=d1[:], scalar=SCALE5, in1=lnS5[:],
                    op0=alu.mult, op1=alu.add)
                ce_part = sb.tile([RPC, 1], dt, tag="ce_part", name="ce_part")
                nc.vector.tensor_reduce(ce_part[:], ce4[:], axis=ax.X,
                                        op=alu.add)
                dbg = ce_part

            if stage >= 4:
                # p1 = softmax + 1e-12, transposed to [128, 256]
                rS1 = sb.tile([RPC, 4], dt, tag="rS1", name="rS1")
                nc.vector.reciprocal(rS1[:], S1[:])
                p1r = sb.tile([RPC, D], dt, tag="p1r", name="p1r")
                for k in range(4):
                    ksl = slice(k * 128, (k + 1) * 128)
                    nc.vector.tensor_scalar(
                        out=p1r[:, ksl], in0=E1[:, ksl],
                        scalar1=rS1[:, k:k + 1], scalar2=1e-12,
                        op0=alu.mult, op1=alu.add)
                p1T = sb.tile([128, MPC], dtb, tag="p1T", name="p1T")
                for k in range(4):
                    pt = pst.tile([128, RPC], dt, tag="pt", name="ptp")
                    nc.tensor.transpose(pt[:], p1r[:, k * 128:(k + 1) * 128],
                                        I[:RPC, :RPC])
                    if k % 2 == 0:
                        nc.vector.tensor_copy(
                            p1T[:, k * RPC:(k + 1) * RPC], pt[:])
                    else:
                        nc.scalar.copy(p1T[:, k * RPC:(k + 1) * RPC], pt[:])
                if stage == 4:
                    dbg = sb.tile([128, 1], dt, tag="dbg4", name="dbg4")
                    nc.vector.tensor_copy(dbg[:], p1T[:, 0:1])

            if stage >= 5:
                # cost matrix / K / K2 / KC
                fnT = []
                for q in range(4):
                    pt = pst.tile([128, 128], dt, tag="pt", name="ptf")
                    nc.tensor.transpose(pt[:], fn[:, q * 128:(q + 1) * 128],
                                        I[:])
                    fnTq = sb.tile([128, 128], dtb, tag=f"fnT{q}",
                                   name=f"fnT{q}")
                    nc.vector.tensor_copy(fnTq[:], pt[:])
                    fnT.append(fnTq)

                pG = psb.tile([128, 128], dt, tag="big", name="pG")
                for q in range(4):
                    nc.tensor.matmul(pG[:], fnT[q][:], fnT[q][:],
                                     start=(q == 0), stop=(q == 3))
                gmax = sb.tile([128, 1], dt, tag="gmax", name="gmax")
                gmin = sb.tile([128, 1], dt, tag="gmin", name="gmin")
                nc.vector.tensor_reduce(gmax[:], pG[:], axis=ax.X, op=alu.max)
                nc.vector.tensor_reduce(gmin[:], pG[:], axis=ax.X, op=alu.min)
                den = sb.tile([128, 1], dt, tag="den", name="den")
                nc.vector.tensor_sub(den[:], gmax[:], gmin[:])
                rden = sb.tile([128, 1], dt, tag="rden", name="rden")
                nc.vector.reciprocal(rden[:], den[:])
                sA = sb.tile([128, 1], dt, tag="sA", name="sA")
                nc.vector.tensor_scalar_mul(sA[:], rden[:], -GAMMA)
                sB = sb.tile([128, 1], dt, tag="sB", name="sB")
                nc.vector.tensor_scalar(
                    out=sB[:], in0=gmax[:], scalar1=rden[:, 0:1],
                    scalar2=GAMMA, op0=alu.mult, op1=alu.mult)
                costm = sb.tile([128, 128], dt, tag="costm", name="costm")
                nc.vector.tensor_scalar(
                    out=costm[:], in0=pG[:], scalar1=sA[:, 0:1],
                    scalar2=sB[:, 0:1], op0=alu.mult, op1=alu.add)
                nc.gpsimd.tensor_add(costm[:], costm[:], I[:])

                K = sb.tile([128, 128], dt, tag="K", name="K")
                nc.scalar.activation(K[:], costm[:], fp.Exp, scale=-2.0)
                ln128t = sb.tile([128, 1], dt, tag="ln128t", name="ln128t")
                nc.vector.memset(ln128t[:], LN128)
                K2 = sb.tile([128, 128], dtb, tag="K2", name="K2")
                nc.scalar.activation(K2[:], costm[:], fp.Exp,
                                     bias=ln128t[:, 0:1], scale=-2.0)
                ptK = pst.tile([128, 128], dt, tag="pt", name="ptK")
                nc.tensor.transpose(ptK[:], costm[:], I[:])
                costmT = sb.tile([128, 128], dt, tag="costmT", name="costmT")
                nc.vector.tensor_copy(costmT[:], ptK[:])
                KT = sb.tile([128, 128], dtb, tag="KT", name="KT")
                nc.scalar.activation(KT[:], costmT[:], fp.Exp, scale=-2.0)
                KC = sb.tile([128, 128], dtb, tag="KC", name="KC")
                nc.gpsimd.tensor_mul(KC[:], K[:], costm[:])
                if stage == 5:
                    dbg = sb.tile([128, 1], dt, tag="dbg5", name="dbg5")
                    nc.vector.tensor_copy(dbg[:], K[:, 0:1])

            if stage >= 6:
                # Sinkhorn loop
                b = sb.tile([128, MPC], dtb, tag="b0", name="b0")
                nc.vector.memset(b[:], 1.0)
                a = None
                for it in range(SINK_ITR):
                    py = psb.tile([128, MPC], dt, tag="big", name=f"py{it}")
                    nc.tensor.matmul(py[:], KT[:], b[:], start=True, stop=True)
                    r = scr.tile([128, MPC], dt, tag="r", name=f"r{it}")
                    nc.vector.reciprocal_approx_fast(out=r[:], in_=py[:])
                    a = scr.tile([128, MPC], dtb, tag="a", name=f"a{it}")
                    nc.vector.tensor_mul(a[:], p1T[:], r[:])
                    pz = psb.tile([128, MPC], dt, tag="big", name=f"pz{it}")
                    nc.tensor.matmul(pz[:], K2[:], a[:], start=True, stop=True)
                    rb = scr.tile([128, MPC], dt, tag="rb", name=f"rb{it}")
                    nc.vector.reciprocal_approx_fast(out=rb[:], in_=pz[:])
                    b = scr.tile([128, MPC], dtb, tag="b", name=f"b{it}")
                    nc.vector.tensor_copy(b[:], rb[:])
                if stage == 6:
                    dbg = sb.tile([128, 1], dt, tag="dbg6", name="dbg6")
                    nc.vector.tensor_copy(dbg[:], b[:, 0:1])

            if stage >= 7:
                # wcp epilogue
                pw = psb.tile([128, MPC], dt, tag="big", name="pw")
                nc.tensor.matmul(pw[:], KC[:], a[:], start=True, stop=True)
                scrW = scr.tile([128, MPC], dt, tag="r", name="scrW")
                wcp_part = sb.tile([128, 1], dt, tag="wcp_part",
                                   name="wcp_part")
                nc.vector.tensor_mul(scrW[:], pw[:], b[:])
                nc.vector.tensor_reduce(wcp_part[:], scrW[:],
                                        axis=ax.X, op=alu.add)

            # ---------------- pack + store ----------------
            outS = sb.tile([128, 2], dt, tag="outS", name="outS")
            nc.vector.memset(outS[:], 0.0)
            if wcp_part is not None:
                nc.vector.tensor_copy(outS[:, 0:1], wcp_part[:])
            elif dbg is not None:
                p = min(dbg.shape[0], 128)
                nc.vector.tensor_copy(outS[0:p, 0:1], dbg[0:p, 0:1])
            if ce_part is not None:
                nc.vector.tensor_copy(outS[0:RPC, 1:2], ce_part[:])
            nc.sync.dma_start(out=outd[:], in_=outS[:])

    nc.compile()
    return nc


def _get_nc(stage=99):
    key = ("nc", stage)
    if key not in _CACHE:
        _CACHE[key] = _build_nc(stage)
    return _CACHE[key]


def _make_in_maps(features):
    ident = np.eye(128, dtype=np.float32)
    onesr = np.ones((1, 128), dtype=np.float32)
    in_maps = []
    for c in range(NCORES):
        maskce = np.zeros((RPC, B), dtype=np.float32)
        off = (c % 2) * 64
        maskce[np.arange(RPC), off + np.arange(RPC)] = 1.0
        in_maps.append({
            "features": features,
            "fslice": np.ascontiguousarray(features[c * RPC:(c + 1) * RPC, :]),
            "maskce": maskce,
            "ident": ident,
            "onesr": onesr,
        })
    return in_maps


def kernel(features, batch=None, **kwargs):
    from concourse.bass_utils import run_bass_kernel_spmd

    features = np.ascontiguousarray(np.asarray(features, dtype=np.float32))
    assert features.shape == (N, D)

    nc = _get_nc()
    res = run_bass_kernel_spmd(nc, _make_in_maps(features),
                               list(range(NCORES)))

    ce_sum = 0.0
    wcp_sum = 0.0
    for c in range(NCORES):
        o = res.results[c]["out"]
        wcp_sum += float(o[:, 0].sum(dtype=np.float64))
        ce_sum += float(o[:RPC, 1].sum(dtype=np.float64))
    loss = ce_sum / M_TOT + wcp_sum / M_TOT
    return np.float32(loss)


if __name__ == "__main__":
    x = np.random.randn(N, D).astype(np.float32)
    print(kernel(x, B))


# revision 18
# speedup vs baseline: 1.3191x; 1.0465x over previous
"""Trainium2 Bass kernel for the CPN/WCP loss (ce + Sinkhorn wcp).

Strategy:
  - M = 2048 Sinkhorn problems sharded 256/core over 8 cores.
  - Per core: compute its 64-row slab of the NxN (-eudis)/2 matrix via PE
    matmuls (rank-1 matmul folds in the -0.5*sq_j term; the per-row sq_i
    shift is dropped -- softmax/log-softmax are shift invariant).
  - CE pieces (row LSE at temp 5, target logit) computed in row layout.
  - Softmax p1 computed in row layout, transposed to [128 class, 256 prob]
    via PE transposes.
  - Sinkhorn runs in multiplicative form: a = p1 / (K@b), b = p2 / (K^T@a)
    with K = exp(-2*cost) fixed => two matmuls + DVE approx-reciprocals per
    iteration, no transcendentals in the loop.
  - wcp_m = ((K.C)^T a) . b ; per-partition partials DMA'd out, host sums.
"""

import sys

for _p in ("/opt/trn_rl_repo",):
    if _p not in sys.path:
        sys.path.insert(0, _p)

import numpy as np

AUG = 4
B = 128
D = 512
N = AUG * B          # 512 feature rows
NCORES = 8
RPC = N // NCORES    # 64 eudis rows per core
MPC = RPC * AUG      # 256 sinkhorn problems per core
M_TOT = N * AUG      # 2048
TEMP = 5.0
GAMMA = 0.2
SINK_ITR = 5
SCALE1 = 2.0 / float(np.sqrt(np.float32(D)))  # softmax scale on h
SCALE5 = 2.0 / TEMP                            # CE scale on h
LN128 = float(np.log(128.0))

_CACHE = {}


def _build_nc(stage=99):
    import concourse.bacc as bacc
    import concourse.tile as tile
    import concourse.mybir as mybir
    from concourse.dve_ops import (RECIP_APPROX_FAST_CONSTS as _RAFC,
                                   RECIPROCAL_APPROX_FAST as _RAF)

    dt = mybir.dt.float32
    dtr = mybir.dt.float32r
    dtb = mybir.dt.bfloat16
    fp = mybir.ActivationFunctionType
    alu = mybir.AluOpType
    ax = mybir.AxisListType

    nc = bacc.Bacc(
        "TRN2",
        target_bir_lowering=False,
        debug=False,
        enable_asserts=False,
        num_devices=NCORES,
    )

    feat = nc.dram_tensor("features", [N, D], dt, kind="ExternalInput").ap()
    fsl = nc.dram_tensor("fslice", [RPC, D], dt, kind="ExternalInput").ap()
    mce = nc.dram_tensor("maskce", [RPC, B], dt, kind="ExternalInput").ap()
    idn = nc.dram_tensor("ident", [128, 128], dt, kind="ExternalInput").ap()
    onr = nc.dram_tensor("onesr", [1, 128], dtr, kind="ExternalInput").ap()
    outd = nc.dram_tensor("out", [128, 2], dt, kind="ExternalOutput").ap()

    with tile.TileContext(nc) as tc:
        with (
            tc.tile_pool(name="sb", bufs=1) as sb,
            tc.tile_pool(name="scr", bufs=2) as scr,
            tc.tile_pool(name="ps_big", bufs=2, space="PSUM") as psb,
            tc.tile_pool(name="ps_t", bufs=3, space="PSUM") as pst,
            tc.tile_pool(name="ps_h", bufs=1, space="PSUM") as psh,
        ):
            dbg = None  # [*,1] tile flushed to out col0 for stage bisection

            # Preload the combined exp+ln ACT table set so the compiler's
            # per-func set picker doesn't ping-pong exp_and_others <->
            # natural_log (each reload costs ~2.7us).
            _tabs = list(__import__("concourse.hw_specs",
                                    fromlist=["hw_specs"]
                                    ).get_activation_tables(nc.m.arch))
            _set_id = _tabs.index("natural_log_exp_and_others")
            nc.scalar.add_instruction(mybir.InstLoadActFuncSet(
                name=nc.get_next_instruction_name(), ins=[], outs=[],
                act_func_set_id=_set_id))

            # ---------------- loads ----------------
            # spread across 4 issuing engines => 4 HWDGE queues in parallel;
            # ident + F0 first (they gate the PE transpose chain).
            I = sb.tile([128, 128], dt, tag="I", name="I")
            nc.sync.dma_start(out=I[:], in_=idn[:])
            F = []
            engs = [nc.gpsimd, nc.scalar, nc.sync, nc.gpsimd]
            for t in range(4):
                Ft = sb.tile([128, D], dt, tag=f"F{t}", name=f"F{t}")
                engs[t].dma_start(out=Ft[:], in_=feat[t * 128:(t + 1) * 128, :])
                F.append(Ft)
            fs = sb.tile([RPC, D], dt, tag="fs", name="fs")
            nc.gpsimd.dma_start(out=fs[:], in_=fsl[:])
            mk = sb.tile([RPC, B], dt, tag="mk", name="mk")
            nc.gpsimd.dma_start(out=mk[:], in_=mce[:])
            ones1 = sb.tile([1, 128], dtr, tag="ones1", name="ones1")
            nc.scalar.dma_start(out=ones1[:], in_=onr[:])

            ce_part = None
            wcp_part = None

            if stage >= 1:
                # ---------------- F^T tiles ----------------
                FT = []
                for q in range(4):
                    FTq = sb.tile([128, D], dtr, tag=f"FT{q}", name=f"FT{q}")
                    FT.append(FTq)
                for t in range(4):
                    for q in range(4):
                        pt = pst.tile([128, 128], dt, tag="pt", name="pt")
                        nc.tensor.transpose(
                            pt[:], F[t][:, q * 128:(q + 1) * 128], I[:])
                        if (t + q) % 2 == 0:
                            nc.scalar.copy(
                                FT[q][:, t * 128:(t + 1) * 128], pt[:])
                        else:
                            nc.vector.tensor_copy(
                                FT[q][:, t * 128:(t + 1) * 128], pt[:])

                fsT = []
                for q in range(4):
                    pt = pst.tile([128, RPC], dt, tag="pt", name="pt")
                    nc.tensor.transpose(
                        pt[:], fs[:, q * 128:(q + 1) * 128], I[:RPC, :RPC])
                    fsTq = sb.tile([128, RPC], dtr, tag=f"fsT{q}",
                                   name=f"fsT{q}")
                    nc.vector.tensor_copy(fsTq[:], pt[:])
                    fsT.append(fsTq)

                # sq_j row: -0.5 * sum_d F[j,:]^2
                sqc = sb.tile([128, 4], dt, tag="sqc", name="sqc")
                for t in range(4):
                    scrF = scr.tile([128, D], dt, tag="scrF", name="scrF")
                    nc.scalar.activation(scrF[:], F[t][:], fp.Square,
                                         accum_out=sqc[:, t:t + 1])
                pt4 = pst.tile([4, 128], dt, tag="pt", name="pt4")
                nc.tensor.transpose(pt4[:], sqc[:], I[:])
                s4 = sb.tile([4, 128], dtr, tag="s4", name="s4")
                nc.vector.tensor_scalar_mul(s4[:], pt4[:], -0.5)
                sqrow = sb.tile([1, D], dtr, tag="sqrow", name="sqrow")
                for t in range(4):
                    nc.sync.dma_start(out=sqrow[0:1, t * 128:(t + 1) * 128],
                                      in_=s4[t:t + 1, :])

                # mean-feature branch (gpsimd: off the DVE critical path)
                g = sb.tile([128, D], dt, tag="g", name="g")
                g2 = sb.tile([128, D], dt, tag="g2", name="g2")
                nc.gpsimd.tensor_add(g2[:], F[0][:], F[1][:])
                nc.gpsimd.tensor_add(g[:], F[2][:], F[3][:])
                nc.gpsimd.tensor_add(g[:], g[:], g2[:])
                gsq = scr.tile([128, D], dt, tag="scrF", name="gsq")
                ssg = sb.tile([128, 1], dt, tag="ssg", name="ssg")
                nc.scalar.activation(gsq[:], g[:], fp.Square,
                                     accum_out=ssg[:])
                lssg = sb.tile([128, 1], dt, tag="lssg", name="lssg")
                nc.scalar.activation(lssg[:], ssg[:], fp.Ln)
                rn = sb.tile([128, 1], dt, tag="rn", name="rn")
                nc.scalar.activation(rn[:], lssg[:], fp.Exp, scale=-0.5)
                fn = sb.tile([128, D], dt, tag="fn", name="fn")
                nc.vector.tensor_scalar_mul(fn[:], g[:], rn[:, 0:1])
                dbg = sqc

            if stage >= 2:
                # dist slab: h2 = dot - 0.5*sq_j  [64, 512]
                ph = psh.tile([RPC, D], dt, tag="ph", name="ph")
                for q in range(4):
                    nc.tensor.matmul(ph[:], fsT[q][:], FT[q][:],
                                     start=(q == 0), stop=False)
                nc.tensor.matmul(ph[:], ones1[0:1, 0:RPC], sqrow[0:1, :],
                                 start=False, stop=True)
                if stage == 2:
                    dbg = sb.tile([RPC, 1], dt, tag="dbg2", name="dbg2")
                    nc.vector.tensor_copy(dbg[:], ph[:, 0:1])

            if stage >= 3:
                # row stats / CE
                mh = sb.tile([RPC, 4], dt, tag="mh", name="mh")
                nc.vector.tensor_reduce(
                    mh[:], ph[:].rearrange("p (k x) -> p k x", k=4),
                    axis=ax.X, op=alu.max)
                bias1 = sb.tile([RPC, 4], dt, tag="bias1", name="bias1")
                nc.vector.tensor_scalar_mul(bias1[:], mh[:], -SCALE1)
                bias5 = sb.tile([RPC, 4], dt, tag="bias5", name="bias5")
                nc.vector.tensor_scalar_mul(bias5[:], mh[:], -SCALE5)

                E1 = sb.tile([RPC, D], dt, tag="E1", name="E1")
                E2 = sb.tile([RPC, D], dt, tag="E2", name="E2")
                for k in range(4):
                    ksl = slice(k * 128, (k + 1) * 128)
                    nc.scalar.activation(E1[:, ksl], ph[:, ksl], fp.Exp,
                                         bias=bias1[:, k:k + 1], scale=SCALE1)
                    nc.scalar.activation(E2[:, ksl], ph[:, ksl], fp.Exp,
                                         bias=bias5[:, k:k + 1], scale=SCALE5)
                S1 = sb.tile([RPC, 4], dt, tag="S1", name="S1")
                nc.vector.tensor_reduce(
                    S1[:], E1[:].rearrange("p (k x) -> p k x", k=4),
                    axis=ax.X, op=alu.add)
                S5 = sb.tile([RPC, 4], dt, tag="S5", name="S5")
                nc.vector.tensor_reduce(
                    S5[:], E2[:].rearrange("p (k x) -> p k x", k=4),
                    axis=ax.X, op=alu.add)

                # ce_m = ln(S5) - (SCALE5/SCALE1)*ln(E1[target]):
                # E1[target] = exp(SCALE1*(h_t - mh)) so this equals
                # ln(S5) + SCALE5*(mh - h_t).
                E1m = scr.tile([RPC, D], dt, tag="scrE", name="E1m")
                for k in range(4):
                    ksl = slice(k * 128, (k + 1) * 128)
                    nc.gpsimd.tensor_mul(E1m[:, ksl], E1[:, ksl], mk[:])
                Ed = sb.tile([RPC, 4], dt, tag="Ed", name="Ed")
                nc.vector.tensor_reduce(
                    Ed[:], E1m[:].rearrange("p (k x) -> p k x", k=4),
                    axis=ax.X, op=alu.add)
                lnS5 = sb.tile([RPC, 4], dt, tag="lnS5", name="lnS5")
                nc.scalar.activation(lnS5[:], S5[:], fp.Ln)
                lnEd = sb.tile([RPC, 4], dt, tag="lnEd", name="lnEd")
                nc.scalar.activation(lnEd[:], Ed[:], fp.Ln)
                ce4 = sb.tile([RPC, 4], dt, tag="ce4", name="ce4")
                nc.vector.scalar_tensor_tensor(
                    out=ce4[:], in0=lnEd[:], scalar=-(SCALE5 / SCALE1),
                    in1=lnS5[:], op0=alu.mult, op1=alu.add)
                ce_part = sb.tile([RPC, 1], dt, tag="ce_part", name="ce_part")
                nc.vector.tensor_reduce(ce_part[:], ce4[:], axis=ax.X,
                                        op=alu.add)
                dbg = ce_part

            if stage >= 4:
                # p1 = softmax + 1e-12, transposed to [128, 256]
                rS1 = sb.tile([RPC, 4], dt, tag="rS1", name="rS1")
                nc.vector.reciprocal(rS1[:], S1[:])
                p1r = sb.tile([RPC, D], dt, tag="p1r", name="p1r")
                for k in range(4):
                    ksl = slice(k * 128, (k + 1) * 128)
                    nc.vector.tensor_scalar(
                        out=p1r[:, ksl], in0=E1[:, ksl],
                        scalar1=rS1[:, k:k + 1], scalar2=1e-12,
                        op0=alu.mult, op1=alu.add)
                p1T = sb.tile([128, MPC], dtb, tag="p1T", name="p1T")
                for k in range(4):
                    pt = pst.tile([128, RPC], dt, tag="pt", name="ptp")
                    nc.tensor.transpose(pt[:], p1r[:, k * 128:(k + 1) * 128],
                                        I[:RPC, :RPC])
                    if k % 2 == 0:
                        nc.vector.tensor_copy(
                            p1T[:, k * RPC:(k + 1) * RPC], pt[:])
                    else:
                        nc.scalar.copy(p1T[:, k * RPC:(k + 1) * RPC], pt[:])
                if stage == 4:
                    dbg = sb.tile([128, 1], dt, tag="dbg4", name="dbg4")
                    nc.vector.tensor_copy(dbg[:], p1T[:, 0:1])

            if stage >= 5:
                # cost matrix / K / K2 / KC
                fnT = []
                for q in range(4):
                    pt = pst.tile([128, 128], dt, tag="pt", name="ptf")
                    nc.tensor.transpose(pt[:], fn[:, q * 128:(q + 1) * 128],
                                        I[:])
                    fnTq = sb.tile([128, 128], dtb, tag=f"fnT{q}",
                                   name=f"fnT{q}")
                    nc.vector.tensor_copy(fnTq[:], pt[:])
                    fnT.append(fnTq)

                pG = psb.tile([128, 128], dt, tag="big", name="pG")
                for q in range(4):
                    nc.tensor.matmul(pG[:], fnT[q][:], fnT[q][:],
                                     start=(q == 0), stop=(q == 3))
                gmax = sb.tile([128, 1], dt, tag="gmax", name="gmax")
                gmin = sb.tile([128, 1], dt, tag="gmin", name="gmin")
                nc.vector.tensor_reduce(gmax[:], pG[:], axis=ax.X, op=alu.max)
                nc.vector.tensor_reduce(gmin[:], pG[:], axis=ax.X, op=alu.min)
                den = sb.tile([128, 1], dt, tag="den", name="den")
                nc.vector.tensor_sub(den[:], gmax[:], gmin[:])
                rden = sb.tile([128, 1], dt, tag="rden", name="rden")
                nc.vector.reciprocal(rden[:], den[:])
                sA = sb.tile([128, 1], dt, tag="sA", name="sA")
                nc.vector.tensor_scalar_mul(sA[:], rden[:], -GAMMA)
                sB = sb.tile([128, 1], dt, tag="sB", name="sB")
                nc.vector.tensor_scalar(
                    out=sB[:], in0=gmax[:], scalar1=rden[:, 0:1],
                    scalar2=GAMMA, op0=alu.mult, op1=alu.mult)
                costm = sb.tile([128, 128], dt, tag="costm", name="costm")
                nc.vector.tensor_scalar(
                    out=costm[:], in0=pG[:], scalar1=sA[:, 0:1],
                    scalar2=sB[:, 0:1], op0=alu.mult, op1=alu.add)
                nc.vector.tensor_add(costm[:], costm[:], I[:])

                K = sb.tile([128, 128], dt, tag="K", name="K")
                nc.scalar.activation(K[:], costm[:], fp.Exp, scale=-2.0)
                ln128t = sb.tile([128, 1], dt, tag="ln128t", name="ln128t")
                nc.vector.memset(ln128t[:], LN128)
                K2 = sb.tile([128, 128], dtb, tag="K2", name="K2")
                nc.scalar.activation(K2[:], costm[:], fp.Exp,
                                     bias=ln128t[:, 0:1], scale=-2.0)
                ptK = pst.tile([128, 128], dt, tag="pt", name="ptK")
                nc.tensor.transpose(ptK[:], costm[:], I[:])
                costmT = sb.tile([128, 128], dt, tag="costmT", name="costmT")
                nc.vector.tensor_copy(costmT[:], ptK[:])
                KT = sb.tile([128, 128], dtb, tag="KT", name="KT")
                nc.scalar.activation(KT[:], costmT[:], fp.Exp, scale=-2.0)
                KC = sb.tile([128, 128], dtb, tag="KC", name="KC")
                nc.gpsimd.tensor_mul(KC[:], K[:], costm[:])
                if stage == 5:
                    dbg = sb.tile([128, 1], dt, tag="dbg5", name="dbg5")
                    nc.vector.tensor_copy(dbg[:], K[:, 0:1])

            if stage >= 6:
                # Sinkhorn loop
                b = sb.tile([128, MPC], dtb, tag="b0", name="b0")
                nc.vector.memset(b[:], 1.0)
                a = None
                for it in range(SINK_ITR):
                    py = psb.tile([128, MPC], dt, tag="big", name=f"py{it}")
                    nc.tensor.matmul(py[:], KT[:], b[:], start=True, stop=True)
                    r = scr.tile([128, MPC], dt, tag="r", name=f"r{it}")
                    nc.vector.reciprocal_approx_fast(out=r[:], in_=py[:])
                    a = scr.tile([128, MPC], dtb, tag="a", name=f"a{it}")
                    nc.vector.tensor_mul(a[:], p1T[:], r[:])
                    pz = psb.tile([128, MPC], dt, tag="big", name=f"pz{it}")
                    nc.tensor.matmul(pz[:], K2[:], a[:], start=True, stop=True)
                    b = scr.tile([128, MPC], dtb, tag="b", name=f"b{it}")
                    _c = _RAFC
                    nc.vector._custom_dve(_RAF, out=b[:], in0=pz[:],
                                          s0=_c["s0"], s1=_c["s1"],
                                          imm2=_c["imm2"])
                if stage == 6:
                    dbg = sb.tile([128, 1], dt, tag="dbg6", name="dbg6")
                    nc.vector.tensor_copy(dbg[:], b[:, 0:1])

            if stage >= 7:
                # wcp epilogue
                pw = psb.tile([128, MPC], dt, tag="big", name="pw")
                nc.tensor.matmul(pw[:], KC[:], a[:], start=True, stop=True)
                scrW = scr.tile([128, MPC], dt, tag="r", name="scrW")
                wcp_part = sb.tile([128, 1], dt, tag="wcp_part",
                                   name="wcp_part")
                nc.vector.tensor_mul(scrW[:], pw[:], b[:])
                nc.vector.tensor_reduce(wcp_part[:], scrW[:],
                                        axis=ax.X, op=alu.add)

            # ---------------- pack + store ----------------
            outS = sb.tile([128, 2], dt, tag="outS", name="outS")
            nc.vector.memset(outS[:], 0.0)
            if wcp_part is not None:
                nc.vector.tensor_copy(outS[:, 0:1], wcp_part[:])
            elif dbg is not None:
                p = min(dbg.shape[0], 128)
                nc.vector.tensor_copy(outS[0:p, 0:1], dbg[0:p, 0:1])
            if ce_part is not None:
                nc.vector.tensor_copy(outS[0:RPC, 1:2], ce_part[:])
            nc.sync.dma_start(out=outd[:], in_=outS[:])

    nc.compile()
    return nc


def _get_nc(stage=99):
    key = ("nc", stage)
    if key not in _CACHE:
        _CACHE[key] = _build_nc(stage)
    return _CACHE[key]


def _make_in_maps(features):
    ident = np.eye(128, dtype=np.float32)
    onesr = np.ones((1, 128), dtype=np.float32)
    in_maps = []
    for c in range(NCORES):
        maskce = np.zeros((RPC, B), dtype=np.float32)
        off = (c % 2) * 64
        maskce[np.arange(RPC), off + np.arange(RPC)] = 1.0
        in_maps.append({
            "features": features,
            "fslice": np.ascontiguousarray(features[c * RPC:(c + 1) * RPC, :]),
            "maskce": maskce,
            "ident": ident,
            "onesr": onesr,
        })
    return in_maps


def kernel(features, batch=None, **kwargs):
    from concourse.bass_utils import run_bass_kernel_spmd

    features = np.ascontiguousarray(np.asarray(features, dtype=np.float32))
    assert features.shape == (N, D)

    nc = _get_nc()
    res = run_bass_kernel_spmd(nc, _make_in_maps(features),
                               list(range(NCORES)))

    ce_sum = 0.0
    wcp_sum = 0.0
    for c in range(NCORES):
        o = res.results[c]["out"]
        wcp_sum += float(o[:, 0].sum(dtype=np.float64))
        ce_sum += float(o[:RPC, 1].sum(dtype=np.float64))
    loss = ce_sum / M_TOT + wcp_sum / M_TOT
    return np.float32(loss)


if __name__ == "__main__":
    x = np.random.randn(N, D).astype(np.float32)
    print(kernel(x, B))


# revision 21
# speedup vs baseline: 1.4282x; 1.0827x over previous
"""Trainium2 Bass kernel for the CPN/WCP loss (ce + Sinkhorn wcp).

Strategy:
  - M = 2048 Sinkhorn problems sharded 256/core over 8 cores.
  - Per core: compute its 64-row slab of the NxN (-eudis)/2 matrix via PE
    matmuls (rank-1 matmul folds in the -0.5*sq_j term; the per-row sq_i
    shift is dropped -- softmax/log-softmax are shift invariant).
  - CE pieces (row LSE at temp 5, target logit) computed in row layout.
  - Softmax p1 computed in row layout, transposed to [128 class, 256 prob]
    via PE transposes.
  - Sinkhorn runs in multiplicative form: a = p1 / (K@b), b = p2 / (K^T@a)
    with K = exp(-2*cost) fixed => two matmuls + DVE approx-reciprocals per
    iteration, no transcendentals in the loop.
  - wcp_m = ((K.C)^T a) . b ; per-partition partials DMA'd out, host sums.
"""

import sys

for _p in ("/opt/trn_rl_repo",):
    if _p not in sys.path:
        sys.path.insert(0, _p)

import numpy as np

AUG = 4
B = 128
D = 512
N = AUG * B          # 512 feature rows
NCORES = 8
RPC = N // NCORES    # 64 eudis rows per core
MPC = RPC * AUG      # 256 sinkhorn problems per core
M_TOT = N * AUG      # 2048
TEMP = 5.0
GAMMA = 0.2
SINK_ITR = 5
SCALE1 = 2.0 / float(np.sqrt(np.float32(D)))  # softmax scale on h
SCALE5 = 2.0 / TEMP                            # CE scale on h
LN128 = float(np.log(128.0))

_CACHE = {}


def _build_nc(stage=99):
    import concourse.bacc as bacc
    import concourse.tile as tile
    import concourse.mybir as mybir
    from concourse.dve_ops import (RECIP_APPROX_FAST_CONSTS as _RAFC,
                                   RECIPROCAL_APPROX_FAST as _RAF)

    dt = mybir.dt.float32
    dtr = mybir.dt.float32r
    dtb = mybir.dt.bfloat16
    fp = mybir.ActivationFunctionType
    alu = mybir.AluOpType
    ax = mybir.AxisListType

    nc = bacc.Bacc(
        "TRN2",
        target_bir_lowering=False,
        debug=False,
        enable_asserts=False,
        num_devices=NCORES,
    )

    feat = nc.dram_tensor("features", [N, D], dt, kind="ExternalInput").ap()
    fsl = nc.dram_tensor("fslice", [RPC, D], dt, kind="ExternalInput").ap()
    mce = nc.dram_tensor("maskce", [RPC, B], dt, kind="ExternalInput").ap()
    onr = nc.dram_tensor("onesr", [1, 128], dtr, kind="ExternalInput").ap()
    outd = nc.dram_tensor("out", [128, 2], dt, kind="ExternalOutput").ap()

    with tile.TileContext(nc) as tc:
        with (
            tc.tile_pool(name="sb", bufs=1) as sb,
            tc.tile_pool(name="scr", bufs=2) as scr,
            tc.tile_pool(name="ps_big", bufs=2, space="PSUM") as psb,
            tc.tile_pool(name="ps_t", bufs=3, space="PSUM") as pst,
            tc.tile_pool(name="ps_h", bufs=1, space="PSUM") as psh,
        ):
            dbg = None  # [*,1] tile flushed to out col0 for stage bisection

            # Preload the combined exp+ln ACT table set so the compiler's
            # per-func set picker doesn't ping-pong exp_and_others <->
            # natural_log (each reload costs ~2.7us).
            _tabs = list(__import__("concourse.hw_specs",
                                    fromlist=["hw_specs"]
                                    ).get_activation_tables(nc.m.arch))
            _set_id = _tabs.index("natural_log_exp_and_others")
            nc.scalar.add_instruction(mybir.InstLoadActFuncSet(
                name=nc.get_next_instruction_name(), ins=[], outs=[],
                act_func_set_id=_set_id))

            # ---------------- loads ----------------
            # identity generated on-chip (a [128,128] DMA costs ~4us of
            # descriptor processing); F tiles split into halves across the
            # 3 DMA-issuing engines so the first tiles land early.
            ones_t = sb.tile([128, 128], dt, tag="ones_t", name="ones_t")
            nc.vector.memset(ones_t[:], 1.0)
            I = sb.tile([128, 128], dt, tag="I", name="I")
            nc.gpsimd.affine_select(I[:], ones_t[:], [[1, 128]],
                                    alu.is_equal, 0.0, base=0,
                                    channel_multiplier=-1)
            F = []
            for t in range(4):
                Ft = sb.tile([128, D], dt, tag=f"F{t}", name=f"F{t}")
                F.append(Ft)
            halves = [(0, 0, nc.sync), (0, 1, nc.gpsimd), (1, 0, nc.scalar),
                      (1, 1, nc.sync), (2, 0, nc.gpsimd), (2, 1, nc.scalar),
                      (3, 0, nc.sync), (3, 1, nc.gpsimd)]
            for t, h, eng in halves:
                eng.dma_start(
                    out=F[t][h * 64:(h + 1) * 64, :],
                    in_=feat[t * 128 + h * 64:t * 128 + (h + 1) * 64, :])
            fs = sb.tile([RPC, D], dt, tag="fs", name="fs")
            nc.scalar.dma_start(out=fs[:], in_=fsl[:])
            mk = sb.tile([RPC, B], dt, tag="mk", name="mk")
            nc.gpsimd.dma_start(out=mk[:], in_=mce[:])
            ones1 = sb.tile([1, 128], dtr, tag="ones1", name="ones1")
            nc.sync.dma_start(out=ones1[:], in_=onr[:])

            ce_part = None
            wcp_part = None

            if stage >= 1:
                # ---------------- F^T tiles ----------------
                FT = []
                for q in range(4):
                    FTq = sb.tile([128, D], dtr, tag=f"FT{q}", name=f"FT{q}")
                    FT.append(FTq)
                for t in range(4):
                    for q in range(4):
                        pt = pst.tile([128, 128], dt, tag="pt", name="pt")
                        nc.tensor.transpose(
                            pt[:], F[t][:, q * 128:(q + 1) * 128], I[:])
                        nc.vector.tensor_copy(
                            FT[q][:, t * 128:(t + 1) * 128], pt[:])

                fsT = []
                for q in range(4):
                    pt = pst.tile([128, RPC], dt, tag="pt", name="pt")
                    nc.tensor.transpose(
                        pt[:], fs[:, q * 128:(q + 1) * 128], I[:RPC, :RPC])
                    fsTq = sb.tile([128, RPC], dtr, tag=f"fsT{q}",
                                   name=f"fsT{q}")
                    nc.vector.tensor_copy(fsTq[:], pt[:])
                    fsT.append(fsTq)

                # sq_j row: -0.5 * sum_d F[j,:]^2
                sqc = sb.tile([128, 4], dt, tag="sqc", name="sqc")
                for t in range(4):
                    scrF = scr.tile([128, D], dt, tag="scrF", name="scrF")
                    nc.scalar.activation(scrF[:], F[t][:], fp.Square,
                                         accum_out=sqc[:, t:t + 1])
                pt4 = pst.tile([4, 128], dt, tag="pt", name="pt4")
                nc.tensor.transpose(pt4[:], sqc[:], I[:])
                s4 = sb.tile([4, 128], dtr, tag="s4", name="s4")
                nc.vector.tensor_scalar_mul(s4[:], pt4[:], -0.5)
                sqrow = sb.tile([1, D], dtr, tag="sqrow", name="sqrow")
                sq_engs = [nc.sync, nc.gpsimd, nc.scalar, nc.sync]
                for t in range(4):
                    sq_engs[t].dma_start(
                        out=sqrow[0:1, t * 128:(t + 1) * 128],
                        in_=s4[t:t + 1, :])

                # mean-feature branch (gpsimd: off the DVE critical path)
                g = sb.tile([128, D], dt, tag="g", name="g")
                g2 = sb.tile([128, D], dt, tag="g2", name="g2")
                nc.gpsimd.tensor_add(g2[:], F[0][:], F[1][:])
                nc.gpsimd.tensor_add(g[:], F[2][:], F[3][:])
                nc.gpsimd.tensor_add(g[:], g[:], g2[:])
                gsq = scr.tile([128, D], dt, tag="scrF", name="gsq")
                ssg = sb.tile([128, 1], dt, tag="ssg", name="ssg")
                nc.scalar.activation(gsq[:], g[:], fp.Square,
                                     accum_out=ssg[:])
                lssg = sb.tile([128, 1], dt, tag="lssg", name="lssg")
                nc.scalar.activation(lssg[:], ssg[:], fp.Ln)
                rn = sb.tile([128, 1], dt, tag="rn", name="rn")
                nc.scalar.activation(rn[:], lssg[:], fp.Exp, scale=-0.5)
                fn = sb.tile([128, D], dt, tag="fn", name="fn")
                nc.vector.tensor_scalar_mul(fn[:], g[:], rn[:, 0:1])
                dbg = sqc

            if stage >= 2:
                # dist slab: h2 = dot - 0.5*sq_j  [64, 512]
                ph = psh.tile([RPC, D], dt, tag="ph", name="ph")
                for q in range(4):
                    nc.tensor.matmul(ph[:], fsT[q][:], FT[q][:],
                                     start=(q == 0), stop=False)
                nc.tensor.matmul(ph[:], ones1[0:1, 0:RPC], sqrow[0:1, :],
                                 start=False, stop=True)
                if stage == 2:
                    dbg = sb.tile([RPC, 1], dt, tag="dbg2", name="dbg2")
                    nc.vector.tensor_copy(dbg[:], ph[:, 0:1])

            if stage >= 3:
                # row stats / CE
                mh = sb.tile([RPC, 4], dt, tag="mh", name="mh")
                nc.vector.tensor_reduce(
                    mh[:], ph[:].rearrange("p (k x) -> p k x", k=4),
                    axis=ax.X, op=alu.max)
                bias1 = sb.tile([RPC, 4], dt, tag="bias1", name="bias1")
                nc.vector.tensor_scalar_mul(bias1[:], mh[:], -SCALE1)
                bias5 = sb.tile([RPC, 4], dt, tag="bias5", name="bias5")
                nc.vector.tensor_scalar_mul(bias5[:], mh[:], -SCALE5)

                E1 = sb.tile([RPC, D], dt, tag="E1", name="E1")
                E2 = sb.tile([RPC, D], dt, tag="E2", name="E2")
                for k in range(4):
                    ksl = slice(k * 128, (k + 1) * 128)
                    nc.scalar.activation(E1[:, ksl], ph[:, ksl], fp.Exp,
                                         bias=bias1[:, k:k + 1], scale=SCALE1)
                    nc.scalar.activation(E2[:, ksl], ph[:, ksl], fp.Exp,
                                         bias=bias5[:, k:k + 1], scale=SCALE5)
                S1 = sb.tile([RPC, 4], dt, tag="S1", name="S1")
                nc.vector.tensor_reduce(
                    S1[:], E1[:].rearrange("p (k x) -> p k x", k=4),
                    axis=ax.X, op=alu.add)
                S5 = sb.tile([RPC, 4], dt, tag="S5", name="S5")
                nc.vector.tensor_reduce(
                    S5[:], E2[:].rearrange("p (k x) -> p k x", k=4),
                    axis=ax.X, op=alu.add)

                # ce_m = ln(S5) - (SCALE5/SCALE1)*ln(E1[target]):
                # E1[target] = exp(SCALE1*(h_t - mh)) so this equals
                # ln(S5) + SCALE5*(mh - h_t).
                E1m = scr.tile([RPC, D], dt, tag="scrE", name="E1m")
                for k in range(4):
                    ksl = slice(k * 128, (k + 1) * 128)
                    nc.gpsimd.tensor_mul(E1m[:, ksl], E1[:, ksl], mk[:])
                Ed = sb.tile([RPC, 4], dt, tag="Ed", name="Ed")
                nc.vector.tensor_reduce(
                    Ed[:], E1m[:].rearrange("p (k x) -> p k x", k=4),
                    axis=ax.X, op=alu.add)
                lnS5 = sb.tile([RPC, 4], dt, tag="lnS5", name="lnS5")
                nc.scalar.activation(lnS5[:], S5[:], fp.Ln)
                lnEd = sb.tile([RPC, 4], dt, tag="lnEd", name="lnEd")
                nc.scalar.activation(lnEd[:], Ed[:], fp.Ln)
                ce4 = sb.tile([RPC, 4], dt, tag="ce4", name="ce4")
                nc.vector.scalar_tensor_tensor(
                    out=ce4[:], in0=lnEd[:], scalar=-(SCALE5 / SCALE1),
                    in1=lnS5[:], op0=alu.mult, op1=alu.add)
                ce_part = sb.tile([RPC, 1], dt, tag="ce_part", name="ce_part")
                nc.vector.tensor_reduce(ce_part[:], ce4[:], axis=ax.X,
                                        op=alu.add)
                dbg = ce_part

            if stage >= 4:
                # p1 = softmax + 1e-12, transposed to [128, 256]
                rS1 = sb.tile([RPC, 4], dt, tag="rS1", name="rS1")
                nc.vector.reciprocal(rS1[:], S1[:])
                p1r = sb.tile([RPC, D], dt, tag="p1r", name="p1r")
                for k in range(4):
                    ksl = slice(k * 128, (k + 1) * 128)
                    nc.vector.tensor_scalar(
                        out=p1r[:, ksl], in0=E1[:, ksl],
                        scalar1=rS1[:, k:k + 1], scalar2=1e-12,
                        op0=alu.mult, op1=alu.add)
                p1T = sb.tile([128, MPC], dtb, tag="p1T", name="p1T")
                for k in range(4):
                    pt = pst.tile([128, RPC], dt, tag="pt", name="ptp")
                    nc.tensor.transpose(pt[:], p1r[:, k * 128:(k + 1) * 128],
                                        I[:RPC, :RPC])
                    if k % 2 == 0:
                        nc.vector.tensor_copy(
                            p1T[:, k * RPC:(k + 1) * RPC], pt[:])
                    else:
                        nc.scalar.copy(p1T[:, k * RPC:(k + 1) * RPC], pt[:])
                if stage == 4:
                    dbg = sb.tile([128, 1], dt, tag="dbg4", name="dbg4")
                    nc.vector.tensor_copy(dbg[:], p1T[:, 0:1])

            if stage >= 5:
                # cost matrix / K / K2 / KC
                fnT = []
                for q in range(4):
                    pt = pst.tile([128, 128], dt, tag="pt", name="ptf")
                    nc.tensor.transpose(pt[:], fn[:, q * 128:(q + 1) * 128],
                                        I[:])
                    fnTq = sb.tile([128, 128], dtb, tag=f"fnT{q}",
                                   name=f"fnT{q}")
                    nc.vector.tensor_copy(fnTq[:], pt[:])
                    fnT.append(fnTq)

                pG = psb.tile([128, 128], dt, tag="big", name="pG")
                for q in range(4):
                    nc.tensor.matmul(pG[:], fnT[q][:], fnT[q][:],
                                     start=(q == 0), stop=(q == 3))
                gmax = sb.tile([128, 1], dt, tag="gmax", name="gmax")
                gmin = sb.tile([128, 1], dt, tag="gmin", name="gmin")
                nc.vector.tensor_reduce(gmax[:], pG[:], axis=ax.X, op=alu.max)
                nc.vector.tensor_reduce(gmin[:], pG[:], axis=ax.X, op=alu.min)
                den = sb.tile([128, 1], dt, tag="den", name="den")
                nc.vector.tensor_sub(den[:], gmax[:], gmin[:])
                rden = sb.tile([128, 1], dt, tag="rden", name="rden")
                nc.vector.reciprocal(rden[:], den[:])
                sA = sb.tile([128, 1], dt, tag="sA", name="sA")
                nc.vector.tensor_scalar_mul(sA[:], rden[:], -GAMMA)
                sB = sb.tile([128, 1], dt, tag="sB", name="sB")
                nc.vector.tensor_scalar(
                    out=sB[:], in0=gmax[:], scalar1=rden[:, 0:1],
                    scalar2=GAMMA, op0=alu.mult, op1=alu.mult)
                costm = sb.tile([128, 128], dt, tag="costm", name="costm")
                nc.vector.tensor_scalar(
                    out=costm[:], in0=pG[:], scalar1=sA[:, 0:1],
                    scalar2=sB[:, 0:1], op0=alu.mult, op1=alu.add)
                nc.vector.tensor_add(costm[:], costm[:], I[:])

                K = sb.tile([128, 128], dt, tag="K", name="K")
                nc.scalar.activation(K[:], costm[:], fp.Exp, scale=-2.0)
                ln128t = sb.tile([128, 1], dt, tag="ln128t", name="ln128t")
                nc.vector.memset(ln128t[:], LN128)
                K2 = sb.tile([128, 128], dtb, tag="K2", name="K2")
                nc.scalar.activation(K2[:], costm[:], fp.Exp,
                                     bias=ln128t[:, 0:1], scale=-2.0)
                ptK = pst.tile([128, 128], dt, tag="pt", name="ptK")
                nc.tensor.transpose(ptK[:], costm[:], I[:])
                costmT = sb.tile([128, 128], dt, tag="costmT", name="costmT")
                nc.vector.tensor_copy(costmT[:], ptK[:])
                KT = sb.tile([128, 128], dtb, tag="KT", name="KT")
                nc.scalar.activation(KT[:], costmT[:], fp.Exp, scale=-2.0)
                KC = sb.tile([128, 128], dtb, tag="KC", name="KC")
                nc.gpsimd.tensor_mul(KC[:], K[:], costm[:])
                if stage == 5:
                    dbg = sb.tile([128, 1], dt, tag="dbg5", name="dbg5")
                    nc.vector.tensor_copy(dbg[:], K[:, 0:1])

            if stage >= 6:
                # Sinkhorn loop
                b = sb.tile([128, MPC], dtb, tag="b0", name="b0")
                nc.vector.memset(b[:], 1.0)
                a = None
                for it in range(SINK_ITR):
                    py = psb.tile([128, MPC], dt, tag="big", name=f"py{it}")
                    nc.tensor.matmul(py[:], KT[:], b[:], start=True, stop=True)
                    r = scr.tile([128, MPC], dt, tag="r", name=f"r{it}")
                    nc.vector.reciprocal_approx_fast(out=r[:], in_=py[:])
                    a = scr.tile([128, MPC], dtb, tag="a", name=f"a{it}")
                    nc.vector.tensor_mul(a[:], p1T[:], r[:])
                    if it == SINK_ITR - 1:
                        pw = psb.tile([128, MPC], dt, tag="pw", name="pw")
                        nc.tensor.matmul(pw[:], KC[:], a[:],
                                         start=True, stop=True)
                    pz = psb.tile([128, MPC], dt, tag="big", name=f"pz{it}")
                    nc.tensor.matmul(pz[:], K2[:], a[:], start=True, stop=True)
                    b = scr.tile([128, MPC], dtb, tag="b", name=f"b{it}")
                    _c = _RAFC
                    nc.vector._custom_dve(_RAF, out=b[:], in0=pz[:],
                                          s0=_c["s0"], s1=_c["s1"],
                                          imm2=_c["imm2"])
                if stage == 6:
                    dbg = sb.tile([128, 1], dt, tag="dbg6", name="dbg6")
                    nc.vector.tensor_copy(dbg[:], b[:, 0:1])

            if stage >= 7:
                # wcp epilogue (pw computed inside the loop's last iter)
                scrW = scr.tile([128, MPC], dt, tag="r", name="scrW")
                wcp_part = sb.tile([128, 1], dt, tag="wcp_part",
                                   name="wcp_part")
                nc.vector.tensor_mul(scrW[:], pw[:], b[:])
                nc.vector.tensor_reduce(wcp_part[:], scrW[:],
                                        axis=ax.X, op=alu.add)

            # ---------------- pack + store ----------------
            outS = sb.tile([128, 2], dt, tag="outS", name="outS")
            nc.vector.memset(outS[:], 0.0)
            if wcp_part is not None:
                nc.vector.tensor_copy(outS[:, 0:1], wcp_part[:])
            elif dbg is not None:
                p = min(dbg.shape[0], 128)
                nc.vector.tensor_copy(outS[0:p, 0:1], dbg[0:p, 0:1])
            if ce_part is not None:
                nc.vector.tensor_copy(outS[0:RPC, 1:2], ce_part[:])
            nc.sync.dma_start(out=outd[:], in_=outS[:])

    nc.compile()
    return nc


def _get_nc(stage=99):
    key = ("nc", stage)
    if key not in _CACHE:
        _CACHE[key] = _build_nc(stage)
    return _CACHE[key]


def _make_in_maps(features):
    onesr = np.ones((1, 128), dtype=np.float32)
    in_maps = []
    for c in range(NCORES):
        maskce = np.zeros((RPC, B), dtype=np.float32)
        off = (c % 2) * 64
        maskce[np.arange(RPC), off + np.arange(RPC)] = 1.0
        in_maps.append({
            "features": features,
            "fslice": np.ascontiguousarray(features[c * RPC:(c + 1) * RPC, :]),
            "maskce": maskce,
            "onesr": onesr,
        })
    return in_maps


def kernel(features, batch=None, **kwargs):
    from concourse.bass_utils import run_bass_kernel_spmd

    features = np.ascontiguousarray(np.asarray(features, dtype=np.float32))
    assert features.shape == (N, D)

    nc = _get_nc()
    res = run_bass_kernel_spmd(nc, _make_in_maps(features),
                               list(range(NCORES)))

    ce_sum = 0.0
    wcp_sum = 0.0
    for c in range(NCORES):
        o = res.results[c]["out"]
        wcp_sum += float(o[:, 0].sum(dtype=np.float64))
        ce_sum += float(o[:RPC, 1].sum(dtype=np.float64))
    loss = ce_sum / M_TOT + wcp_sum / M_TOT
    return np.float32(loss)


if __name__ == "__main__":
    x = np.random.randn(N, D).astype(np.float32)
    print(kernel(x, B))


# revision 24
# speedup vs baseline: 1.4321x; 1.0027x over previous
"""Trainium2 Bass kernel for the CPN/WCP loss (ce + Sinkhorn wcp).

Strategy:
  - M = 2048 Sinkhorn problems sharded 256/core over 8 cores.
  - Per core: compute its 64-row slab of the NxN (-eudis)/2 matrix via PE
    matmuls (rank-1 matmul folds in the -0.5*sq_j term; the per-row sq_i
    shift is dropped -- softmax/log-softmax are shift invariant).
  - CE pieces (row LSE at temp 5, target logit) computed in row layout.
  - Softmax p1 computed in row layout, transposed to [128 class, 256 prob]
    via PE transposes.
  - Sinkhorn runs in multiplicative form: a = p1 / (K@b), b = p2 / (K^T@a)
    with K = exp(-2*cost) fixed => two matmuls + DVE approx-reciprocals per
    iteration, no transcendentals in the loop.
  - wcp_m = ((K.C)^T a) . b ; per-partition partials DMA'd out, host sums.
"""

import sys

for _p in ("/opt/trn_rl_repo",):
    if _p not in sys.path:
        sys.path.insert(0, _p)

import numpy as np

AUG = 4
B = 128
D = 512
N = AUG * B          # 512 feature rows
NCORES = 8
RPC = N // NCORES    # 64 eudis rows per core
MPC = RPC * AUG      # 256 sinkhorn problems per core
M_TOT = N * AUG      # 2048
TEMP = 5.0
GAMMA = 0.2
SINK_ITR = 5
SCALE1 = 2.0 / float(np.sqrt(np.float32(D)))  # softmax scale on h
SCALE5 = 2.0 / TEMP                            # CE scale on h
LN128 = float(np.log(128.0))

_CACHE = {}


def _build_nc(stage=99):
    import concourse.bacc as bacc
    import concourse.tile as tile
    import concourse.mybir as mybir
    from concourse.dve_ops import (RECIP_APPROX_FAST_CONSTS as _RAFC,
                                   RECIPROCAL_APPROX_FAST as _RAF)

    dt = mybir.dt.float32
    dtr = mybir.dt.float32r
    dtb = mybir.dt.bfloat16
    fp = mybir.ActivationFunctionType
    alu = mybir.AluOpType
    ax = mybir.AxisListType

    nc = bacc.Bacc(
        "TRN2",
        target_bir_lowering=False,
        debug=False,
        enable_asserts=False,
        num_devices=NCORES,
    )

    feat = nc.dram_tensor("features", [N, D], dt, kind="ExternalInput").ap()
    fsl = nc.dram_tensor("fslice", [RPC, D], dt, kind="ExternalInput").ap()
    mce = nc.dram_tensor("maskce", [RPC, B], dt, kind="ExternalInput").ap()
    onr = nc.dram_tensor("onesr", [1, 128], dtr, kind="ExternalInput").ap()
    outd = nc.dram_tensor("out", [128, 2], dt, kind="ExternalOutput").ap()

    with tile.TileContext(nc) as tc:
        with (
            tc.tile_pool(name="sb", bufs=1) as sb,
            tc.tile_pool(name="scr", bufs=2) as scr,
            tc.tile_pool(name="ps_big", bufs=3, space="PSUM") as psb,
            tc.tile_pool(name="ps_t", bufs=2, space="PSUM") as pst,
            tc.tile_pool(name="ps_h", bufs=1, space="PSUM") as psh,
        ):
            dbg = None  # [*,1] tile flushed to out col0 for stage bisection

            # Preload the combined exp+ln ACT table set so the compiler's
            # per-func set picker doesn't ping-pong exp_and_others <->
            # natural_log (each reload costs ~2.7us).
            _tabs = list(__import__("concourse.hw_specs",
                                    fromlist=["hw_specs"]
                                    ).get_activation_tables(nc.m.arch))
            _set_id = _tabs.index("natural_log_exp_and_others")
            nc.scalar.add_instruction(mybir.InstLoadActFuncSet(
                name=nc.get_next_instruction_name(), ins=[], outs=[],
                act_func_set_id=_set_id))

            # ---------------- loads ----------------
            # identity generated on-chip (a [128,128] DMA costs ~4us of
            # descriptor processing); F tiles split into halves across the
            # 3 DMA-issuing engines so the first tiles land early.
            ones_t = sb.tile([128, 128], dt, tag="ones_t", name="ones_t")
            nc.vector.memset(ones_t[:], 1.0)
            I = sb.tile([128, 128], dt, tag="I", name="I")
            nc.gpsimd.affine_select(I[:], ones_t[:], [[1, 128]],
                                    alu.is_equal, 0.0, base=0,
                                    channel_multiplier=-1)
            F = []
            for t in range(4):
                Ft = sb.tile([128, D], dt, tag=f"F{t}", name=f"F{t}")
                F.append(Ft)
            halves = [(0, 0, nc.sync), (0, 1, nc.gpsimd), (1, 0, nc.scalar),
                      (1, 1, nc.sync), (2, 0, nc.gpsimd), (2, 1, nc.scalar),
                      (3, 0, nc.sync), (3, 1, nc.gpsimd)]
            for t, h, eng in halves:
                eng.dma_start(
                    out=F[t][h * 64:(h + 1) * 64, :],
                    in_=feat[t * 128 + h * 64:t * 128 + (h + 1) * 64, :])
            fs = sb.tile([RPC, D], dt, tag="fs", name="fs")
            nc.scalar.dma_start(out=fs[:], in_=fsl[:])
            mk = sb.tile([RPC, B], dt, tag="mk", name="mk")
            nc.gpsimd.dma_start(out=mk[:], in_=mce[:])
            ones1 = sb.tile([1, 128], dtr, tag="ones1", name="ones1")
            nc.sync.dma_start(out=ones1[:], in_=onr[:])

            ce_part = None
            wcp_part = None

            if stage >= 1:
                # ---------------- F^T tiles ----------------
                FT = []
                for q in range(4):
                    FTq = sb.tile([128, D], dtr, tag=f"FT{q}", name=f"FT{q}")
                    FT.append(FTq)
                for t in range(4):
                    for q in range(4):
                        pt = pst.tile([128, 128], dt, tag="pt", name="pt")
                        nc.tensor.transpose(
                            pt[:], F[t][:, q * 128:(q + 1) * 128], I[:])
                        nc.vector.tensor_copy(
                            FT[q][:, t * 128:(t + 1) * 128], pt[:])

                fsT = []
                for q in range(4):
                    pt = pst.tile([128, RPC], dt, tag="pt", name="pt")
                    nc.tensor.transpose(
                        pt[:], fs[:, q * 128:(q + 1) * 128], I[:RPC, :RPC])
                    fsTq = sb.tile([128, RPC], dtr, tag=f"fsT{q}",
                                   name=f"fsT{q}")
                    nc.vector.tensor_copy(fsTq[:], pt[:])
                    fsT.append(fsTq)

                # sq_j row: -0.5 * sum_d F[j,:]^2
                sqc = sb.tile([128, 4], dt, tag="sqc", name="sqc")
                for t in range(4):
                    scrF = scr.tile([128, D], dt, tag="scrF", name="scrF")
                    nc.scalar.activation(scrF[:], F[t][:], fp.Square,
                                         accum_out=sqc[:, t:t + 1])
                pt4 = pst.tile([4, 128], dt, tag="pt", name="pt4")
                nc.tensor.transpose(pt4[:], sqc[:], I[:])
                s4 = sb.tile([4, 128], dtr, tag="s4", name="s4")
                nc.vector.tensor_scalar_mul(s4[:], pt4[:], -0.5)
                sqrow = sb.tile([1, D], dtr, tag="sqrow", name="sqrow")
                nc.sync.dma_start(
                    out=sqrow[0:1, :].rearrange("o (t x) -> o t x", t=4),
                    in_=s4[:])

                # mean-feature branch (gpsimd: off the DVE critical path)
                g = sb.tile([128, D], dt, tag="g", name="g")
                g2 = sb.tile([128, D], dt, tag="g2", name="g2")
                nc.gpsimd.tensor_add(g2[:], F[0][:], F[1][:])
                nc.gpsimd.tensor_add(g[:], F[2][:], F[3][:])
                nc.gpsimd.tensor_add(g[:], g[:], g2[:])
                gsq = scr.tile([128, D], dt, tag="scrF", name="gsq")
                ssg = sb.tile([128, 1], dt, tag="ssg", name="ssg")
                nc.scalar.activation(gsq[:], g[:], fp.Square,
                                     accum_out=ssg[:])
                lssg = sb.tile([128, 1], dt, tag="lssg", name="lssg")
                nc.scalar.activation(lssg[:], ssg[:], fp.Ln)
                rn = sb.tile([128, 1], dt, tag="rn", name="rn")
                nc.scalar.activation(rn[:], lssg[:], fp.Exp, scale=-0.5)
                fn = sb.tile([128, D], dt, tag="fn", name="fn")
                nc.vector.tensor_scalar_mul(fn[:], g[:], rn[:, 0:1])
                dbg = sqc

            if stage >= 2:
                # dist slab: h2 = dot - 0.5*sq_j  [64, 512]
                ph = psh.tile([RPC, D], dt, tag="ph", name="ph")
                for q in range(4):
                    nc.tensor.matmul(ph[:], fsT[q][:], FT[q][:],
                                     start=(q == 0), stop=False)
                nc.tensor.matmul(ph[:], ones1[0:1, 0:RPC], sqrow[0:1, :],
                                 start=False, stop=True)

                # fnT / G / cost normalization (overlaps the softmax phase;
                # the K exponentials stay later so they don't delay E1/E2
                # on the ACT engine).
                fnT = []
                for q in range(4):
                    pt = pst.tile([128, 128], dt, tag="pt", name="ptf")
                    nc.tensor.transpose(pt[:], fn[:, q * 128:(q + 1) * 128],
                                        I[:])
                    fnTq = sb.tile([128, 128], dtb, tag=f"fnT{q}",
                                   name=f"fnT{q}")
                    nc.vector.tensor_copy(fnTq[:], pt[:])
                    fnT.append(fnTq)
                pG = psb.tile([128, 128], dt, tag="big", name="pG")
                for q in range(4):
                    nc.tensor.matmul(pG[:], fnT[q][:], fnT[q][:],
                                     start=(q == 0), stop=(q == 3))
                gmax = sb.tile([128, 1], dt, tag="gmax", name="gmax")
                gmin = sb.tile([128, 1], dt, tag="gmin", name="gmin")
                nc.vector.tensor_reduce(gmax[:], pG[:], axis=ax.X, op=alu.max)
                nc.vector.tensor_reduce(gmin[:], pG[:], axis=ax.X, op=alu.min)
                den = sb.tile([128, 1], dt, tag="den", name="den")
                nc.vector.tensor_sub(den[:], gmax[:], gmin[:])
                rden = sb.tile([128, 1], dt, tag="rden", name="rden")
                nc.vector.reciprocal(rden[:], den[:])
                sA = sb.tile([128, 1], dt, tag="sA", name="sA")
                nc.vector.tensor_scalar_mul(sA[:], rden[:], -GAMMA)
                sB = sb.tile([128, 1], dt, tag="sB", name="sB")
                nc.vector.tensor_scalar(
                    out=sB[:], in0=gmax[:], scalar1=rden[:, 0:1],
                    scalar2=GAMMA, op0=alu.mult, op1=alu.mult)
                costm = sb.tile([128, 128], dt, tag="costm", name="costm")
                nc.vector.tensor_scalar(
                    out=costm[:], in0=pG[:], scalar1=sA[:, 0:1],
                    scalar2=sB[:, 0:1], op0=alu.mult, op1=alu.add)
                nc.vector.tensor_add(costm[:], costm[:], I[:])

                if stage == 2:
                    dbg = sb.tile([RPC, 1], dt, tag="dbg2", name="dbg2")
                    nc.vector.tensor_copy(dbg[:], ph[:, 0:1])

            if stage >= 3:
                # row stats / CE
                mh = sb.tile([RPC, 4], dt, tag="mh", name="mh")
                for k in range(4):
                    nc.vector.tensor_reduce(
                        mh[:, k:k + 1], ph[:, k * 128:(k + 1) * 128],
                        axis=ax.X, op=alu.max)
                bias1 = sb.tile([RPC, 4], dt, tag="bias1", name="bias1")
                nc.vector.tensor_scalar_mul(bias1[:], mh[:], -SCALE1)
                bias5 = sb.tile([RPC, 4], dt, tag="bias5", name="bias5")
                nc.vector.tensor_scalar_mul(bias5[:], mh[:], -SCALE5)

                E1 = sb.tile([RPC, D], dt, tag="E1", name="E1")
                E2 = sb.tile([RPC, D], dt, tag="E2", name="E2")
                for k in range(4):
                    ksl = slice(k * 128, (k + 1) * 128)
                    nc.scalar.activation(E1[:, ksl], ph[:, ksl], fp.Exp,
                                         bias=bias1[:, k:k + 1], scale=SCALE1)
                    nc.scalar.activation(E2[:, ksl], ph[:, ksl], fp.Exp,
                                         bias=bias5[:, k:k + 1], scale=SCALE5)
                S1 = sb.tile([RPC, 4], dt, tag="S1", name="S1")
                nc.vector.tensor_reduce(
                    S1[:], E1[:].rearrange("p (k x) -> p k x", k=4),
                    axis=ax.X, op=alu.add)
                S5 = sb.tile([RPC, 4], dt, tag="S5", name="S5")
                nc.vector.tensor_reduce(
                    S5[:], E2[:].rearrange("p (k x) -> p k x", k=4),
                    axis=ax.X, op=alu.add)

                # ce_m = ln(S5) - (SCALE5/SCALE1)*ln(E1[target]):
                # E1[target] = exp(SCALE1*(h_t - mh)) so this equals
                # ln(S5) + SCALE5*(mh - h_t).
                E1m = scr.tile([RPC, D], dt, tag="scrE", name="E1m")
                for k in range(4):
                    ksl = slice(k * 128, (k + 1) * 128)
                    nc.gpsimd.tensor_mul(E1m[:, ksl], E1[:, ksl], mk[:])
                Ed = sb.tile([RPC, 4], dt, tag="Ed", name="Ed")
                nc.vector.tensor_reduce(
                    Ed[:], E1m[:].rearrange("p (k x) -> p k x", k=4),
                    axis=ax.X, op=alu.add)
                lnS5 = sb.tile([RPC, 4], dt, tag="lnS5", name="lnS5")
                nc.scalar.activation(lnS5[:], S5[:], fp.Ln)
                lnEd = sb.tile([RPC, 4], dt, tag="lnEd", name="lnEd")
                nc.scalar.activation(lnEd[:], Ed[:], fp.Ln)
                ce4 = sb.tile([RPC, 4], dt, tag="ce4", name="ce4")
                nc.vector.scalar_tensor_tensor(
                    out=ce4[:], in0=lnEd[:], scalar=-(SCALE5 / SCALE1),
                    in1=lnS5[:], op0=alu.mult, op1=alu.add)
                ce_part = sb.tile([RPC, 1], dt, tag="ce_part", name="ce_part")
                nc.vector.tensor_reduce(ce_part[:], ce4[:], axis=ax.X,
                                        op=alu.add)
                dbg = ce_part

            if stage >= 4:
                # p1 = softmax + 1e-12, transposed to [128, 256]
                rS1 = sb.tile([RPC, 4], dt, tag="rS1", name="rS1")
                nc.vector.reciprocal(rS1[:], S1[:])
                p1r = sb.tile([RPC, D], dt, tag="p1r", name="p1r")
                for k in range(4):
                    ksl = slice(k * 128, (k + 1) * 128)
                    nc.vector.tensor_scalar(
                        out=p1r[:, ksl], in0=E1[:, ksl],
                        scalar1=rS1[:, k:k + 1], scalar2=1e-12,
                        op0=alu.mult, op1=alu.add)
                p1T = sb.tile([128, MPC], dtb, tag="p1T", name="p1T")
                for k in range(4):
                    pt = pst.tile([128, RPC], dt, tag="pt", name="ptp")
                    nc.tensor.transpose(pt[:], p1r[:, k * 128:(k + 1) * 128],
                                        I[:RPC, :RPC])
                    if k % 2 == 0:
                        nc.vector.tensor_copy(
                            p1T[:, k * RPC:(k + 1) * RPC], pt[:])
                    else:
                        nc.scalar.copy(p1T[:, k * RPC:(k + 1) * RPC], pt[:])
                if stage == 4:
                    dbg = sb.tile([128, 1], dt, tag="dbg4", name="dbg4")
                    nc.vector.tensor_copy(dbg[:], p1T[:, 0:1])

            if stage >= 5:
                # K / K2 / KC (exps kept after the softmax phase)
                K = sb.tile([128, 128], dt, tag="K", name="K")
                nc.scalar.activation(K[:], costm[:], fp.Exp, scale=-2.0)
                ln128t = sb.tile([128, 1], dt, tag="ln128t", name="ln128t")
                nc.vector.memset(ln128t[:], LN128)
                K2 = sb.tile([128, 128], dtb, tag="K2", name="K2")
                nc.scalar.activation(K2[:], costm[:], fp.Exp,
                                     bias=ln128t[:, 0:1], scale=-2.0)
                ptK = pst.tile([128, 128], dt, tag="pt", name="ptK")
                nc.tensor.transpose(ptK[:], costm[:], I[:])
                costmT = sb.tile([128, 128], dt, tag="costmT", name="costmT")
                nc.vector.tensor_copy(costmT[:], ptK[:])
                KT = sb.tile([128, 128], dtb, tag="KT", name="KT")
                nc.scalar.activation(KT[:], costmT[:], fp.Exp, scale=-2.0)
                KC = sb.tile([128, 128], dtb, tag="KC", name="KC")
                nc.gpsimd.tensor_mul(KC[:], K[:], costm[:])
                if stage == 5:
                    dbg = sb.tile([128, 1], dt, tag="dbg5", name="dbg5")
                    nc.vector.tensor_copy(dbg[:], K[:, 0:1])

            if stage >= 6:
                # Sinkhorn loop: two independent 128-problem chains so
                # PE / DVE / GpSimd pipeline across chains.
                HB = MPC // 2
                _c = _RAFC
                bs = []
                for h in range(2):
                    bh = sb.tile([128, HB], dtb, tag=f"b0{h}", name=f"b0{h}")
                    nc.vector.memset(bh[:], 1.0)
                    bs.append(bh)
                As = [None, None]
                pws = [None, None]
                for it in range(SINK_ITR):
                    pys = []
                    for h in range(2):
                        py = psb.tile([128, HB], dt, tag="big",
                                      name=f"py{it}{h}")
                        nc.tensor.matmul(py[:], KT[:], bs[h][:],
                                         start=True, stop=True)
                        pys.append(py)
                    rs = []
                    for h in range(2):
                        r = scr.tile([128, HB], dt, tag=f"r{h}",
                                     name=f"r{it}{h}")
                        nc.vector.reciprocal_approx_fast(out=r[:],
                                                         in_=pys[h][:])
                        rs.append(r)
                    for h in range(2):
                        a = scr.tile([128, HB], dtb, tag=f"a{h}",
                                     name=f"a{it}{h}")
                        eng = nc.vector if h == 0 else nc.gpsimd
                        eng.tensor_mul(a[:], p1T[:, h * HB:(h + 1) * HB],
                                       rs[h][:])
                        As[h] = a
                    if it == SINK_ITR - 1:
                        for h in range(2):
                            pw = psb.tile([128, HB], dt, tag=f"pw{h}",
                                          name=f"pw{h}", bufs=1)
                            nc.tensor.matmul(pw[:], KC[:], As[h][:],
                                             start=True, stop=True)
                            pws[h] = pw
                    pzs = []
                    for h in range(2):
                        pz = psb.tile([128, HB], dt, tag="big",
                                      name=f"pz{it}{h}")
                        nc.tensor.matmul(pz[:], K2[:], As[h][:],
                                         start=True, stop=True)
                        pzs.append(pz)
                    bs = []
                    for h in range(2):
                        bh = scr.tile([128, HB], dtb, tag=f"b{h}",
                                      name=f"b{it}{h}")
                        nc.vector._custom_dve(_RAF, out=bh[:], in0=pzs[h][:],
                                              s0=_c["s0"], s1=_c["s1"],
                                              imm2=_c["imm2"])
                        bs.append(bh)
                if stage == 6:
                    dbg = sb.tile([128, 1], dt, tag="dbg6", name="dbg6")
                    nc.vector.tensor_copy(dbg[:], bs[0][:, 0:1])

            if stage >= 7:
                # wcp epilogue (pw computed inside the loop's last iter)
                wcp_part = sb.tile([128, 1], dt, tag="wcp_part",
                                   name="wcp_part")
                wp = []
                for h in range(2):
                    scrW = scr.tile([128, HB], dt, tag=f"r{h}",
                                    name=f"scrW{h}")
                    nc.vector.tensor_mul(scrW[:], pws[h][:], bs[h][:])
                    wph = sb.tile([128, 1], dt, tag=f"wcp{h}", name=f"wcp{h}")
                    nc.vector.tensor_reduce(wph[:], scrW[:],
                                            axis=ax.X, op=alu.add)
                    wp.append(wph)
                nc.vector.tensor_add(wcp_part[:], wp[0][:], wp[1][:])

            # ---------------- pack + store ----------------
            outS = sb.tile([128, 2], dt, tag="outS", name="outS")
            nc.vector.memset(outS[:], 0.0)
            if wcp_part is not None:
                nc.vector.tensor_copy(outS[:, 0:1], wcp_part[:])
            elif dbg is not None:
                p = min(dbg.shape[0], 128)
                nc.vector.tensor_copy(outS[0:p, 0:1], dbg[0:p, 0:1])
            if ce_part is not None:
                nc.vector.tensor_copy(outS[0:RPC, 1:2], ce_part[:])
            nc.sync.dma_start(out=outd[:], in_=outS[:])

    nc.compile()
    return nc


def _get_nc(stage=99):
    key = ("nc", stage)
    if key not in _CACHE:
        _CACHE[key] = _build_nc(stage)
    return _CACHE[key]


def _make_in_maps(features):
    onesr = np.ones((1, 128), dtype=np.float32)
    in_maps = []
    for c in range(NCORES):
        maskce = np.zeros((RPC, B), dtype=np.float32)
        off = (c % 2) * 64
        maskce[np.arange(RPC), off + np.arange(RPC)] = 1.0
        in_maps.append({
            "features": features,
            "fslice": np.ascontiguousarray(features[c * RPC:(c + 1) * RPC, :]),
            "maskce": maskce,
            "onesr": onesr,
        })
    return in_maps


def kernel(features, batch=None, **kwargs):
    from concourse.bass_utils import run_bass_kernel_spmd

    features = np.ascontiguousarray(np.asarray(features, dtype=np.float32))
    assert features.shape == (N, D)

    nc = _get_nc()
    res = run_bass_kernel_spmd(nc, _make_in_maps(features),
                               list(range(NCORES)))

    ce_sum = 0.0
    wcp_sum = 0.0
    for c in range(NCORES):
        o = res.results[c]["out"]
        wcp_sum += float(o[:, 0].sum(dtype=np.float64))
        ce_sum += float(o[:RPC, 1].sum(dtype=np.float64))
    loss = ce_sum / M_TOT + wcp_sum / M_TOT
    return np.float32(loss)


if __name__ == "__main__":
    x = np.random.randn(N, D).astype(np.float32)
    print(kernel(x, B))


# revision 26
# speedup vs baseline: 1.5205x; 1.0618x over previous
"""Trainium2 Bass kernel for the CPN/WCP loss (ce + Sinkhorn wcp).

Strategy:
  - M = 2048 Sinkhorn problems sharded 256/core over 8 cores.
  - Per core: compute its 64-row slab of the NxN (-eudis)/2 matrix via PE
    matmuls (rank-1 matmul folds in the -0.5*sq_j term; the per-row sq_i
    shift is dropped -- softmax/log-softmax are shift invariant).
  - CE pieces (row LSE at temp 5, target logit) computed in row layout.
  - Softmax p1 computed in row layout, transposed to [128 class, 256 prob]
    via PE transposes.
  - Sinkhorn runs in multiplicative form: a = p1 / (K@b), b = p2 / (K^T@a)
    with K = exp(-2*cost) fixed => two matmuls + DVE approx-reciprocals per
    iteration, no transcendentals in the loop.
  - wcp_m = ((K.C)^T a) . b ; per-partition partials DMA'd out, host sums.
"""

import sys

for _p in ("/opt/trn_rl_repo",):
    if _p not in sys.path:
        sys.path.insert(0, _p)

import numpy as np

AUG = 4
B = 128
D = 512
N = AUG * B          # 512 feature rows
NCORES = 8
RPC = N // NCORES    # 64 eudis rows per core
MPC = RPC * AUG      # 256 sinkhorn problems per core
M_TOT = N * AUG      # 2048
TEMP = 5.0
GAMMA = 0.2
SINK_ITR = 5
SCALE1 = 2.0 / float(np.sqrt(np.float32(D)))  # softmax scale on h
SCALE5 = 2.0 / TEMP                            # CE scale on h
LN128 = float(np.log(128.0))

_CACHE = {}


def _build_nc(stage=99):
    import concourse.bacc as bacc
    import concourse.tile as tile
    import concourse.mybir as mybir
    from concourse.dve_ops import (RECIP_APPROX_FAST_CONSTS as _RAFC,
                                   RECIPROCAL_APPROX_FAST as _RAF)

    dt = mybir.dt.float32
    dtr = mybir.dt.float32r
    dtb = mybir.dt.bfloat16
    fp = mybir.ActivationFunctionType
    alu = mybir.AluOpType
    ax = mybir.AxisListType

    nc = bacc.Bacc(
        "TRN2",
        target_bir_lowering=False,
        debug=False,
        enable_asserts=False,
        num_devices=NCORES,
    )

    feat = nc.dram_tensor("features", [N, D], dt, kind="ExternalInput").ap()
    fsl = nc.dram_tensor("fslice", [RPC, D], dt, kind="ExternalInput").ap()
    mce = nc.dram_tensor("maskce", [RPC, B], dt, kind="ExternalInput").ap()
    outd = nc.dram_tensor("out", [128, 2], dt, kind="ExternalOutput").ap()

    with tile.TileContext(nc) as tc:
        with (
            tc.tile_pool(name="sb", bufs=1) as sb,
            tc.tile_pool(name="scr", bufs=2) as scr,
            tc.tile_pool(name="ps_big", bufs=3, space="PSUM") as psb,
            tc.tile_pool(name="ps_t", bufs=2, space="PSUM") as pst,
            tc.tile_pool(name="ps_h", bufs=1, space="PSUM") as psh,
        ):
            dbg = None  # [*,1] tile flushed to out col0 for stage bisection

            # Preload the combined exp+ln ACT table set so the compiler's
            # per-func set picker doesn't ping-pong exp_and_others <->
            # natural_log (each reload costs ~2.7us).
            _tabs = list(__import__("concourse.hw_specs",
                                    fromlist=["hw_specs"]
                                    ).get_activation_tables(nc.m.arch))
            _set_id = _tabs.index("natural_log_exp_and_others")
            nc.scalar.add_instruction(mybir.InstLoadActFuncSet(
                name=nc.get_next_instruction_name(), ins=[], outs=[],
                act_func_set_id=_set_id))

            # ---------------- loads ----------------
            # identity generated on-chip (a [128,128] DMA costs ~4us of
            # descriptor processing); F tiles split into halves across the
            # 3 DMA-issuing engines so the first tiles land early.
            ones_t = sb.tile([128, 128], dt, tag="ones_t", name="ones_t")
            nc.vector.memset(ones_t[:], 1.0)
            I = sb.tile([128, 128], dt, tag="I", name="I")
            nc.gpsimd.affine_select(I[:], ones_t[:], [[1, 128]],
                                    alu.is_equal, 0.0, base=0,
                                    channel_multiplier=-1)
            F = []
            for t in range(4):
                Ft = sb.tile([128, D], dt, tag=f"F{t}", name=f"F{t}")
                F.append(Ft)
            halves = [(0, 0, nc.sync), (0, 1, nc.gpsimd), (1, 0, nc.scalar),
                      (1, 1, nc.sync), (2, 0, nc.gpsimd), (2, 1, nc.scalar),
                      (3, 0, nc.sync), (3, 1, nc.gpsimd)]
            for t, h, eng in halves:
                eng.dma_start(
                    out=F[t][h * 64:(h + 1) * 64, :],
                    in_=feat[t * 128 + h * 64:t * 128 + (h + 1) * 64, :])
            fs = sb.tile([RPC, D], dt, tag="fs", name="fs")
            nc.scalar.dma_start(out=fs[:], in_=fsl[:])
            mk = sb.tile([RPC, B], dt, tag="mk", name="mk")
            nc.gpsimd.dma_start(out=mk[:], in_=mce[:])

            ce_part = None
            wcp_part = None

            if stage >= 1:
                # ---------------- F^T tiles ----------------
                FT = []
                for q in range(4):
                    FTq = sb.tile([128, D], dtr, tag=f"FT{q}", name=f"FT{q}")
                    FT.append(FTq)
                for t in range(4):
                    for q in range(4):
                        pt = pst.tile([128, 128], dt, tag="pt", name="pt")
                        nc.tensor.transpose(
                            pt[:], F[t][:, q * 128:(q + 1) * 128], I[:])
                        nc.vector.tensor_copy(
                            FT[q][:, t * 128:(t + 1) * 128], pt[:])

                fsT = []
                for q in range(4):
                    pt = pst.tile([128, RPC], dt, tag="pt", name="pt")
                    nc.tensor.transpose(
                        pt[:], fs[:, q * 128:(q + 1) * 128], I[:RPC, :RPC])
                    fsTq = sb.tile([128, RPC], dtr, tag=f"fsT{q}",
                                   name=f"fsT{q}")
                    nc.vector.tensor_copy(fsTq[:], pt[:])
                    fsT.append(fsTq)

                # sq_j row: -0.5 * sum_d F[j,:]^2
                sqc = sb.tile([128, 4], dt, tag="sqc", name="sqc")
                for t in range(4):
                    scrF = scr.tile([128, D], dt, tag="scrF", name="scrF")
                    nc.scalar.activation(scrF[:], F[t][:], fp.Square,
                                         accum_out=sqc[:, t:t + 1])
                sqc2 = sb.tile([128, 4], dt, tag="sqc2", name="sqc2")
                nc.vector.tensor_scalar_mul(sqc2[:], sqc[:], -0.5)

                # mean-feature branch (gpsimd: off the DVE critical path)
                g = sb.tile([128, D], dt, tag="g", name="g")
                g2 = sb.tile([128, D], dt, tag="g2", name="g2")
                nc.gpsimd.tensor_add(g2[:], F[0][:], F[1][:])
                nc.gpsimd.tensor_add(g[:], F[2][:], F[3][:])
                nc.gpsimd.tensor_add(g[:], g[:], g2[:])
                gsq = scr.tile([128, D], dt, tag="scrF", name="gsq")
                ssg = sb.tile([128, 1], dt, tag="ssg", name="ssg")
                nc.scalar.activation(gsq[:], g[:], fp.Square,
                                     accum_out=ssg[:])
                lssg = sb.tile([128, 1], dt, tag="lssg", name="lssg")
                nc.scalar.activation(lssg[:], ssg[:], fp.Ln)
                rn = sb.tile([128, 1], dt, tag="rn", name="rn")
                nc.scalar.activation(rn[:], lssg[:], fp.Exp, scale=-0.5)
                fn = sb.tile([128, D], dt, tag="fn", name="fn")
                nc.vector.tensor_scalar_mul(fn[:], g[:], rn[:, 0:1])
                dbg = sqc

            if stage >= 2:
                # dist slab: h2 = dot - 0.5*sq_j  [64, 512]
                ph = psh.tile([RPC, D], dt, tag="ph", name="ph")
                for q in range(4):
                    nc.tensor.matmul(ph[:], fsT[q][:], FT[q][:],
                                     start=(q == 0), stop=False)
                # -0.5*sq_j via broadcast-lhsT against identity:
                # out[i,j'] = sum_k sqc2[k,t]*I[k,j'] = sqc2[j',t]
                for t in range(4):
                    nc.tensor.matmul(
                        ph[:, t * 128:(t + 1) * 128],
                        sqc2[:, t:t + 1].to_broadcast((128, RPC)),
                        I[:], start=False, stop=(t == 3))


                if stage == 2:
                    dbg = sb.tile([RPC, 1], dt, tag="dbg2", name="dbg2")
                    nc.vector.tensor_copy(dbg[:], ph[:, 0:1])

            if stage >= 3:
                # row stats / CE
                mh = sb.tile([RPC, 4], dt, tag="mh", name="mh")
                for k in range(4):
                    nc.vector.tensor_reduce(
                        mh[:, k:k + 1], ph[:, k * 128:(k + 1) * 128],
                        axis=ax.X, op=alu.max)
                bias1 = sb.tile([RPC, 4], dt, tag="bias1", name="bias1")
                nc.vector.tensor_scalar_mul(bias1[:], mh[:], -SCALE1)
                bias5 = sb.tile([RPC, 4], dt, tag="bias5", name="bias5")
                nc.vector.tensor_scalar_mul(bias5[:], mh[:], -SCALE5)

                E1 = sb.tile([RPC, D], dt, tag="E1", name="E1")
                E2 = sb.tile([RPC, D], dt, tag="E2", name="E2")
                for k in range(4):
                    ksl = slice(k * 128, (k + 1) * 128)
                    nc.scalar.activation(E1[:, ksl], ph[:, ksl], fp.Exp,
                                         bias=bias1[:, k:k + 1], scale=SCALE1)
                    nc.scalar.activation(E2[:, ksl], ph[:, ksl], fp.Exp,
                                         bias=bias5[:, k:k + 1], scale=SCALE5)
                S1 = sb.tile([RPC, 4], dt, tag="S1", name="S1")
                nc.vector.tensor_reduce(
                    S1[:], E1[:].rearrange("p (k x) -> p k x", k=4),
                    axis=ax.X, op=alu.add)
                S5 = sb.tile([RPC, 4], dt, tag="S5", name="S5")
                nc.vector.tensor_reduce(
                    S5[:], E2[:].rearrange("p (k x) -> p k x", k=4),
                    axis=ax.X, op=alu.add)

                # ce_m = ln(S5) - (SCALE5/SCALE1)*ln(E1[target]):
                # E1[target] = exp(SCALE1*(h_t - mh)) so this equals
                # ln(S5) + SCALE5*(mh - h_t).
                E1m = scr.tile([RPC, D], dt, tag="scrE", name="E1m")
                for k in range(4):
                    ksl = slice(k * 128, (k + 1) * 128)
                    nc.gpsimd.tensor_mul(E1m[:, ksl], E1[:, ksl], mk[:])
                Ed = sb.tile([RPC, 4], dt, tag="Ed", name="Ed")
                nc.vector.tensor_reduce(
                    Ed[:], E1m[:].rearrange("p (k x) -> p k x", k=4),
                    axis=ax.X, op=alu.add)
                lnS5 = sb.tile([RPC, 4], dt, tag="lnS5", name="lnS5")
                nc.scalar.activation(lnS5[:], S5[:], fp.Ln)
                lnEd = sb.tile([RPC, 4], dt, tag="lnEd", name="lnEd")
                nc.scalar.activation(lnEd[:], Ed[:], fp.Ln)
                ce4 = sb.tile([RPC, 4], dt, tag="ce4", name="ce4")
                nc.vector.scalar_tensor_tensor(
                    out=ce4[:], in0=lnEd[:], scalar=-(SCALE5 / SCALE1),
                    in1=lnS5[:], op0=alu.mult, op1=alu.add)
                ce_part = sb.tile([RPC, 1], dt, tag="ce_part", name="ce_part")
                nc.vector.tensor_reduce(ce_part[:], ce4[:], axis=ax.X,
                                        op=alu.add)
                dbg = ce_part

            if stage >= 4:
                # p1 = softmax + 1e-12, transposed to [128, 256]
                rS1 = sb.tile([RPC, 4], dt, tag="rS1", name="rS1")
                nc.vector.reciprocal(rS1[:], S1[:])
                p1r = sb.tile([RPC, D], dt, tag="p1r", name="p1r")
                for k in range(4):
                    ksl = slice(k * 128, (k + 1) * 128)
                    nc.vector.tensor_scalar(
                        out=p1r[:, ksl], in0=E1[:, ksl],
                        scalar1=rS1[:, k:k + 1], scalar2=1e-12,
                        op0=alu.mult, op1=alu.add)
                if stage == 4:
                    dbg = sb.tile([128, 1], dt, tag="dbg4", name="dbg4")
                    nc.vector.tensor_copy(dbg[:], p1T[:, 0:1])

            if stage >= 5:
                # fnT / G / cost normalization (overlaps the softmax phase;
                # the K exponentials stay later so they don't delay E1/E2
                # on the ACT engine).
                fnT = []
                for q in range(4):
                    pt = pst.tile([128, 128], dt, tag="pt", name="ptf")
                    nc.tensor.transpose(pt[:], fn[:, q * 128:(q + 1) * 128],
                                        I[:])
                    fnTq = sb.tile([128, 128], dtb, tag=f"fnT{q}",
                                   name=f"fnT{q}")
                    nc.vector.tensor_copy(fnTq[:], pt[:])
                    fnT.append(fnTq)
                pG = psb.tile([128, 128], dt, tag="big", name="pG")
                for q in range(4):
                    nc.tensor.matmul(pG[:], fnT[q][:], fnT[q][:],
                                     start=(q == 0), stop=(q == 3))
                gmax = sb.tile([128, 1], dt, tag="gmax", name="gmax")
                gmin = sb.tile([128, 1], dt, tag="gmin", name="gmin")
                nc.vector.tensor_reduce(gmax[:], pG[:], axis=ax.X, op=alu.max)
                nc.vector.tensor_reduce(gmin[:], pG[:], axis=ax.X, op=alu.min)
                den = sb.tile([128, 1], dt, tag="den", name="den")
                nc.vector.tensor_sub(den[:], gmax[:], gmin[:])
                rden = sb.tile([128, 1], dt, tag="rden", name="rden")
                nc.vector.reciprocal(rden[:], den[:])
                sA = sb.tile([128, 1], dt, tag="sA", name="sA")
                nc.vector.tensor_scalar_mul(sA[:], rden[:], -GAMMA)
                sB = sb.tile([128, 1], dt, tag="sB", name="sB")
                nc.vector.tensor_scalar(
                    out=sB[:], in0=gmax[:], scalar1=rden[:, 0:1],
                    scalar2=GAMMA, op0=alu.mult, op1=alu.mult)
                costm = sb.tile([128, 128], dt, tag="costm", name="costm")
                nc.vector.tensor_scalar(
                    out=costm[:], in0=pG[:], scalar1=sA[:, 0:1],
                    scalar2=sB[:, 0:1], op0=alu.mult, op1=alu.add)
                nc.vector.tensor_add(costm[:], costm[:], I[:])

                # K / K2 / KC (exps kept after the softmax phase)
                K = sb.tile([128, 128], dt, tag="K", name="K")
                nc.scalar.activation(K[:], costm[:], fp.Exp, scale=-2.0)
                ln128t = sb.tile([128, 1], dt, tag="ln128t", name="ln128t")
                nc.vector.memset(ln128t[:], LN128)
                K2 = sb.tile([128, 128], dtb, tag="K2", name="K2")
                nc.scalar.activation(K2[:], costm[:], fp.Exp,
                                     bias=ln128t[:, 0:1], scale=-2.0)
                ptK = pst.tile([128, 128], dt, tag="pt", name="ptK")
                nc.tensor.transpose(ptK[:], costm[:], I[:])
                costmT = sb.tile([128, 128], dt, tag="costmT", name="costmT")
                nc.vector.tensor_copy(costmT[:], ptK[:])
                KT = sb.tile([128, 128], dtb, tag="KT", name="KT")
                nc.scalar.activation(KT[:], costmT[:], fp.Exp, scale=-2.0)
                KC = sb.tile([128, 128], dtb, tag="KC", name="KC")
                nc.gpsimd.tensor_mul(KC[:], K[:], costm[:])

                p1T = sb.tile([128, MPC], dtb, tag="p1T", name="p1T")
                for k in range(4):
                    pt = pst.tile([128, RPC], dt, tag="pt", name="ptp")
                    nc.tensor.transpose(pt[:], p1r[:, k * 128:(k + 1) * 128],
                                        I[:RPC, :RPC])
                    if k % 2 == 0:
                        nc.vector.tensor_copy(
                            p1T[:, k * RPC:(k + 1) * RPC], pt[:])
                    else:
                        nc.scalar.copy(p1T[:, k * RPC:(k + 1) * RPC], pt[:])
                if stage == 5:
                    dbg = sb.tile([128, 1], dt, tag="dbg5", name="dbg5")
                    nc.vector.tensor_copy(dbg[:], K[:, 0:1])

            if stage >= 6:
                # Sinkhorn loop: two independent 128-problem chains so
                # PE / DVE / GpSimd pipeline across chains.
                HB = MPC // 2
                _c = _RAFC
                bs = []
                for h in range(2):
                    bh = sb.tile([128, HB], dtb, tag=f"b0{h}", name=f"b0{h}")
                    nc.vector.memset(bh[:], 1.0)
                    bs.append(bh)
                As = [None, None]
                pws = [None, None]
                for it in range(SINK_ITR):
                    pys = []
                    for h in range(2):
                        py = psb.tile([128, HB], dt, tag="big",
                                      name=f"py{it}{h}")
                        nc.tensor.matmul(py[:], KT[:], bs[h][:],
                                         start=True, stop=True)
                        pys.append(py)
                    rs = []
                    for h in range(2):
                        r = scr.tile([128, HB], dt, tag=f"r{h}",
                                     name=f"r{it}{h}")
                        nc.vector.reciprocal_approx_fast(out=r[:],
                                                         in_=pys[h][:])
                        rs.append(r)
                    for h in range(2):
                        a = scr.tile([128, HB], dtb, tag=f"a{h}",
                                     name=f"a{it}{h}")
                        eng = nc.vector if h == 0 else nc.gpsimd
                        eng.tensor_mul(a[:], p1T[:, h * HB:(h + 1) * HB],
                                       rs[h][:])
                        As[h] = a
                    if it == SINK_ITR - 1:
                        for h in range(2):
                            pw = psb.tile([128, HB], dt, tag=f"pw{h}",
                                          name=f"pw{h}", bufs=1)
                            nc.tensor.matmul(pw[:], KC[:], As[h][:],
                                             start=True, stop=True)
                            pws[h] = pw
                    pzs = []
                    for h in range(2):
                        pz = psb.tile([128, HB], dt, tag="big",
                                      name=f"pz{it}{h}")
                        nc.tensor.matmul(pz[:], K2[:], As[h][:],
                                         start=True, stop=True)
                        pzs.append(pz)
                    bs = []
                    for h in range(2):
                        bh = scr.tile([128, HB], dtb, tag=f"b{h}",
                                      name=f"b{it}{h}")
                        nc.vector._custom_dve(_RAF, out=bh[:], in0=pzs[h][:],
                                              s0=_c["s0"], s1=_c["s1"],
                                              imm2=_c["imm2"])
                        bs.append(bh)
                if stage == 6:
                    dbg = sb.tile([128, 1], dt, tag="dbg6", name="dbg6")
                    nc.vector.tensor_copy(dbg[:], bs[0][:, 0:1])

            if stage >= 7:
                # wcp epilogue (pw computed inside the loop's last iter)
                wcp_part = sb.tile([128, 1], dt, tag="wcp_part",
                                   name="wcp_part")
                wp = []
                for h in range(2):
                    scrW = scr.tile([128, HB], dt, tag=f"r{h}",
                                    name=f"scrW{h}")
                    nc.vector.tensor_mul(scrW[:], pws[h][:], bs[h][:])
                    wph = sb.tile([128, 1], dt, tag=f"wcp{h}", name=f"wcp{h}")
                    nc.vector.tensor_reduce(wph[:], scrW[:],
                                            axis=ax.X, op=alu.add)
                    wp.append(wph)
                nc.vector.tensor_add(wcp_part[:], wp[0][:], wp[1][:])

            # ---------------- pack + store ----------------
            outS = sb.tile([128, 2], dt, tag="outS", name="outS")
            nc.vector.memset(outS[:], 0.0)
            if wcp_part is not None:
                nc.vector.tensor_copy(outS[:, 0:1], wcp_part[:])
            elif dbg is not None:
                p = min(dbg.shape[0], 128)
                nc.vector.tensor_copy(outS[0:p, 0:1], dbg[0:p, 0:1])
            if ce_part is not None:
                nc.vector.tensor_copy(outS[0:RPC, 1:2], ce_part[:])
            nc.sync.dma_start(out=outd[:], in_=outS[:])

    nc.compile()
    return nc


def _get_nc(stage=99):
    key = ("nc", stage)
    if key not in _CACHE:
        _CACHE[key] = _build_nc(stage)
    return _CACHE[key]


def _make_in_maps(features):
    in_maps = []
    for c in range(NCORES):
        maskce = np.zeros((RPC, B), dtype=np.float32)
        off = (c % 2) * 64
        maskce[np.arange(RPC), off + np.arange(RPC)] = 1.0
        in_maps.append({
            "features": features,
            "fslice": np.ascontiguousarray(features[c * RPC:(c + 1) * RPC, :]),
            "maskce": maskce,
        })
    return in_maps


def kernel(features, batch=None, **kwargs):
    from concourse.bass_utils import run_bass_kernel_spmd

    features = np.ascontiguousarray(np.asarray(features, dtype=np.float32))
    assert features.shape == (N, D)

    nc = _get_nc()
    res = run_bass_kernel_spmd(nc, _make_in_maps(features),
                               list(range(NCORES)))

    ce_sum = 0.0
    wcp_sum = 0.0
    for c in range(NCORES):
        o = res.results[c]["out"]
        wcp_sum += float(o[:, 0].sum(dtype=np.float64))
        ce_sum += float(o[:RPC, 1].sum(dtype=np.float64))
    loss = ce_sum / M_TOT + wcp_sum / M_TOT
    return np.float32(loss)


if __name__ == "__main__":
    x = np.random.randn(N, D).astype(np.float32)
    print(kernel(x, B))


# revision 28
# speedup vs baseline: 1.6552x; 1.0886x over previous
"""Trainium2 Bass kernel for the CPN/WCP loss (ce + Sinkhorn wcp).

Strategy:
  - M = 2048 Sinkhorn problems sharded 256/core over 8 cores.
  - Per core: compute its 64-row slab of the NxN (-eudis)/2 matrix via PE
    matmuls (rank-1 matmul folds in the -0.5*sq_j term; the per-row sq_i
    shift is dropped -- softmax/log-softmax are shift invariant).
  - CE pieces (row LSE at temp 5, target logit) computed in row layout.
  - Softmax p1 computed in row layout, transposed to [128 class, 256 prob]
    via PE transposes.
  - Sinkhorn runs in multiplicative form: a = p1 / (K@b), b = p2 / (K^T@a)
    with K = exp(-2*cost) fixed => two matmuls + DVE approx-reciprocals per
    iteration, no transcendentals in the loop.
  - wcp_m = ((K.C)^T a) . b ; per-partition partials DMA'd out, host sums.
"""

import sys

for _p in ("/opt/trn_rl_repo",):
    if _p not in sys.path:
        sys.path.insert(0, _p)

import numpy as np

AUG = 4
B = 128
D = 512
N = AUG * B          # 512 feature rows
NCORES = 8
RPC = N // NCORES    # 64 eudis rows per core
MPC = RPC * AUG      # 256 sinkhorn problems per core
M_TOT = N * AUG      # 2048
TEMP = 5.0
GAMMA = 0.2
SINK_ITR = 5
SCALE1 = 2.0 / float(np.sqrt(np.float32(D)))  # softmax scale on h
SCALE5 = 2.0 / TEMP                            # CE scale on h
LN128 = float(np.log(128.0))

_CACHE = {}


def _build_nc(stage=99):
    import concourse.bacc as bacc
    import concourse.tile as tile
    import concourse.mybir as mybir
    from concourse.dve_ops import (RECIP_APPROX_FAST_CONSTS as _RAFC,
                                   RECIPROCAL_APPROX_FAST as _RAF)

    dt = mybir.dt.float32
    dtr = mybir.dt.float32r
    dtb = mybir.dt.bfloat16
    fp = mybir.ActivationFunctionType
    alu = mybir.AluOpType
    ax = mybir.AxisListType

    nc = bacc.Bacc(
        "TRN2",
        target_bir_lowering=False,
        debug=False,
        enable_asserts=False,
        num_devices=NCORES,
    )

    feat = nc.dram_tensor("features", [N, D], dtr, kind="ExternalInput").ap()
    fsl = nc.dram_tensor("fslice", [RPC, D], dtr, kind="ExternalInput").ap()
    mce = nc.dram_tensor("maskce", [RPC, B], dt, kind="ExternalInput").ap()
    outd = nc.dram_tensor("out", [128, 2], dt, kind="ExternalOutput").ap()

    with tile.TileContext(nc) as tc:
        with (
            tc.tile_pool(name="sb", bufs=1) as sb,
            tc.tile_pool(name="scr", bufs=2) as scr,
            tc.tile_pool(name="ps_big", bufs=3, space="PSUM") as psb,
            tc.tile_pool(name="ps_t", bufs=3, space="PSUM") as pst,
            tc.tile_pool(name="ps_h", bufs=1, space="PSUM") as psh,
        ):
            dbg = None  # [*,1] tile flushed to out col0 for stage bisection

            # Preload the combined exp+ln ACT table set so the compiler's
            # per-func set picker doesn't ping-pong exp_and_others <->
            # natural_log (each reload costs ~2.7us).
            _tabs = list(__import__("concourse.hw_specs",
                                    fromlist=["hw_specs"]
                                    ).get_activation_tables(nc.m.arch))
            _set_id = _tabs.index("natural_log_exp_and_others")
            nc.scalar.add_instruction(mybir.InstLoadActFuncSet(
                name=nc.get_next_instruction_name(), ins=[], outs=[],
                act_func_set_id=_set_id))

            # ---------------- loads ----------------
            # identity generated on-chip (a [128,128] DMA costs ~4us of
            # descriptor processing); F tiles split into halves across the
            # 3 DMA-issuing engines so the first tiles land early.
            ones_t = sb.tile([128, 128], dt, tag="ones_t", name="ones_t")
            nc.vector.memset(ones_t[:], 1.0)
            I = sb.tile([128, 128], dt, tag="I", name="I")
            nc.gpsimd.affine_select(I[:], ones_t[:], [[1, 128]],
                                    alu.is_equal, 0.0, base=0,
                                    channel_multiplier=-1)
            I_r = sb.tile([128, 128], dtr, tag="I_r", name="I_r")
            nc.vector.tensor_copy(I_r[:], I[:])
            F = []
            for t in range(4):
                Ft = sb.tile([128, D], dtr, tag=f"F{t}", name=f"F{t}")
                F.append(Ft)
            halves = [(0, 0, nc.sync), (0, 1, nc.gpsimd), (1, 0, nc.scalar),
                      (1, 1, nc.sync), (2, 0, nc.gpsimd), (2, 1, nc.scalar),
                      (3, 0, nc.sync), (3, 1, nc.gpsimd)]
            for t, h, eng in halves:
                eng.dma_start(
                    out=F[t][h * 64:(h + 1) * 64, :],
                    in_=feat[t * 128 + h * 64:t * 128 + (h + 1) * 64, :])
            fs = sb.tile([RPC, D], dtr, tag="fs", name="fs")
            nc.scalar.dma_start(out=fs[:], in_=fsl[:])
            mk = sb.tile([RPC, B], dt, tag="mk", name="mk")
            nc.gpsimd.dma_start(out=mk[:], in_=mce[:])

            ce_part = None
            wcp_part = None

            if stage >= 1:
                # ---------------- F^T tiles ----------------
                FT = []
                for q in range(4):
                    FTq = sb.tile([128, D], dtr, tag=f"FT{q}", name=f"FT{q}")
                    FT.append(FTq)
                for t in range(4):
                    for q in range(4):
                        pt = pst.tile([128, 128], dt, tag="pt", name="pt")
                        nc.tensor.transpose(
                            pt[:].bitcast(dtr),
                            F[t][:, q * 128:(q + 1) * 128], I_r[:])
                        nc.vector.tensor_copy(
                            FT[q][:, t * 128:(t + 1) * 128], pt[:])

                fsT = []
                for q in range(4):
                    pt = pst.tile([128, RPC], dt, tag="pt", name="pt")
                    nc.tensor.transpose(
                        pt[:].bitcast(dtr),
                        fs[:, q * 128:(q + 1) * 128], I_r[:RPC, :RPC])
                    fsTq = sb.tile([128, RPC], dtr, tag=f"fsT{q}",
                                   name=f"fsT{q}")
                    nc.vector.tensor_copy(fsTq[:], pt[:])
                    fsT.append(fsTq)

                # sq_j row: -0.5 * sum_d F[j,:]^2
                sqc = sb.tile([128, 4], dt, tag="sqc", name="sqc")
                for t in range(4):
                    scrF = scr.tile([128, D], dt, tag="scrF", name="scrF")
                    nc.scalar.activation(scrF[:], F[t][:], fp.Square,
                                         accum_out=sqc[:, t:t + 1])
                sqc2 = sb.tile([128, 4], dtr, tag="sqc2", name="sqc2")
                nc.vector.tensor_scalar_mul(sqc2[:], sqc[:], -0.5)

                # mean-feature branch (gpsimd: off the DVE critical path)
                g = sb.tile([128, D], dt, tag="g", name="g")
                g2 = sb.tile([128, D], dt, tag="g2", name="g2")
                nc.gpsimd.tensor_add(g2[:], F[0][:], F[1][:])
                nc.gpsimd.tensor_add(g[:], F[2][:], F[3][:])
                nc.gpsimd.tensor_add(g[:], g[:], g2[:])
                gsq = scr.tile([128, D], dt, tag="scrF", name="gsq")
                ssg = sb.tile([128, 1], dt, tag="ssg", name="ssg")
                nc.scalar.activation(gsq[:], g[:], fp.Square,
                                     accum_out=ssg[:])
                lssg = sb.tile([128, 1], dt, tag="lssg", name="lssg")
                nc.scalar.activation(lssg[:], ssg[:], fp.Ln)
                rn = sb.tile([128, 1], dt, tag="rn", name="rn")
                nc.scalar.activation(rn[:], lssg[:], fp.Exp, scale=-0.5)
                fn = sb.tile([128, D], dt, tag="fn", name="fn")
                nc.vector.tensor_scalar_mul(fn[:], g[:], rn[:, 0:1])
                dbg = sqc

            if stage >= 2:
                # dist slab: h2 = dot - 0.5*sq_j  [64, 512]
                ph = psh.tile([RPC, D], dt, tag="ph", name="ph")
                for q in range(4):
                    nc.tensor.matmul(ph[:], fsT[q][:], FT[q][:],
                                     start=(q == 0), stop=False)
                # -0.5*sq_j via broadcast-lhsT against identity:
                # out[i,j'] = sum_k sqc2[k,t]*I[k,j'] = sqc2[j',t]
                for t in range(4):
                    nc.tensor.matmul(
                        ph[:, t * 128:(t + 1) * 128],
                        sqc2[:, t:t + 1].to_broadcast((128, RPC)),
                        I_r[:], start=False, stop=(t == 3))


                if stage == 2:
                    dbg = sb.tile([RPC, 1], dt, tag="dbg2", name="dbg2")
                    nc.vector.tensor_copy(dbg[:], ph[:, 0:1])

            if stage >= 3:
                # row stats / CE
                mh = sb.tile([RPC, 4], dt, tag="mh", name="mh")
                for k in range(4):
                    nc.vector.tensor_reduce(
                        mh[:, k:k + 1], ph[:, k * 128:(k + 1) * 128],
                        axis=ax.X, op=alu.max)
                bias1 = sb.tile([RPC, 4], dt, tag="bias1", name="bias1")
                nc.vector.tensor_scalar_mul(bias1[:], mh[:], -SCALE1)
                bias5 = sb.tile([RPC, 4], dt, tag="bias5", name="bias5")
                nc.vector.tensor_scalar_mul(bias5[:], mh[:], -SCALE5)

                E1 = sb.tile([RPC, D], dt, tag="E1", name="E1")
                E2 = sb.tile([RPC, D], dt, tag="E2", name="E2")
                for k in range(4):
                    ksl = slice(k * 128, (k + 1) * 128)
                    nc.scalar.activation(E1[:, ksl], ph[:, ksl], fp.Exp,
                                         bias=bias1[:, k:k + 1], scale=SCALE1)
                    nc.scalar.activation(E2[:, ksl], ph[:, ksl], fp.Exp,
                                         bias=bias5[:, k:k + 1], scale=SCALE5)
                S1 = sb.tile([RPC, 4], dt, tag="S1", name="S1")
                nc.vector.tensor_reduce(
                    S1[:], E1[:].rearrange("p (k x) -> p k x", k=4),
                    axis=ax.X, op=alu.add)
                S5 = sb.tile([RPC, 4], dt, tag="S5", name="S5")
                nc.vector.tensor_reduce(
                    S5[:], E2[:].rearrange("p (k x) -> p k x", k=4),
                    axis=ax.X, op=alu.add)

                # ce_m = ln(S5) - (SCALE5/SCALE1)*ln(E1[target]):
                # E1[target] = exp(SCALE1*(h_t - mh)) so this equals
                # ln(S5) + SCALE5*(mh - h_t).
                E1m = scr.tile([RPC, D], dt, tag="scrE", name="E1m")
                for k in range(4):
                    ksl = slice(k * 128, (k + 1) * 128)
                    nc.gpsimd.tensor_mul(E1m[:, ksl], E1[:, ksl], mk[:])
                Ed = sb.tile([RPC, 4], dt, tag="Ed", name="Ed")
                nc.vector.tensor_reduce(
                    Ed[:], E1m[:].rearrange("p (k x) -> p k x", k=4),
                    axis=ax.X, op=alu.add)
                lnS5 = sb.tile([RPC, 4], dt, tag="lnS5", name="lnS5")
                nc.scalar.activation(lnS5[:], S5[:], fp.Ln)
                lnEd = sb.tile([RPC, 4], dt, tag="lnEd", name="lnEd")
                nc.scalar.activation(lnEd[:], Ed[:], fp.Ln)
                ce4 = sb.tile([RPC, 4], dt, tag="ce4", name="ce4")
                nc.vector.scalar_tensor_tensor(
                    out=ce4[:], in0=lnEd[:], scalar=-(SCALE5 / SCALE1),
                    in1=lnS5[:], op0=alu.mult, op1=alu.add)
                ce_part = sb.tile([RPC, 1], dt, tag="ce_part", name="ce_part")
                nc.vector.tensor_reduce(ce_part[:], ce4[:], axis=ax.X,
                                        op=alu.add)
                dbg = ce_part

            if stage >= 4:
                # p1 = softmax + 1e-12, transposed to [128, 256]
                rS1 = sb.tile([RPC, 4], dt, tag="rS1", name="rS1")
                nc.vector.reciprocal(rS1[:], S1[:])
                p1r = sb.tile([RPC, D], dt, tag="p1r", name="p1r")
                for k in range(4):
                    ksl = slice(k * 128, (k + 1) * 128)
                    nc.vector.tensor_scalar(
                        out=p1r[:, ksl], in0=E1[:, ksl],
                        scalar1=rS1[:, k:k + 1], scalar2=1e-12,
                        op0=alu.mult, op1=alu.add)
                if stage == 4:
                    dbg = sb.tile([128, 1], dt, tag="dbg4", name="dbg4")
                    nc.vector.tensor_copy(dbg[:], p1T[:, 0:1])

            if stage >= 5:
                # fnT / G / cost normalization (overlaps the softmax phase;
                # the K exponentials stay later so they don't delay E1/E2
                # on the ACT engine).
                fnT = []
                for q in range(4):
                    pt = pst.tile([128, 128], dt, tag="pt", name="ptf")
                    nc.tensor.transpose(pt[:], fn[:, q * 128:(q + 1) * 128],
                                        I[:])
                    fnTq = sb.tile([128, 128], dtb, tag=f"fnT{q}",
                                   name=f"fnT{q}")
                    nc.scalar.copy(fnTq[:], pt[:])
                    fnT.append(fnTq)
                pG = psb.tile([128, 128], dt, tag="big", name="pG")
                for q in range(4):
                    nc.tensor.matmul(pG[:], fnT[q][:], fnT[q][:],
                                     start=(q == 0), stop=(q == 3))
                gmax = sb.tile([128, 1], dt, tag="gmax", name="gmax")
                gmin = sb.tile([128, 1], dt, tag="gmin", name="gmin")
                nc.vector.tensor_reduce(gmax[:], pG[:], axis=ax.X, op=alu.max)
                nc.vector.tensor_reduce(gmin[:], pG[:], axis=ax.X, op=alu.min)
                den = sb.tile([128, 1], dt, tag="den", name="den")
                nc.vector.tensor_sub(den[:], gmax[:], gmin[:])
                rden = sb.tile([128, 1], dt, tag="rden", name="rden")
                nc.vector.reciprocal(rden[:], den[:])
                sA = sb.tile([128, 1], dt, tag="sA", name="sA")
                nc.vector.tensor_scalar_mul(sA[:], rden[:], -GAMMA)
                sB = sb.tile([128, 1], dt, tag="sB", name="sB")
                nc.vector.tensor_scalar(
                    out=sB[:], in0=gmax[:], scalar1=rden[:, 0:1],
                    scalar2=GAMMA, op0=alu.mult, op1=alu.mult)
                costm = sb.tile([128, 128], dt, tag="costm", name="costm")
                nc.vector.tensor_scalar(
                    out=costm[:], in0=pG[:], scalar1=sA[:, 0:1],
                    scalar2=sB[:, 0:1], op0=alu.mult, op1=alu.add)
                nc.vector.tensor_add(costm[:], costm[:], I[:])

                # K / K2 / KC (exps kept after the softmax phase)
                K = sb.tile([128, 128], dt, tag="K", name="K")
                nc.scalar.activation(K[:], costm[:], fp.Exp, scale=-2.0)
                ln128t = sb.tile([128, 1], dt, tag="ln128t", name="ln128t")
                nc.vector.memset(ln128t[:], LN128)
                K2 = sb.tile([128, 128], dtb, tag="K2", name="K2")
                nc.scalar.activation(K2[:], costm[:], fp.Exp,
                                     bias=ln128t[:, 0:1], scale=-2.0)
                ptK = pst.tile([128, 128], dt, tag="pt", name="ptK")
                nc.tensor.transpose(ptK[:], costm[:], I[:])
                costmT = sb.tile([128, 128], dt, tag="costmT", name="costmT")
                nc.vector.tensor_copy(costmT[:], ptK[:])
                KT = sb.tile([128, 128], dtb, tag="KT", name="KT")
                nc.scalar.activation(KT[:], costmT[:], fp.Exp, scale=-2.0)
                KC = sb.tile([128, 128], dtb, tag="KC", name="KC")
                nc.gpsimd.tensor_mul(KC[:], K[:], costm[:])

                p1T = sb.tile([128, MPC], dtb, tag="p1T", name="p1T")
                for k in range(4):
                    pt = pst.tile([128, RPC], dt, tag="pt", name="ptp")
                    nc.tensor.transpose(pt[:], p1r[:, k * 128:(k + 1) * 128],
                                        I[:RPC, :RPC])
                    if k % 2 == 0:
                        nc.vector.tensor_copy(
                            p1T[:, k * RPC:(k + 1) * RPC], pt[:])
                    else:
                        nc.scalar.copy(p1T[:, k * RPC:(k + 1) * RPC], pt[:])
                if stage == 5:
                    dbg = sb.tile([128, 1], dt, tag="dbg5", name="dbg5")
                    nc.vector.tensor_copy(dbg[:], K[:, 0:1])

            if stage >= 6:
                # Sinkhorn loop: two independent 128-problem chains so
                # PE / DVE / GpSimd pipeline across chains.
                HB = MPC // 2
                _c = _RAFC
                bs = []
                for h in range(2):
                    bh = sb.tile([128, HB], dtb, tag=f"b0{h}", name=f"b0{h}")
                    nc.vector.memset(bh[:], 1.0)
                    bs.append(bh)
                As = [None, None]
                pws = [None, None]
                for it in range(SINK_ITR):
                    pys = []
                    for h in range(2):
                        py = psb.tile([128, HB], dt, tag="big",
                                      name=f"py{it}{h}")
                        nc.tensor.matmul(py[:], KT[:], bs[h][:],
                                         start=True, stop=True)
                        pys.append(py)
                    rs = []
                    for h in range(2):
                        r = scr.tile([128, HB], dt, tag=f"r{h}",
                                     name=f"r{it}{h}")
                        nc.vector.reciprocal_approx_fast(out=r[:],
                                                         in_=pys[h][:])
                        rs.append(r)
                    for h in range(2):
                        a = scr.tile([128, HB], dtb, tag=f"a{h}",
                                     name=f"a{it}{h}")
                        eng = nc.vector if h == 0 else nc.gpsimd
                        eng.tensor_mul(a[:], p1T[:, h * HB:(h + 1) * HB],
                                       rs[h][:])
                        As[h] = a
                    if it == SINK_ITR - 1:
                        for h in range(2):
                            pw = psb.tile([128, HB], dt, tag="big",
                                          name=f"pw{h}")
                            nc.tensor.matmul(pw[:], KC[:], As[h][:],
                                             start=True, stop=True)
                            pws[h] = pw
                    pzs = []
                    for h in range(2):
                        pz = psb.tile([128, HB], dt, tag="big",
                                      name=f"pz{it}{h}")
                        nc.tensor.matmul(pz[:], K2[:], As[h][:],
                                         start=True, stop=True)
                        pzs.append(pz)
                    bs = []
                    for h in range(2):
                        bh = scr.tile([128, HB], dtb, tag=f"b{h}",
                                      name=f"b{it}{h}")
                        nc.vector._custom_dve(_RAF, out=bh[:], in0=pzs[h][:],
                                              s0=_c["s0"], s1=_c["s1"],
                                              imm2=_c["imm2"])
                        bs.append(bh)
                if stage == 6:
                    dbg = sb.tile([128, 1], dt, tag="dbg6", name="dbg6")
                    nc.vector.tensor_copy(dbg[:], bs[0][:, 0:1])

            if stage >= 7:
                # wcp epilogue (pw computed inside the loop's last iter)
                wcp_part = sb.tile([128, 1], dt, tag="wcp_part",
                                   name="wcp_part")
                wp = []
                for h in range(2):
                    scrW = scr.tile([128, HB], dt, tag=f"r{h}",
                                    name=f"scrW{h}")
                    nc.vector.tensor_mul(scrW[:], pws[h][:], bs[h][:])
                    wph = sb.tile([128, 1], dt, tag=f"wcp{h}", name=f"wcp{h}")
                    nc.vector.tensor_reduce(wph[:], scrW[:],
                                            axis=ax.X, op=alu.add)
                    wp.append(wph)
                nc.vector.tensor_add(wcp_part[:], wp[0][:], wp[1][:])

            # ---------------- pack + store ----------------
            outS = sb.tile([128, 2], dt, tag="outS", name="outS")
            nc.vector.memset(outS[:], 0.0)
            if wcp_part is not None:
                nc.vector.tensor_copy(outS[:, 0:1], wcp_part[:])
            elif dbg is not None:
                p = min(dbg.shape[0], 128)
                nc.vector.tensor_copy(outS[0:p, 0:1], dbg[0:p, 0:1])
            if ce_part is not None:
                nc.vector.tensor_copy(outS[0:RPC, 1:2], ce_part[:])
            nc.sync.dma_start(out=outd[:], in_=outS[:])

    nc.compile()
    return nc


def _get_nc(stage=99):
    key = ("nc", stage)
    if key not in _CACHE:
        _CACHE[key] = _build_nc(stage)
    return _CACHE[key]


def _make_in_maps(features):
    in_maps = []
    for c in range(NCORES):
        maskce = np.zeros((RPC, B), dtype=np.float32)
        off = (c % 2) * 64
        maskce[np.arange(RPC), off + np.arange(RPC)] = 1.0
        in_maps.append({
            "features": features,
            "fslice": np.ascontiguousarray(features[c * RPC:(c + 1) * RPC, :]),
            "maskce": maskce,
        })
    return in_maps


def kernel(features, batch=None, **kwargs):
    from concourse.bass_utils import run_bass_kernel_spmd

    features = np.ascontiguousarray(np.asarray(features, dtype=np.float32))
    assert features.shape == (N, D)

    nc = _get_nc()
    res = run_bass_kernel_spmd(nc, _make_in_maps(features),
                               list(range(NCORES)))

    ce_sum = 0.0
    wcp_sum = 0.0
    for c in range(NCORES):
        o = res.results[c]["out"]
        wcp_sum += float(o[:, 0].sum(dtype=np.float64))
        ce_sum += float(o[:RPC, 1].sum(dtype=np.float64))
    loss = ce_sum / M_TOT + wcp_sum / M_TOT
    return np.float32(loss)


if __name__ == "__main__":
    x = np.random.randn(N, D).astype(np.float32)
    print(kernel(x, B))


# revision 29
# speedup vs baseline: 1.6781x; 1.0138x over previous
"""Trainium2 Bass kernel for the CPN/WCP loss (ce + Sinkhorn wcp).

Strategy:
  - M = 2048 Sinkhorn problems sharded 256/core over 8 cores.
  - Per core: compute its 64-row slab of the NxN (-eudis)/2 matrix via PE
    matmuls (rank-1 matmul folds in the -0.5*sq_j term; the per-row sq_i
    shift is dropped -- softmax/log-softmax are shift invariant).
  - CE pieces (row LSE at temp 5, target logit) computed in row layout.
  - Softmax p1 computed in row layout, transposed to [128 class, 256 prob]
    via PE transposes.
  - Sinkhorn runs in multiplicative form: a = p1 / (K@b), b = p2 / (K^T@a)
    with K = exp(-2*cost) fixed => two matmuls + DVE approx-reciprocals per
    iteration, no transcendentals in the loop.
  - wcp_m = ((K.C)^T a) . b ; per-partition partials DMA'd out, host sums.
"""

import sys

for _p in ("/opt/trn_rl_repo",):
    if _p not in sys.path:
        sys.path.insert(0, _p)

import numpy as np

AUG = 4
B = 128
D = 512
N = AUG * B          # 512 feature rows
NCORES = 8
RPC = N // NCORES    # 64 eudis rows per core
MPC = RPC * AUG      # 256 sinkhorn problems per core
M_TOT = N * AUG      # 2048
TEMP = 5.0
GAMMA = 0.2
SINK_ITR = 5
SCALE1 = 2.0 / float(np.sqrt(np.float32(D)))  # softmax scale on h
SCALE5 = 2.0 / TEMP                            # CE scale on h
LN128 = float(np.log(128.0))

_CACHE = {}


def _build_nc(stage=99):
    import concourse.bacc as bacc
    import concourse.tile as tile
    import concourse.mybir as mybir
    from concourse.dve_ops import (RECIP_APPROX_FAST_CONSTS as _RAFC,
                                   RECIPROCAL_APPROX_FAST as _RAF)

    dt = mybir.dt.float32
    dtr = mybir.dt.float32r
    dtb = mybir.dt.bfloat16
    fp = mybir.ActivationFunctionType
    alu = mybir.AluOpType
    ax = mybir.AxisListType

    nc = bacc.Bacc(
        "TRN2",
        target_bir_lowering=False,
        debug=False,
        enable_asserts=False,
        num_devices=NCORES,
    )

    feat = nc.dram_tensor("features", [N, D], dtr, kind="ExternalInput").ap()
    fsl = nc.dram_tensor("fslice", [RPC, D], dtr, kind="ExternalInput").ap()
    mce = nc.dram_tensor("maskce", [RPC, B], dt, kind="ExternalInput").ap()
    outd = nc.dram_tensor("out", [128, 2], dt, kind="ExternalOutput").ap()

    with tile.TileContext(nc) as tc:
        with (
            tc.tile_pool(name="sb", bufs=1) as sb,
            tc.tile_pool(name="scr", bufs=2) as scr,
            tc.tile_pool(name="ps_big", bufs=3, space="PSUM") as psb,
            tc.tile_pool(name="ps_t", bufs=3, space="PSUM") as pst,
            tc.tile_pool(name="ps_h", bufs=1, space="PSUM") as psh,
        ):
            dbg = None  # [*,1] tile flushed to out col0 for stage bisection

            # Preload the combined exp+ln ACT table set so the compiler's
            # per-func set picker doesn't ping-pong exp_and_others <->
            # natural_log (each reload costs ~2.7us).
            _tabs = list(__import__("concourse.hw_specs",
                                    fromlist=["hw_specs"]
                                    ).get_activation_tables(nc.m.arch))
            _set_id = _tabs.index("natural_log_exp_and_others")
            nc.scalar.add_instruction(mybir.InstLoadActFuncSet(
                name=nc.get_next_instruction_name(), ins=[], outs=[],
                act_func_set_id=_set_id))

            # ---------------- loads ----------------
            # identity generated on-chip (a [128,128] DMA costs ~4us of
            # descriptor processing); F tiles split into halves across the
            # 3 DMA-issuing engines so the first tiles land early.
            ones_t = sb.tile([128, 128], dt, tag="ones_t", name="ones_t")
            nc.vector.memset(ones_t[:], 1.0)
            I = sb.tile([128, 128], dt, tag="I", name="I")
            nc.gpsimd.affine_select(I[:], ones_t[:], [[1, 128]],
                                    alu.is_equal, 0.0, base=0,
                                    channel_multiplier=-1)
            I_r = sb.tile([128, 128], dtr, tag="I_r", name="I_r")
            nc.vector.tensor_copy(I_r[:], I[:])
            F = []
            for t in range(4):
                Ft = sb.tile([128, D], dtr, tag=f"F{t}", name=f"F{t}")
                F.append(Ft)
            halves = [(0, 0, nc.sync), (0, 1, nc.gpsimd), (1, 0, nc.scalar),
                      (1, 1, nc.sync), (2, 0, nc.gpsimd), (2, 1, nc.scalar),
                      (3, 0, nc.sync), (3, 1, nc.gpsimd)]
            for t, h, eng in halves:
                eng.dma_start(
                    out=F[t][h * 64:(h + 1) * 64, :],
                    in_=feat[t * 128 + h * 64:t * 128 + (h + 1) * 64, :])
            fs = sb.tile([RPC, D], dtr, tag="fs", name="fs")
            nc.scalar.dma_start(out=fs[:], in_=fsl[:])
            mk = sb.tile([RPC, B], dt, tag="mk", name="mk")
            nc.gpsimd.dma_start(out=mk[:], in_=mce[:])

            ce_part = None
            wcp_part = None

            if stage >= 1:
                # ---------------- F^T tiles ----------------
                FT = []
                for q in range(4):
                    FTq = sb.tile([128, D], dtr, tag=f"FT{q}", name=f"FT{q}")
                    FT.append(FTq)
                for t in range(4):
                    for q in range(4):
                        pt = pst.tile([128, 128], dt, tag="pt", name="pt")
                        nc.tensor.transpose(
                            pt[:].bitcast(dtr),
                            F[t][:, q * 128:(q + 1) * 128], I_r[:])
                        nc.vector.tensor_copy(
                            FT[q][:, t * 128:(t + 1) * 128], pt[:])

                fsT = []
                for q in range(4):
                    pt = pst.tile([128, RPC], dt, tag="pt", name="pt")
                    nc.tensor.transpose(
                        pt[:].bitcast(dtr),
                        fs[:, q * 128:(q + 1) * 128], I_r[:RPC, :RPC])
                    fsTq = sb.tile([128, RPC], dtr, tag=f"fsT{q}",
                                   name=f"fsT{q}")
                    nc.vector.tensor_copy(fsTq[:], pt[:])
                    fsT.append(fsTq)

                # sq_j row: -0.5 * sum_d F[j,:]^2
                sqc = sb.tile([128, 4], dt, tag="sqc", name="sqc")
                for t in range(4):
                    scrF = scr.tile([128, D], dt, tag="scrF", name="scrF")
                    nc.scalar.activation(scrF[:], F[t][:], fp.Square,
                                         accum_out=sqc[:, t:t + 1])
                sqc2 = sb.tile([128, 4], dtr, tag="sqc2", name="sqc2")
                nc.vector.tensor_scalar_mul(sqc2[:], sqc[:], -0.5)

                # mean-feature branch (gpsimd: off the DVE critical path)
                g = sb.tile([128, D], dt, tag="g", name="g")
                g2 = sb.tile([128, D], dt, tag="g2", name="g2")
                nc.gpsimd.tensor_add(g2[:], F[0][:], F[1][:])
                nc.gpsimd.tensor_add(g[:], F[2][:], F[3][:])
                nc.gpsimd.tensor_add(g[:], g[:], g2[:])
                gsq = scr.tile([128, D], dt, tag="scrF", name="gsq")
                ssg = sb.tile([128, 1], dt, tag="ssg", name="ssg")
                nc.scalar.activation(gsq[:], g[:], fp.Square,
                                     accum_out=ssg[:])
                lssg = sb.tile([128, 1], dt, tag="lssg", name="lssg")
                nc.scalar.activation(lssg[:], ssg[:], fp.Ln)
                rn = sb.tile([128, 1], dt, tag="rn", name="rn")
                nc.scalar.activation(rn[:], lssg[:], fp.Exp, scale=-0.5)
                fn = sb.tile([128, D], dt, tag="fn", name="fn")
                nc.vector.tensor_scalar_mul(fn[:], g[:], rn[:, 0:1])
                dbg = sqc

            if stage >= 2:
                # dist slab: h2 = dot - 0.5*sq_j  [64, 512]
                ph = psh.tile([RPC, D], dt, tag="ph", name="ph")
                for q in range(4):
                    nc.tensor.matmul(ph[:], fsT[q][:], FT[q][:],
                                     start=(q == 0), stop=False)
                # -0.5*sq_j via broadcast-lhsT against identity:
                # out[i,j'] = sum_k sqc2[k,t]*I[k,j'] = sqc2[j',t]
                for t in range(4):
                    nc.tensor.matmul(
                        ph[:, t * 128:(t + 1) * 128],
                        sqc2[:, t:t + 1].to_broadcast((128, RPC)),
                        I_r[:], start=False, stop=(t == 3))


                if stage == 2:
                    dbg = sb.tile([RPC, 1], dt, tag="dbg2", name="dbg2")
                    nc.vector.tensor_copy(dbg[:], ph[:, 0:1])

            if stage >= 3:
                # row stats / CE
                mh = sb.tile([RPC, 4], dt, tag="mh", name="mh")
                for k in range(4):
                    nc.vector.tensor_reduce(
                        mh[:, k:k + 1], ph[:, k * 128:(k + 1) * 128],
                        axis=ax.X, op=alu.max)
                bias1 = sb.tile([RPC, 4], dt, tag="bias1", name="bias1")
                nc.vector.tensor_scalar_mul(bias1[:], mh[:], -SCALE1)
                bias5 = sb.tile([RPC, 4], dt, tag="bias5", name="bias5")
                nc.vector.tensor_scalar_mul(bias5[:], mh[:], -SCALE5)

                E1 = sb.tile([RPC, D], dt, tag="E1", name="E1")
                for k in range(4):
                    ksl = slice(k * 128, (k + 1) * 128)
                    nc.scalar.activation(E1[:, ksl], ph[:, ksl], fp.Exp,
                                         bias=bias1[:, k:k + 1], scale=SCALE1)
                S1 = sb.tile([RPC, 4], dt, tag="S1", name="S1")
                nc.vector.tensor_reduce(
                    S1[:], E1[:].rearrange("p (k x) -> p k x", k=4),
                    axis=ax.X, op=alu.add)
                rS1 = sb.tile([RPC, 4], dt, tag="rS1", name="rS1")
                nc.vector.reciprocal(rS1[:], S1[:])
                p1r = sb.tile([RPC, D], dt, tag="p1r", name="p1r")
                for k in range(4):
                    ksl = slice(k * 128, (k + 1) * 128)
                    nc.vector.tensor_scalar(
                        out=p1r[:, ksl], in0=E1[:, ksl],
                        scalar1=rS1[:, k:k + 1], scalar2=1e-12,
                        op0=alu.mult, op1=alu.add)
                dbg = ce_part

            if stage >= 4:
                pass
                if stage == 4:
                    dbg = sb.tile([128, 1], dt, tag="dbg4", name="dbg4")
                    nc.vector.tensor_copy(dbg[:], p1T[:, 0:1])

            if stage >= 5:
                # fnT / G / cost normalization (overlaps the softmax phase;
                # the K exponentials stay later so they don't delay E1/E2
                # on the ACT engine).
                fnT = []
                for q in range(4):
                    pt = pst.tile([128, 128], dt, tag="pt", name="ptf")
                    nc.tensor.transpose(pt[:], fn[:, q * 128:(q + 1) * 128],
                                        I[:])
                    fnTq = sb.tile([128, 128], dtb, tag=f"fnT{q}",
                                   name=f"fnT{q}")
                    if q % 2 == 0:
                        nc.scalar.copy(fnTq[:], pt[:])
                    else:
                        nc.vector.tensor_copy(fnTq[:], pt[:])
                    fnT.append(fnTq)
                pG = psb.tile([128, 128], dt, tag="big", name="pG")
                for q in range(4):
                    nc.tensor.matmul(pG[:], fnT[q][:], fnT[q][:],
                                     start=(q == 0), stop=(q == 3))
                gmax = sb.tile([128, 1], dt, tag="gmax", name="gmax")
                gmin = sb.tile([128, 1], dt, tag="gmin", name="gmin")
                nc.vector.tensor_reduce(gmax[:], pG[:], axis=ax.X, op=alu.max)
                nc.vector.tensor_reduce(gmin[:], pG[:], axis=ax.X, op=alu.min)
                den = sb.tile([128, 1], dt, tag="den", name="den")
                nc.vector.tensor_sub(den[:], gmax[:], gmin[:])
                rden = sb.tile([128, 1], dt, tag="rden", name="rden")
                nc.vector.reciprocal(rden[:], den[:])
                sA = sb.tile([128, 1], dt, tag="sA", name="sA")
                nc.vector.tensor_scalar_mul(sA[:], rden[:], -GAMMA)
                sB = sb.tile([128, 1], dt, tag="sB", name="sB")
                nc.vector.tensor_scalar(
                    out=sB[:], in0=gmax[:], scalar1=rden[:, 0:1],
                    scalar2=GAMMA, op0=alu.mult, op1=alu.mult)
                costm = sb.tile([128, 128], dt, tag="costm", name="costm")
                nc.vector.tensor_scalar(
                    out=costm[:], in0=pG[:], scalar1=sA[:, 0:1],
                    scalar2=sB[:, 0:1], op0=alu.mult, op1=alu.add)
                nc.vector.tensor_add(costm[:], costm[:], I[:])

                # KT / K2 (gate the loop -> early); K/KC deferred.
                ln128t = sb.tile([128, 1], dt, tag="ln128t", name="ln128t")
                nc.vector.memset(ln128t[:], LN128)
                ptK = pst.tile([128, 128], dt, tag="pt", name="ptK")
                nc.tensor.transpose(ptK[:], costm[:], I[:])
                costmT = sb.tile([128, 128], dt, tag="costmT", name="costmT")
                nc.vector.tensor_copy(costmT[:], ptK[:])
                KT = sb.tile([128, 128], dtb, tag="KT", name="KT")
                nc.scalar.activation(KT[:], costmT[:], fp.Exp, scale=-2.0)
                K2 = sb.tile([128, 128], dtb, tag="K2", name="K2")
                nc.scalar.activation(K2[:], costm[:], fp.Exp,
                                     bias=ln128t[:, 0:1], scale=-2.0)

                p1T = sb.tile([128, MPC], dtb, tag="p1T", name="p1T")
                for k in range(4):
                    pt = pst.tile([128, RPC], dt, tag="pt", name="ptp")
                    nc.tensor.transpose(pt[:], p1r[:, k * 128:(k + 1) * 128],
                                        I[:RPC, :RPC])
                    if k % 2 == 0:
                        nc.vector.tensor_copy(
                            p1T[:, k * RPC:(k + 1) * RPC], pt[:])
                    else:
                        nc.scalar.copy(p1T[:, k * RPC:(k + 1) * RPC], pt[:])

                # deferred CE path (E2/S5/diag) + K/KC for the wcp epilogue;
                # none of this gates the Sinkhorn loop.
                E2 = sb.tile([RPC, D], dt, tag="E2", name="E2")
                for k in range(4):
                    ksl = slice(k * 128, (k + 1) * 128)
                    nc.scalar.activation(E2[:, ksl], ph[:, ksl], fp.Exp,
                                         bias=bias5[:, k:k + 1], scale=SCALE5)
                S5 = sb.tile([RPC, 4], dt, tag="S5", name="S5")
                nc.vector.tensor_reduce(
                    S5[:], E2[:].rearrange("p (k x) -> p k x", k=4),
                    axis=ax.X, op=alu.add)
                E1m = scr.tile([RPC, D], dt, tag="scrE", name="E1m")
                for k in range(4):
                    ksl = slice(k * 128, (k + 1) * 128)
                    nc.gpsimd.tensor_mul(E1m[:, ksl], E1[:, ksl], mk[:])
                Ed = sb.tile([RPC, 4], dt, tag="Ed", name="Ed")
                nc.vector.tensor_reduce(
                    Ed[:], E1m[:].rearrange("p (k x) -> p k x", k=4),
                    axis=ax.X, op=alu.add)
                lnS5 = sb.tile([RPC, 4], dt, tag="lnS5", name="lnS5")
                nc.scalar.activation(lnS5[:], S5[:], fp.Ln)
                lnEd = sb.tile([RPC, 4], dt, tag="lnEd", name="lnEd")
                nc.scalar.activation(lnEd[:], Ed[:], fp.Ln)
                ce4 = sb.tile([RPC, 4], dt, tag="ce4", name="ce4")
                nc.vector.scalar_tensor_tensor(
                    out=ce4[:], in0=lnEd[:], scalar=-(SCALE5 / SCALE1),
                    in1=lnS5[:], op0=alu.mult, op1=alu.add)
                ce_part = sb.tile([RPC, 1], dt, tag="ce_part", name="ce_part")
                nc.vector.tensor_reduce(ce_part[:], ce4[:], axis=ax.X,
                                        op=alu.add)
                K = sb.tile([128, 128], dt, tag="K", name="K")
                nc.scalar.activation(K[:], costm[:], fp.Exp, scale=-2.0)
                KC = sb.tile([128, 128], dtb, tag="KC", name="KC")
                nc.gpsimd.tensor_mul(KC[:], K[:], costm[:])
                if stage == 5:
                    dbg = sb.tile([128, 1], dt, tag="dbg5", name="dbg5")
                    nc.vector.tensor_copy(dbg[:], K[:, 0:1])

            if stage >= 6:
                # Sinkhorn loop: two independent 128-problem chains so
                # PE / DVE / GpSimd pipeline across chains.
                HB = MPC // 2
                _c = _RAFC
                bs = []
                for h in range(2):
                    bh = sb.tile([128, HB], dtb, tag=f"b0{h}", name=f"b0{h}")
                    nc.vector.memset(bh[:], 1.0)
                    bs.append(bh)
                As = [None, None]
                pws = [None, None]
                for it in range(SINK_ITR):
                    pys = []
                    for h in range(2):
                        py = psb.tile([128, HB], dt, tag="big",
                                      name=f"py{it}{h}")
                        nc.tensor.matmul(py[:], KT[:], bs[h][:],
                                         start=True, stop=True)
                        pys.append(py)
                    rs = []
                    for h in range(2):
                        r = scr.tile([128, HB], dt, tag=f"r{h}",
                                     name=f"r{it}{h}")
                        nc.vector.reciprocal_approx_fast(out=r[:],
                                                         in_=pys[h][:])
                        rs.append(r)
                    for h in range(2):
                        a = scr.tile([128, HB], dtb, tag=f"a{h}",
                                     name=f"a{it}{h}")
                        eng = nc.vector if h == 0 else nc.gpsimd
                        eng.tensor_mul(a[:], p1T[:, h * HB:(h + 1) * HB],
                                       rs[h][:])
                        As[h] = a
                    if it == SINK_ITR - 1:
                        for h in range(2):
                            pw = psb.tile([128, HB], dt, tag="big",
                                          name=f"pw{h}")
                            nc.tensor.matmul(pw[:], KC[:], As[h][:],
                                             start=True, stop=True)
                            pws[h] = pw
                    pzs = []
                    for h in range(2):
                        pz = psb.tile([128, HB], dt, tag="big",
                                      name=f"pz{it}{h}")
                        nc.tensor.matmul(pz[:], K2[:], As[h][:],
                                         start=True, stop=True)
                        pzs.append(pz)
                    bs = []
                    for h in range(2):
                        bh = scr.tile([128, HB], dtb, tag=f"b{h}",
                                      name=f"b{it}{h}")
                        nc.vector._custom_dve(_RAF, out=bh[:], in0=pzs[h][:],
                                              s0=_c["s0"], s1=_c["s1"],
                                              imm2=_c["imm2"])
                        bs.append(bh)
                if stage == 6:
                    dbg = sb.tile([128, 1], dt, tag="dbg6", name="dbg6")
                    nc.vector.tensor_copy(dbg[:], bs[0][:, 0:1])

            if stage >= 7:
                # wcp epilogue (pw computed inside the loop's last iter)
                wcp_part = sb.tile([128, 1], dt, tag="wcp_part",
                                   name="wcp_part")
                wp = []
                for h in range(2):
                    scrW = scr.tile([128, HB], dt, tag=f"r{h}",
                                    name=f"scrW{h}")
                    nc.vector.tensor_mul(scrW[:], pws[h][:], bs[h][:])
                    wph = sb.tile([128, 1], dt, tag=f"wcp{h}", name=f"wcp{h}")
                    nc.vector.tensor_reduce(wph[:], scrW[:],
                                            axis=ax.X, op=alu.add)
                    wp.append(wph)
                nc.vector.tensor_add(wcp_part[:], wp[0][:], wp[1][:])

            # ---------------- pack + store ----------------
            outS = sb.tile([128, 2], dt, tag="outS", name="outS")
            nc.vector.memset(outS[:], 0.0)
            if wcp_part is not None:
                nc.vector.tensor_copy(outS[:, 0:1], wcp_part[:])
            elif dbg is not None:
                p = min(dbg.shape[0], 128)
                nc.vector.tensor_copy(outS[0:p, 0:1], dbg[0:p, 0:1])
            if ce_part is not None:
                nc.vector.tensor_copy(outS[0:RPC, 1:2], ce_part[:])
            nc.sync.dma_start(out=outd[:], in_=outS[:])

    nc.compile()
    return nc


def _get_nc(stage=99):
    key = ("nc", stage)
    if key not in _CACHE:
        _CACHE[key] = _build_nc(stage)
    return _CACHE[key]


def _make_in_maps(features):
    in_maps = []
    for c in range(NCORES):
        maskce = np.zeros((RPC, B), dtype=np.float32)
        off = (c % 2) * 64
        maskce[np.arange(RPC), off + np.arange(RPC)] = 1.0
        in_maps.append({
            "features": features,
            "fslice": np.ascontiguousarray(features[c * RPC:(c + 1) * RPC, :]),
            "maskce": maskce,
        })
    return in_maps


def kernel(features, batch=None, **kwargs):
    from concourse.bass_utils import run_bass_kernel_spmd

    features = np.ascontiguousarray(np.asarray(features, dtype=np.float32))
    assert features.shape == (N, D)

    nc = _get_nc()
    res = run_bass_kernel_spmd(nc, _make_in_maps(features),
                               list(range(NCORES)))

    ce_sum = 0.0
    wcp_sum = 0.0
    for c in range(NCORES):
        o = res.results[c]["out"]
        wcp_sum += float(o[:, 0].sum(dtype=np.float64))
        ce_sum += float(o[:RPC, 1].sum(dtype=np.float64))
    loss = ce_sum / M_TOT + wcp_sum / M_TOT
    return np.float32(loss)


if __name__ == "__main__":
    x = np.random.randn(N, D).astype(np.float32)
    print(kernel(x, B))


# revision 30
# speedup vs baseline: 1.6828x; 1.0028x over previous
"""Trainium2 Bass kernel for the CPN/WCP loss (ce + Sinkhorn wcp).

Strategy:
  - M = 2048 Sinkhorn problems sharded 256/core over 8 cores.
  - Per core: compute its 64-row slab of the NxN (-eudis)/2 matrix via PE
    matmuls (rank-1 matmul folds in the -0.5*sq_j term; the per-row sq_i
    shift is dropped -- softmax/log-softmax are shift invariant).
  - CE pieces (row LSE at temp 5, target logit) computed in row layout.
  - Softmax p1 computed in row layout, transposed to [128 class, 256 prob]
    via PE transposes.
  - Sinkhorn runs in multiplicative form: a = p1 / (K@b), b = p2 / (K^T@a)
    with K = exp(-2*cost) fixed => two matmuls + DVE approx-reciprocals per
    iteration, no transcendentals in the loop.
  - wcp_m = ((K.C)^T a) . b ; per-partition partials DMA'd out, host sums.
"""

import sys

for _p in ("/opt/trn_rl_repo",):
    if _p not in sys.path:
        sys.path.insert(0, _p)

import numpy as np

AUG = 4
B = 128
D = 512
N = AUG * B          # 512 feature rows
NCORES = 8
RPC = N // NCORES    # 64 eudis rows per core
MPC = RPC * AUG      # 256 sinkhorn problems per core
M_TOT = N * AUG      # 2048
TEMP = 5.0
GAMMA = 0.2
SINK_ITR = 5
SCALE1 = 2.0 / float(np.sqrt(np.float32(D)))  # softmax scale on h
SCALE5 = 2.0 / TEMP                            # CE scale on h
LN128 = float(np.log(128.0))

_CACHE = {}


def _build_nc(stage=99):
    import concourse.bacc as bacc
    import concourse.tile as tile
    import concourse.mybir as mybir
    from concourse.dve_ops import (RECIP_APPROX_FAST_CONSTS as _RAFC,
                                   RECIPROCAL_APPROX_FAST as _RAF)

    dt = mybir.dt.float32
    dtr = mybir.dt.float32r
    dtb = mybir.dt.bfloat16
    fp = mybir.ActivationFunctionType
    alu = mybir.AluOpType
    ax = mybir.AxisListType

    nc = bacc.Bacc(
        "TRN2",
        target_bir_lowering=False,
        debug=False,
        enable_asserts=False,
        num_devices=NCORES,
    )

    feat = nc.dram_tensor("features", [N, D], dtr, kind="ExternalInput").ap()
    fsl = nc.dram_tensor("fslice", [RPC, D], dtr, kind="ExternalInput").ap()
    mce = nc.dram_tensor("maskce", [RPC, B], dt, kind="ExternalInput").ap()
    outd = nc.dram_tensor("out", [128, 2], dt, kind="ExternalOutput").ap()

    with tile.TileContext(nc) as tc:
        with (
            tc.tile_pool(name="sb", bufs=1) as sb,
            tc.tile_pool(name="scr", bufs=2) as scr,
            tc.tile_pool(name="ps_big", bufs=3, space="PSUM") as psb,
            tc.tile_pool(name="ps_t", bufs=3, space="PSUM") as pst,
            tc.tile_pool(name="ps_h", bufs=1, space="PSUM") as psh,
        ):
            dbg = None  # [*,1] tile flushed to out col0 for stage bisection

            # Preload the combined exp+ln ACT table set so the compiler's
            # per-func set picker doesn't ping-pong exp_and_others <->
            # natural_log (each reload costs ~2.7us).
            _tabs = list(__import__("concourse.hw_specs",
                                    fromlist=["hw_specs"]
                                    ).get_activation_tables(nc.m.arch))
            _set_id = _tabs.index("natural_log_exp_and_others")
            nc.scalar.add_instruction(mybir.InstLoadActFuncSet(
                name=nc.get_next_instruction_name(), ins=[], outs=[],
                act_func_set_id=_set_id))

            # ---------------- loads ----------------
            # identity generated on-chip (a [128,128] DMA costs ~4us of
            # descriptor processing); F tiles split into halves across the
            # 3 DMA-issuing engines so the first tiles land early.
            ones_t = sb.tile([128, 128], dt, tag="ones_t", name="ones_t")
            nc.vector.memset(ones_t[:], 1.0)
            I = sb.tile([128, 128], dt, tag="I", name="I")
            nc.gpsimd.affine_select(I[:], ones_t[:], [[1, 128]],
                                    alu.is_equal, 0.0, base=0,
                                    channel_multiplier=-1)
            I_r = sb.tile([128, 128], dtr, tag="I_r", name="I_r")
            nc.vector.tensor_copy(I_r[:], I[:])
            F = []
            for t in range(4):
                Ft = sb.tile([128, D], dtr, tag=f"F{t}", name=f"F{t}")
                F.append(Ft)
            halves = [(0, 0, nc.sync), (0, 1, nc.gpsimd), (1, 0, nc.scalar),
                      (1, 1, nc.sync), (2, 0, nc.gpsimd), (2, 1, nc.scalar),
                      (3, 0, nc.sync), (3, 1, nc.gpsimd)]
            for t, h, eng in halves:
                eng.dma_start(
                    out=F[t][h * 64:(h + 1) * 64, :],
                    in_=feat[t * 128 + h * 64:t * 128 + (h + 1) * 64, :])
            fs = sb.tile([RPC, D], dtr, tag="fs", name="fs")
            nc.scalar.dma_start(out=fs[:], in_=fsl[:])
            mk = sb.tile([RPC, B], dt, tag="mk", name="mk")
            nc.gpsimd.dma_start(out=mk[:], in_=mce[:])

            ce_part = None
            wcp_part = None

            if stage >= 1:
                # ---------------- F^T tiles ----------------
                FT = []
                for q in range(4):
                    FTq = sb.tile([128, D], dtr, tag=f"FT{q}", name=f"FT{q}")
                    FT.append(FTq)
                for t in range(4):
                    for q in range(4):
                        pt = pst.tile([128, 128], dt, tag="pt", name="pt")
                        nc.tensor.transpose(
                            pt[:].bitcast(dtr),
                            F[t][:, q * 128:(q + 1) * 128], I_r[:])
                        nc.vector.tensor_copy(
                            FT[q][:, t * 128:(t + 1) * 128], pt[:])

                fsT = []
                for q in range(4):
                    pt = pst.tile([128, RPC], dt, tag="pt", name="pt")
                    nc.tensor.transpose(
                        pt[:].bitcast(dtr),
                        fs[:, q * 128:(q + 1) * 128], I_r[:RPC, :RPC])
                    fsTq = sb.tile([128, RPC], dtr, tag=f"fsT{q}",
                                   name=f"fsT{q}")
                    nc.vector.tensor_copy(fsTq[:], pt[:])
                    fsT.append(fsTq)

                # sq_j row: -0.5 * sum_d F[j,:]^2
                sqc = sb.tile([128, 4], dt, tag="sqc", name="sqc")
                for t in range(4):
                    scrF = scr.tile([128, D], dt, tag="scrF", name="scrF")
                    nc.scalar.activation(scrF[:], F[t][:], fp.Square,
                                         accum_out=sqc[:, t:t + 1])
                sqc2 = sb.tile([128, 4], dtr, tag="sqc2", name="sqc2")
                nc.vector.tensor_scalar_mul(sqc2[:], sqc[:], -0.5)

                # mean-feature branch (gpsimd: off the DVE critical path)
                g = sb.tile([128, D], dt, tag="g", name="g")
                g2 = sb.tile([128, D], dt, tag="g2", name="g2")
                nc.gpsimd.tensor_add(g2[:], F[0][:], F[1][:])
                nc.gpsimd.tensor_add(g[:], F[2][:], F[3][:])
                nc.gpsimd.tensor_add(g[:], g[:], g2[:])
                gsq = scr.tile([128, D], dt, tag="scrF", name="gsq")
                ssg = sb.tile([128, 1], dt, tag="ssg", name="ssg")
                nc.scalar.activation(gsq[:], g[:], fp.Square,
                                     accum_out=ssg[:])
                lssg = sb.tile([128, 1], dt, tag="lssg", name="lssg")
                nc.scalar.activation(lssg[:], ssg[:], fp.Ln)
                rn = sb.tile([128, 1], dt, tag="rn", name="rn")
                nc.scalar.activation(rn[:], lssg[:], fp.Exp, scale=-0.5)
                fn = sb.tile([128, D], dt, tag="fn", name="fn")
                nc.vector.tensor_scalar_mul(fn[:], g[:], rn[:, 0:1])
                dbg = sqc

            if stage >= 2:
                # dist slab: h2 = dot - 0.5*sq_j  [64, 512]
                ph = psh.tile([RPC, D], dt, tag="ph", name="ph")
                for q in range(4):
                    nc.tensor.matmul(ph[:], fsT[q][:], FT[q][:],
                                     start=(q == 0), stop=False)
                # -0.5*sq_j via broadcast-lhsT against identity:
                # out[i,j'] = sum_k sqc2[k,t]*I[k,j'] = sqc2[j',t]
                for t in range(4):
                    nc.tensor.matmul(
                        ph[:, t * 128:(t + 1) * 128],
                        sqc2[:, t:t + 1].to_broadcast((128, RPC)),
                        I_r[:], start=False, stop=(t == 3))


                if stage == 2:
                    dbg = sb.tile([RPC, 1], dt, tag="dbg2", name="dbg2")
                    nc.vector.tensor_copy(dbg[:], ph[:, 0:1])

            if stage >= 3:
                # row stats / CE
                mh = sb.tile([RPC, 4], dt, tag="mh", name="mh")
                nc.vector.tensor_reduce(
                    mh[:], ph[:].rearrange("p (k x) -> p k x", k=4),
                    axis=ax.X, op=alu.max)
                bias1 = sb.tile([RPC, 4], dt, tag="bias1", name="bias1")
                nc.vector.tensor_scalar_mul(bias1[:], mh[:], -SCALE1)

                E1 = sb.tile([RPC, D], dt, tag="E1", name="E1")
                for k in range(4):
                    ksl = slice(k * 128, (k + 1) * 128)
                    nc.scalar.activation(E1[:, ksl], ph[:, ksl], fp.Exp,
                                         bias=bias1[:, k:k + 1], scale=SCALE1)
                S1 = sb.tile([RPC, 4], dt, tag="S1", name="S1")
                nc.vector.tensor_reduce(
                    S1[:], E1[:].rearrange("p (k x) -> p k x", k=4),
                    axis=ax.X, op=alu.add)
                rS1 = sb.tile([RPC, 4], dt, tag="rS1", name="rS1")
                nc.vector.reciprocal(rS1[:], S1[:])
                p1r = sb.tile([RPC, D], dt, tag="p1r", name="p1r")
                for k in range(4):
                    ksl = slice(k * 128, (k + 1) * 128)
                    eng = nc.vector if k % 2 == 0 else nc.gpsimd
                    eng.tensor_scalar(
                        out=p1r[:, ksl], in0=E1[:, ksl],
                        scalar1=rS1[:, k:k + 1], scalar2=1e-12,
                        op0=alu.mult, op1=alu.add)
                dbg = ce_part

            if stage >= 4:
                pass
                if stage == 4:
                    dbg = sb.tile([128, 1], dt, tag="dbg4", name="dbg4")
                    nc.vector.tensor_copy(dbg[:], p1T[:, 0:1])

            if stage >= 5:
                # fnT / G / cost normalization (overlaps the softmax phase;
                # the K exponentials stay later so they don't delay E1/E2
                # on the ACT engine).
                fnT = []
                for q in range(4):
                    pt = pst.tile([128, 128], dt, tag="pt", name="ptf")
                    nc.tensor.transpose(pt[:], fn[:, q * 128:(q + 1) * 128],
                                        I[:])
                    fnTq = sb.tile([128, 128], dtb, tag=f"fnT{q}",
                                   name=f"fnT{q}")
                    nc.scalar.copy(fnTq[:], pt[:])
                    fnT.append(fnTq)
                pG = psb.tile([128, 128], dt, tag="big", name="pG")
                for q in range(4):
                    nc.tensor.matmul(pG[:], fnT[q][:], fnT[q][:],
                                     start=(q == 0), stop=(q == 3))
                gmax = sb.tile([128, 1], dt, tag="gmax", name="gmax")
                gmin = sb.tile([128, 1], dt, tag="gmin", name="gmin")
                nc.vector.tensor_reduce(gmax[:], pG[:], axis=ax.X, op=alu.max)
                nc.vector.tensor_reduce(gmin[:], pG[:], axis=ax.X, op=alu.min)
                den = sb.tile([128, 1], dt, tag="den", name="den")
                nc.vector.tensor_sub(den[:], gmax[:], gmin[:])
                rden = sb.tile([128, 1], dt, tag="rden", name="rden")
                nc.vector.reciprocal(rden[:], den[:])
                sA = sb.tile([128, 1], dt, tag="sA", name="sA")
                nc.vector.tensor_scalar_mul(sA[:], rden[:], -GAMMA)
                sB = sb.tile([128, 1], dt, tag="sB", name="sB")
                nc.vector.tensor_scalar(
                    out=sB[:], in0=gmax[:], scalar1=rden[:, 0:1],
                    scalar2=GAMMA, op0=alu.mult, op1=alu.mult)
                costm = sb.tile([128, 128], dt, tag="costm", name="costm")
                nc.vector.tensor_scalar(
                    out=costm[:], in0=pG[:], scalar1=sA[:, 0:1],
                    scalar2=sB[:, 0:1], op0=alu.mult, op1=alu.add)
                nc.vector.tensor_add(costm[:], costm[:], I[:])

                # KT / K2 (gate the loop -> early); K/KC deferred.
                ln128t = sb.tile([128, 1], dt, tag="ln128t", name="ln128t")
                nc.vector.memset(ln128t[:], LN128)
                ptK = pst.tile([128, 128], dt, tag="pt", name="ptK")
                nc.tensor.transpose(ptK[:], costm[:], I[:])
                costmT = sb.tile([128, 128], dt, tag="costmT", name="costmT")
                nc.vector.tensor_copy(costmT[:], ptK[:])
                KT = sb.tile([128, 128], dtb, tag="KT", name="KT")
                nc.scalar.activation(KT[:], costmT[:], fp.Exp, scale=-2.0)
                K2 = sb.tile([128, 128], dtb, tag="K2", name="K2")
                nc.scalar.activation(K2[:], costm[:], fp.Exp,
                                     bias=ln128t[:, 0:1], scale=-2.0)

                p1T = sb.tile([128, MPC], dtb, tag="p1T", name="p1T")
                for k in range(4):
                    pt = pst.tile([128, RPC], dt, tag="pt", name="ptp")
                    nc.tensor.transpose(pt[:], p1r[:, k * 128:(k + 1) * 128],
                                        I[:RPC, :RPC])
                    if k % 2 == 0:
                        nc.vector.tensor_copy(
                            p1T[:, k * RPC:(k + 1) * RPC], pt[:])
                    else:
                        nc.scalar.copy(p1T[:, k * RPC:(k + 1) * RPC], pt[:])

                # deferred CE path (E2/S5/diag) + K/KC for the wcp epilogue;
                # none of this gates the Sinkhorn loop.
                bias5 = sb.tile([RPC, 4], dt, tag="bias5", name="bias5")
                nc.vector.tensor_scalar_mul(bias5[:], mh[:], -SCALE5)
                E2 = sb.tile([RPC, D], dt, tag="E2", name="E2")
                for k in range(4):
                    ksl = slice(k * 128, (k + 1) * 128)
                    nc.scalar.activation(E2[:, ksl], ph[:, ksl], fp.Exp,
                                         bias=bias5[:, k:k + 1], scale=SCALE5)
                S5 = sb.tile([RPC, 4], dt, tag="S5", name="S5")
                nc.vector.tensor_reduce(
                    S5[:], E2[:].rearrange("p (k x) -> p k x", k=4),
                    axis=ax.X, op=alu.add)
                E1m = scr.tile([RPC, D], dt, tag="scrE", name="E1m")
                for k in range(4):
                    ksl = slice(k * 128, (k + 1) * 128)
                    nc.gpsimd.tensor_mul(E1m[:, ksl], E1[:, ksl], mk[:])
                Ed = sb.tile([RPC, 4], dt, tag="Ed", name="Ed")
                nc.vector.tensor_reduce(
                    Ed[:], E1m[:].rearrange("p (k x) -> p k x", k=4),
                    axis=ax.X, op=alu.add)
                lnS5 = sb.tile([RPC, 4], dt, tag="lnS5", name="lnS5")
                nc.scalar.activation(lnS5[:], S5[:], fp.Ln)
                lnEd = sb.tile([RPC, 4], dt, tag="lnEd", name="lnEd")
                nc.scalar.activation(lnEd[:], Ed[:], fp.Ln)
                ce4 = sb.tile([RPC, 4], dt, tag="ce4", name="ce4")
                nc.vector.scalar_tensor_tensor(
                    out=ce4[:], in0=lnEd[:], scalar=-(SCALE5 / SCALE1),
                    in1=lnS5[:], op0=alu.mult, op1=alu.add)
                ce_part = sb.tile([RPC, 1], dt, tag="ce_part", name="ce_part")
                nc.vector.tensor_reduce(ce_part[:], ce4[:], axis=ax.X,
                                        op=alu.add)
                K = sb.tile([128, 128], dt, tag="K", name="K")
                nc.scalar.activation(K[:], costm[:], fp.Exp, scale=-2.0)
                KC = sb.tile([128, 128], dtb, tag="KC", name="KC")
                nc.gpsimd.tensor_mul(KC[:], K[:], costm[:])
                if stage == 5:
                    dbg = sb.tile([128, 1], dt, tag="dbg5", name="dbg5")
                    nc.vector.tensor_copy(dbg[:], K[:, 0:1])

            if stage >= 6:
                # Sinkhorn loop: two independent 128-problem chains so
                # PE / DVE / GpSimd pipeline across chains.
                HB = MPC // 2
                _c = _RAFC
                bs = []
                for h in range(2):
                    bh = sb.tile([128, HB], dtb, tag=f"b0{h}", name=f"b0{h}")
                    nc.vector.memset(bh[:], 1.0)
                    bs.append(bh)
                As = [None, None]
                pws = [None, None]
                for it in range(SINK_ITR):
                    pys = []
                    for h in range(2):
                        py = psb.tile([128, HB], dt, tag="big",
                                      name=f"py{it}{h}")
                        nc.tensor.matmul(py[:], KT[:], bs[h][:],
                                         start=True, stop=True)
                        pys.append(py)
                    rs = []
                    for h in range(2):
                        r = scr.tile([128, HB], dt, tag=f"r{h}",
                                     name=f"r{it}{h}")
                        nc.vector.reciprocal_approx_fast(out=r[:],
                                                         in_=pys[h][:])
                        rs.append(r)
                    for h in range(2):
                        a = scr.tile([128, HB], dtb, tag=f"a{h}",
                                     name=f"a{it}{h}")
                        eng = nc.vector if h == 0 else nc.gpsimd
                        eng.tensor_mul(a[:], p1T[:, h * HB:(h + 1) * HB],
                                       rs[h][:])
                        As[h] = a
                    if it == SINK_ITR - 1:
                        for h in range(2):
                            pw = psb.tile([128, HB], dt, tag="big",
                                          name=f"pw{h}")
                            nc.tensor.matmul(pw[:], KC[:], As[h][:],
                                             start=True, stop=True)
                            pws[h] = pw
                    pzs = []
                    for h in range(2):
                        pz = psb.tile([128, HB], dt, tag="big",
                                      name=f"pz{it}{h}")
                        nc.tensor.matmul(pz[:], K2[:], As[h][:],
                                         start=True, stop=True)
                        pzs.append(pz)
                    bs = []
                    for h in range(2):
                        bh = scr.tile([128, HB], dtb, tag=f"b{h}",
                                      name=f"b{it}{h}")
                        nc.vector._custom_dve(_RAF, out=bh[:], in0=pzs[h][:],
                                              s0=_c["s0"], s1=_c["s1"],
                                              imm2=_c["imm2"])
                        bs.append(bh)
                if stage == 6:
                    dbg = sb.tile([128, 1], dt, tag="dbg6", name="dbg6")
                    nc.vector.tensor_copy(dbg[:], bs[0][:, 0:1])

            if stage >= 7:
                # wcp epilogue (pw computed inside the loop's last iter)
                wcp_part = sb.tile([128, 1], dt, tag="wcp_part",
                                   name="wcp_part")
                wp = []
                for h in range(2):
                    scrW = scr.tile([128, HB], dt, tag=f"r{h}",
                                    name=f"scrW{h}")
                    nc.vector.tensor_mul(scrW[:], pws[h][:], bs[h][:])
                    wph = sb.tile([128, 1], dt, tag=f"wcp{h}", name=f"wcp{h}")
                    nc.vector.tensor_reduce(wph[:], scrW[:],
                                            axis=ax.X, op=alu.add)
                    wp.append(wph)
                nc.vector.tensor_add(wcp_part[:], wp[0][:], wp[1][:])

            # ---------------- pack + store ----------------
            outS = sb.tile([128, 2], dt, tag="outS", name="outS")
            nc.vector.memset(outS[:], 0.0)
            if wcp_part is not None:
                nc.vector.tensor_copy(outS[:, 0:1], wcp_part[:])
            elif dbg is not None:
                p = min(dbg.shape[0], 128)
                nc.vector.tensor_copy(outS[0:p, 0:1], dbg[0:p, 0:1])
            if ce_part is not None:
                nc.vector.tensor_copy(outS[0:RPC, 1:2], ce_part[:])
            nc.sync.dma_start(out=outd[:], in_=outS[:])

    nc.compile()
    return nc


def _get_nc(stage=99):
    key = ("nc", stage)
    if key not in _CACHE:
        _CACHE[key] = _build_nc(stage)
    return _CACHE[key]


def _make_in_maps(features):
    in_maps = []
    for c in range(NCORES):
        maskce = np.zeros((RPC, B), dtype=np.float32)
        off = (c % 2) * 64
        maskce[np.arange(RPC), off + np.arange(RPC)] = 1.0
        in_maps.append({
            "features": features,
            "fslice": np.ascontiguousarray(features[c * RPC:(c + 1) * RPC, :]),
            "maskce": maskce,
        })
    return in_maps


def kernel(features, batch=None, **kwargs):
    from concourse.bass_utils import run_bass_kernel_spmd

    features = np.ascontiguousarray(np.asarray(features, dtype=np.float32))
    assert features.shape == (N, D)

    nc = _get_nc()
    res = run_bass_kernel_spmd(nc, _make_in_maps(features),
                               list(range(NCORES)))

    ce_sum = 0.0
    wcp_sum = 0.0
    for c in range(NCORES):
        o = res.results[c]["out"]
        wcp_sum += float(o[:, 0].sum(dtype=np.float64))
        ce_sum += float(o[:RPC, 1].sum(dtype=np.float64))
    loss = ce_sum / M_TOT + wcp_sum / M_TOT
    return np.float32(loss)


if __name__ == "__main__":
    x = np.random.randn(N, D).astype(np.float32)
    print(kernel(x, B))
